# revision 52
# baseline (speedup 1.0000x reference)
"""Trainium2 Bass kernel for nn_EnergyAE (B=64, D=12288, N=32, H=2048) on 8 cores.

Hybrid sharding, bf16 matmuls (fp32 vector math):
  phase E  (model-parallel over H): encoder -> z* partial (bf16, folded into
           C-chunk0 AllReduce below)
  phase C  (contraction-parallel over D): upper-triangular 512-col blocks of
           Cpart = w2Ts @ w2Ts^T, AllReduced in 4 bf16 chunks pipelined with
           the build; mirrored to full C in SBUF locally after readback.
           C stays SBUF-resident for both PG stages.
  phase S1 (data-parallel, 8 samples/core): A1 = W1*m1, P1T = C@A1T,
           G = P1T^T A1T (batched 4 samples/matmul), Prec, LDLT, Lt^-1, dz,
           tr, logdet, z_s = z* + dz -> AllGather z_s (8KB)
  phase S2 (model-parallel over D): h2, x_star slice, delta, d_sq,
           Wd = delta@W2s^T (w2Ts reused from SBUF) -> AllReduce [Wd | d_sq]
  phase S3 (data-parallel): t = W1T^T(m2*Wd), G2 = A2 C A2^T, LDLT2,
           fwd solve, d_proj_sq, recon -> out (8 per core)

Identities replacing eigvalsh/cholesky/solve_triangular:
  Prec = Lt D Lt^T (unit-lower LDLT)
  sum(log eig)/2 = 0.5*sum(log D);   sum(1/eig) = ||D^-1/2 Lt^-1||_F^2
  U^-1 eps = Lt^-T (eps/sqrt(D));    t^T G2^-1 t = ||D2^-1/2 Lt2^-1 t||^2
  sig_term = (n w0 w0^T + (D-n) w1 w1^T)/2   (constant across batch)
"""
import sys

for _p in ("/opt/trn_rl_repo", "/root/.axon_site/_ro/trn_rl_repo"):
    if _p not in sys.path:
        sys.path.append(_p)

import numpy as np
import ml_dtypes
from contextlib import ExitStack

import concourse.bass as bass
import concourse.mybir as mybir
import concourse.tile as tile
from concourse.masks import make_identity

B, D, N, H = 64, 12288, 32, 2048
NCORES = 8
BL = B // NCORES          # 8 local samples
HS = H // NCORES          # 256
DS = D // NCORES          # 1536
KT_H = H // 128           # 16
KT_D = D // 128           # 96
KT_DS = DS // 128         # 12
P = 128
# upper-triangular C slab layout: col-block q holds block-rows 0..4q+3
CUP_OFF = [0, 4, 12, 24]          # slab index offset per col-block
CUP_NROW = [4, 8, 12, 16]         # slabs per col-block
CUP_TOT = 40
ZROWS = 4                         # z* partial occupies cup rows 0..3 (4x512)

F32 = mybir.dt.float32
BF16 = mybir.dt.bfloat16
Alu = mybir.AluOpType
Act = mybir.ActivationFunctionType
RG = [list(range(NCORES))]


def sub_ap(t, extra_off, dims):
    """Custom free-dim AP on a [P, F] tile; dims = [[step,count],...] in elems."""
    base = t[:, 0:1]
    return bass.AP(base.tensor, base.offset + extra_off, [base.ap[0]] + dims)


def pe_T(nc, out_ps, in_ap, ident):
    """PE transpose: out_ps [f, p] = in_ap [p, f].T"""
    kp = in_ap.shape[0]
    nc.tensor.transpose(out_ps, in_ap, ident[0:kp, 0:kp])


def emit_ldlt(nc, T, OUT, invD, n=32):
    """In-place unit-lower LDLT of T [BL, n*n] (row-major per sample).
    After: strict lower of T holds unscaled columns u; diag holds D; invD = 1/D."""
    for j in range(n):
        nc.vector.reciprocal(invD[:, j:j + 1], T[:, (n + 1) * j:(n + 1) * j + 1])
        m = n - 1 - j
        if m == 0:
            break
        base = (j + 1) * n + j
        u_i = sub_ap(T, base, [[n, m], [0, m]])
        u_k = sub_ap(T, base, [[0, m], [n, m]])
        outer = sub_ap(OUT, 0, [[m, m], [1, m]])
        nc.vector.scalar_tensor_tensor(
            outer, u_i, invD[:, j:j + 1], u_k, Alu.mult, Alu.mult)
        trail = sub_ap(T, (j + 1) * (n + 1), [[n, m], [1, m]])
        nc.vector.tensor_tensor(trail, trail, outer, Alu.subtract)


def emit_ltinv(eng, LT, X, OUT, n=32):
    """X = LT^{-1} for unit-lower LT [BL, n*n]; X preset to I by caller.
    Uses only tensor_tensor (runs on GpSimd, whose ISA lacks STT)."""
    for k in range(n - 1):
        rows = n - 1 - k
        cols = k + 1
        lcol = sub_ap(LT, (k + 1) * n + k, [[n, rows], [0, cols]])
        xrow = sub_ap(X, k * n, [[0, rows], [1, cols]])
        prod = sub_ap(OUT, 0, [[cols, rows], [1, cols]])
        eng.tensor_tensor(prod, lcol, xrow, Alu.mult)
        xblk = sub_ap(X, (k + 1) * n, [[n, rows], [1, cols]])
        eng.tensor_tensor(xblk, xblk, prod, Alu.subtract)


def emit_bwd_solve(nc, LT, y, OUT, n=32):
    """y <- LT^{-T} y for unit-lower LT [BL, n*n], y [BL, n] in place."""
    for k in range(n - 1, 0, -1):
        lrow = sub_ap(LT, k * n, [[1, k]])
        nc.vector.scalar_tensor_tensor(
            OUT[:, 0:k], lrow, -1.0, y[:, k:k + 1].broadcast_to([BL, k]),
            Alu.mult, Alu.mult)
        nc.vector.tensor_tensor(y[:, 0:k], y[:, 0:k], OUT[:, 0:k], Alu.add)


def emit_fwd_solve(nc, LT, y, OUT, n=32):
    """y <- LT^{-1} y for unit-lower LT [BL, n*n], y [BL, n] in place."""
    for k in range(n - 1):
        rows = n - 1 - k
        lcol = sub_ap(LT, (k + 1) * n + k, [[n, rows]])
        nc.vector.scalar_tensor_tensor(
            OUT[:, 0:rows], lcol, -1.0, y[:, k:k + 1].broadcast_to([BL, rows]),
            Alu.mult, Alu.mult)
        nc.vector.tensor_tensor(y[:, k + 1:n], y[:, k + 1:n], OUT[:, 0:rows], Alu.add)


def legalize_waits(nc, maxw=1):
    """Split multi-wait sync_info into standalone EventSemaphore instructions."""
    for f in nc.m.functions:
        for bb in f.blocks:
            insts = list(bb.instructions)
            out = []
            changed = False
            for inst in insts:
                si = inst.sync_info
                if si is not None and si.on_wait and len(si.on_wait) > maxw:
                    waits = list(si.on_wait)
                    imm = [w for w in waits if w.uses_immediate]
                    reg = [w for w in waits if not w.uses_immediate]
                    keep = (reg + imm)[:maxw] if len(reg) <= maxw else reg
                    extra = [w for w in waits if w not in keep]
                    if len(keep) > maxw:
                        raise RuntimeError(f"{inst.name}: {len(keep)} register waits")
                    for w in extra:
                        ev = mybir.InstEventSemaphore(
                            name=nc.get_next_instruction_name(), ins=[], outs=[])
                        ev.engine = inst.engine
                        ev.sync_info = mybir.SyncInfo(on_wait=[w], on_update=[])
                        out.append(ev)
                    inst.sync_info = mybir.SyncInfo(
                        on_wait=keep, on_update=list(si.on_update or []))
                    changed = True
                out.append(inst)
            if changed:
                bb.instructions = out
    return nc


def build_nc():
    nc = bass.Bass()

    # ---- I/O ----
    # xTp/w1esp: pre-packed partition-major [(p) (k b)] so DMA descriptors are
    # per-partition contiguous (12KB / 6KB) instead of 128B/512B strided
    xTp = nc.dram_tensor("xTp", [P, KT_D * B], BF16, kind="ExternalInput")
    xmb = nc.dram_tensor("xmb", [B, DS], BF16, kind="ExternalInput")
    w1esp = nc.dram_tensor("w1esp", [P, KT_D * HS], BF16, kind="ExternalInput")
    b1es = nc.dram_tensor("b1es", [1, HS], BF16, kind="ExternalInput")
    w2es = nc.dram_tensor("w2es", [HS, N], BF16, kind="ExternalInput")
    b2e = nc.dram_tensor("b2e", [1, N], BF16, kind="ExternalInput")
    w2Ts = nc.dram_tensor("w2Ts", [DS, H], BF16, kind="ExternalInput")
    w2s = nc.dram_tensor("w2s", [H, DS], BF16, kind="ExternalInput")
    w1 = nc.dram_tensor("w1", [N, H], BF16, kind="ExternalInput")
    w1Td = nc.dram_tensor("w1Td", [H, N], BF16, kind="ExternalInput")
    b1d = nc.dram_tensor("b1d", [1, H], BF16, kind="ExternalInput")
    sigw = nc.dram_tensor("sigw", [1, 130], F32, kind="ExternalInput")
    sel8 = nc.dram_tensor("sel8", [B, BL], BF16, kind="ExternalInput")
    epsin = nc.dram_tensor("epsin", [BL, N], F32, kind="ExternalInput")
    out = nc.dram_tensor("out", [BL, 1], F32, kind="ExternalOutput")

    # ---- internal DRAM ----
    # chunk q: upper-C col-block q slabs (+ z* partial rows in chunk 0)
    cup = [nc.dram_tensor(f"cup{q}", [(ZROWS if q == 0 else 0) + CUP_NROW[q] * P, 512],
                          BF16) for q in range(4)]
    cup_sh = [nc.dram_tensor(f"cup_sh{q}", [(ZROWS if q == 0 else 0) + CUP_NROW[q] * P, 512],
                             BF16, addr_space="Shared") for q in range(4)]
    zs_b = nc.dram_tensor("zs_b", [BL, N], F32)
    zs_sh = nc.dram_tensor("zs_sh", [B, N], F32, addr_space="Shared")
    wd_b = nc.dram_tensor("wd_b", [B, H + 1], F32)
    wds_b = nc.dram_tensor("wds_b", [BL, H + 1], F32)

    with tile.TileContext(nc) as tc, ExitStack() as ctx:
        consts = ctx.enter_context(tc.tile_pool(name="consts", bufs=1))
        work = ctx.enter_context(tc.tile_pool(name="work", bufs=2))
        stream = ctx.enter_context(tc.tile_pool(name="stream", bufs=3))
        psum = ctx.enter_context(tc.tile_pool(name="psum", bufs=2, space="PSUM"))
        psum_acc = ctx.enter_context(tc.tile_pool(name="psacc", bufs=1, space="PSUM"))
        lin = ctx.enter_context(tc.tile_pool(name="lin", bufs=1))
        res = ctx.enter_context(tc.tile_pool(name="res", bufs=1))
        encs = ctx.enter_context(tc.tile_pool(name="encs", bufs=2))

        # ---- constants / small loads ----
        identb = consts.tile([P, P], BF16)
        make_identity(nc, identb)
        ones1 = consts.tile([1, B], F32)
        nc.vector.memset(ones1, 1.0)
        onesb = consts.tile([1, B], BF16)
        nc.vector.memset(onesb, 1.0)
        sigw_sb = consts.tile([1, 130], F32)
        nc.sync.dma_start(sigw_sb, sigw[:])
        sigw_rep = consts.tile([BL, 130], F32)
        sigw_ps = psum.tile([BL, 130], F32, tag="small_ps")
        nc.tensor.matmul(sigw_ps, ones1[:, 0:BL], sigw_sb, start=True, stop=True)
        nc.vector.tensor_copy(sigw_rep, sigw_ps)
        sel8_sb = consts.tile([B, BL], BF16)
        nc.sync.dma_start(sel8_sb, sel8[:])
        eps_sb = consts.tile([BL, N], F32)
        nc.sync.dma_start(eps_sb, epsin[:])
        b1es_sb = consts.tile([1, HS], BF16)
        nc.sync.dma_start(b1es_sb, b1es[:])
        b2e_sb = consts.tile([1, N], BF16)
        nc.sync.dma_start(b2e_sb, b2e[:])
        # decoder bias as per-partition columns [P, KT_H] (+ negated copy)
        b1dcol = consts.tile([P, KT_H], BF16)
        nc.sync.dma_start(b1dcol, b1d[:].rearrange("o (k p) -> p (o k)", p=P))
        nb1col = consts.tile([P, KT_H], F32)
        nc.vector.tensor_scalar(nb1col, b1dcol, -1.0, None, Alu.mult)
        w1_sb = consts.tile([N, H], BF16)
        nc.sync.dma_start(w1_sb, w1[:])
        w1T_sb = consts.tile([P, KT_H, N], BF16)
        nc.sync.dma_start(w1T_sb, w1Td[:].rearrange("(k p) n -> p k n", p=P))

        # ---- resident weights: w2Ts (used by phase C and Wd) ----
        w2Ts_sb = res.tile([P, KT_DS, H], BF16, tag="w2Ts")
        w2Ts_r = w2Ts[:].rearrange("(k p) h -> p k h", p=P)
        for kt in range(KT_DS):
            nc.sync.dma_start(w2Ts_sb[:, kt, :], w2Ts_r[:, kt, :])

        # ================= phase E: encoder (z* partial -> cup rows 0:4) ====
        a1_ps = psum_acc.tile([B, HS], F32, tag="acc")
        KSUP = 8  # k-tiles per packed super-chunk
        for kc in range(KT_D // KSUP):
            xp_t = encs.tile([P, KSUP, B], BF16, tag="xp_t")
            nc.sync.dma_start(
                xp_t, xTp[:, kc * KSUP * B:(kc + 1) * KSUP * B]
                .rearrange("p (k b) -> p k b", b=B))
            w1t = encs.tile([P, KSUP, HS], BF16, tag="w1es_t")
            nc.sync.dma_start(
                w1t, w1esp[:, kc * KSUP * HS:(kc + 1) * KSUP * HS]
                .rearrange("p (k h) -> p k h", h=HS))
            for kj in range(KSUP):
                kt = kc * KSUP + kj
                nc.tensor.matmul(a1_ps, xp_t[:, kj, :], w1t[:, kj, :],
                                 start=(kt == 0), stop=False)
        nc.tensor.matmul(a1_ps, onesb[:, 0:B], b1es_sb, start=False, stop=True)
        h1_sb = work.tile([B, HS], BF16, tag="h1")
        nc.vector.tensor_scalar(h1_sb, a1_ps, 0.0, None, Alu.max)
        h1T_sb = work.tile([P, 2, B], BF16, tag="h1T")
        for i in range(2):
            tp = psum.tile([P, B], BF16, tag="t_ps")
            pe_T(nc, tp, h1_sb[:, i * P:(i + 1) * P], identb)
            nc.vector.tensor_copy(h1T_sb[:, i, :], tp)
        w2es_sb = work.tile([P, 2, N], BF16, tag="w2es")
        nc.sync.dma_start(w2es_sb, w2es[:].rearrange("(k p) n -> p k n", p=P))
        zp_ps = psum.tile([B, N], F32, tag="small_ps")
        for i in range(2):
            nc.tensor.matmul(zp_ps, h1T_sb[:, i, :], w2es_sb[:, i, :],
                             start=(i == 0), stop=(i == 1))
        zp_sb = work.tile([B, N], BF16, tag="zstar_part")
        nc.vector.tensor_copy(zp_sb, zp_ps)
        nc.sync.dma_start(cup[0][0:ZROWS, :], zp_sb)

        # ========= phase C: upper C slabs + chunked AllReduce ==============
        # col-block q: cols [512q, 512q+512), block-rows j in 0..4q+3
        for q in range(4):
            zr = ZROWS if q == 0 else 0
            for jg in range(q + 1):          # groups of 4 slabs
                cs = work.tile([P, 4, 512], BF16, tag="c_out")
                for jj in range(4):
                    j = jg * 4 + jj
                    cps = psum.tile([P, 512], F32, tag="big_ps")
                    for kd in range(KT_DS):
                        nc.tensor.matmul(
                            cps,
                            w2Ts_sb[:, kd, j * P:(j + 1) * P],
                            w2Ts_sb[:, kd, 512 * q:512 * (q + 1)],
                            start=(kd == 0), stop=(kd == KT_DS - 1))
                    nc.scalar.copy(cs[:, jj, :], cps)
                r0 = zr + jg * 4 * P
                nc.sync.dma_start(
                    cup[q][r0:r0 + 4 * P, :].rearrange("(s p) c -> p s c", p=P),
                    cs)
            # AllReduce this chunk (chunk 0 also carries the z* partial)
            nc.gpsimd.collective_compute(
                "AllReduce", Alu.add, replica_groups=RG,
                ins=[cup[q][:]], outs=[cup_sh[q][:]])

        # ---- z* full readback (available after chunk-0 AR) ----
        zf_sb = work.tile([B, N], BF16, tag="z_full")
        nc.sync.dma_start(zf_sb, cup_sh[0][0:ZROWS, :])

        # ---- z* post: local slice, sig1, masks, A1T (overlaps C build) ----
        zlT_ps = psum.tile([N, BL], F32, tag="small_ps")
        nc.tensor.matmul(zlT_ps, zf_sb, sel8_sb, start=True, stop=False)
        nc.tensor.matmul(zlT_ps, b2e_sb, onesb[:, 0:BL], start=False, stop=True)
        zlT_sb = work.tile([N, BL], BF16, tag="zlT")   # (z*loc + b2)^T
        nc.vector.tensor_copy(zlT_sb, zlT_ps)
        zloc_ps = psum.tile([BL, N], F32, tag="small_ps")
        nc.tensor.matmul(zloc_ps, sel8_sb, zf_sb, start=True, stop=False)
        nc.tensor.matmul(zloc_ps, onesb[:, 0:BL], b2e_sb, start=False, stop=True)
        zloc_sb = lin.tile([BL, N], F32, tag="z_loc")   # z* local + b2
        nc.vector.tensor_copy(zloc_sb, zloc_ps)

        def emit_sig(z_loc, name):
            lg = lin.tile([BL, 2, 32], F32, tag="sig_lg")
            nc.vector.tensor_tensor(
                lg, z_loc.unsqueeze(1).broadcast_to([BL, 2, 32]),
                sigw_rep[:, 0:64].rearrange("p (c n) -> p c n", c=2), Alu.mult)
            red = lin.tile([BL, 2], F32, tag=f"sig_red_{name}")
            nc.vector.tensor_reduce(red, lg, mybir.AxisListType.X, Alu.add)
            nc.vector.tensor_tensor(red, red, sigw_rep[:, 64:66], Alu.add)
            s = lin.tile([BL, 2], F32, tag=f"sig_s_{name}")
            nc.scalar.activation(s, red, Act.Exp)
            return s

        s1 = emit_sig(zloc_sb, "s1")
        invsp2 = lin.tile([BL, 1], F32, tag="invsp2")
        sp2t = lin.tile([BL, 1], F32, tag="sp2t")
        nc.vector.tensor_tensor(sp2t, s1[:, 0:1], s1[:, 0:1], Alu.mult)
        nc.vector.reciprocal(invsp2, sp2t)

        # a1T (local) -> mask m1T [P, KT_H, BL] -> A1T  (mask: a1 > -b1)
        m1T_sb = work.tile([P, KT_H, BL], BF16, tag="m1T")
        for mt in range(KT_H):
            aps = psum.tile([P, BL], F32, tag="small_ps")
            nc.tensor.matmul(aps, w1_sb[:, mt * P:(mt + 1) * P],
                             zlT_sb, start=True, stop=True)
            nc.vector.tensor_tensor(
                m1T_sb[:, mt, :], aps,
                nb1col[:, mt:mt + 1].broadcast_to([P, BL]), Alu.is_gt)
        AT_sb = res.tile([P, KT_H, BL, N], BF16, tag="AT")
        nc.vector.tensor_tensor(
            AT_sb,
            w1T_sb.unsqueeze(2).broadcast_to([P, KT_H, BL, N]),
            m1T_sb.unsqueeze(3).broadcast_to([P, KT_H, BL, N]), Alu.mult)

        # ---- C readback into SBUF + local mirror of lower blocks ----
        csb = res.tile([P, KT_H, H], BF16, tag="csb")
        for q in range(4):
            zr = ZROWS if q == 0 else 0
            shr = cup_sh[q][zr:, :].rearrange("(s p) c -> p s c", p=P)
            # upper col-block q -> csb[:, 0:4q+4, 512q:512q+512]
            nc.sync.dma_start(
                csb[:, 0:4 * q + 4, 512 * q:512 * (q + 1)], shr)
        # mirror: block (i, j) with i//4 > j//4  <-  XBAR DMA-transpose of the
        # upper block T_{j,i} straight from the AllReduced chunk in DRAM
        for qi in range(1, 4):
            zr = ZROWS if qi == 0 else 0
            for i in range(4 * qi, 4 * qi + 4):
                for j in range(4 * qi):
                    src = cup_sh[qi][zr + j * P:zr + (j + 1) * P,
                                     (i - 4 * qi) * P:(i - 4 * qi + 1) * P]
                    nc.sync.dma_start_transpose(
                        csb[:, i, j * P:(j + 1) * P], src)

        # ---- P*T = C @ A*T ; G = P*T^T A*T  (C resident in SBUF) ----
        def emit_PG(AT, tag):
            PT_sb = res.tile([P, KT_H, BL * N], BF16, tag="PT")
            for mt in range(KT_H):
                pps = psum.tile([P, BL * N], F32, tag="big_ps")
                for kt in range(KT_H):
                    nc.tensor.matmul(
                        pps, csb[:, kt, mt * P:(mt + 1) * P], AT[:, kt, :, :],
                        start=(kt == 0), stop=(kt == KT_H - 1))
                nc.scalar.copy(PT_sb[:, mt, :], pps)
            g_sb = work.tile([P, 2, P], F32, tag="g_sb")
            for grp in range(2):
                g_ps = psum.tile([P, P], F32, tag="big_ps")
                for kt in range(KT_H):
                    nc.tensor.matmul(
                        g_ps,
                        PT_sb[:, kt, grp * P:(grp + 1) * P],
                        AT[:, kt, 4 * grp:4 * grp + 4, :],
                        start=(kt == 0), stop=(kt == KT_H - 1))
                nc.vector.tensor_copy(g_sb[:, grp, :], g_ps)
            return g_sb

        # ---- Prec = G*invsp2 + sig_term + I ----
        # preset Tm with replicated sig_term while PG runs
        st_ps = psum.tile([N, N], F32, tag="small_ps")
        nc.tensor.matmul(st_ps, sigw_sb[:, 66:98], sigw_sb[:, 66:98],
                         start=True, stop=False)
        nc.tensor.matmul(st_ps, sigw_sb[:, 98:130], sigw_sb[:, 98:130],
                         start=False, stop=True)
        st_sb = work.tile([N, N], F32, tag="st_sb")
        nc.vector.tensor_copy(st_sb, st_ps)
        Tm = lin.tile([BL, N * N], F32, tag="Tmat")
        for s in range(BL):
            nc.sync.dma_start(Tm[s:s + 1, :], st_sb)
        # add diag I up front
        diag1 = sub_ap(Tm, 0, [[N + 1, N]])
        nc.vector.tensor_scalar(diag1, diag1, 1.0, None, Alu.add)

        g_sb = emit_PG(AT_sb, "1")
        SCR = lin.tile([BL, N * N], F32, tag="scr")
        for s in range(BL):
            grp, sl = s // 4, s % 4
            nc.sync.dma_start(
                SCR[s:s + 1, :],
                g_sb[sl * N:(sl + 1) * N, grp, sl * N:(sl + 1) * N])
        nc.vector.scalar_tensor_tensor(Tm, SCR, invsp2, Tm, Alu.mult, Alu.add)

        # ---- LDLT, dz (backward solve; Lt^-1/tr deferred off critical path) ----
        invD = lin.tile([BL, N], F32, tag="invD")
        emit_ldlt(nc, Tm, SCR, invD)
        LT = lin.tile([BL, N * N], F32, tag="LTmat")
        nc.vector.tensor_tensor(
            LT.rearrange("p (a b) -> p a b", b=N),
            Tm.rearrange("p (a b) -> p a b", b=N),
            invD.unsqueeze(1).broadcast_to([BL, N, N]), Alu.mult)
        srD = lin.tile([BL, N], F32, tag="srD")
        nc.scalar.activation(srD, invD, Act.Sqrt)        # 1/sqrt(D)
        epss = lin.tile([BL, N], F32, tag="epss")
        nc.vector.tensor_tensor(epss, eps_sb, srD, Alu.mult)
        emit_bwd_solve(nc, LT, epss, SCR)                # epss <- Lt^-T epss = dz
        zs_loc = lin.tile([BL, N], F32, tag="zs_loc")
        nc.vector.tensor_tensor(zs_loc, zloc_sb, epss, Alu.add)
        nc.sync.dma_start(zs_b[:], zs_loc)
        nc.gpsimd.collective_compute("AllGather", Alu.bypass, replica_groups=RG,
                                     ins=[zs_b[:]], outs=[zs_sh[:]])

        # ---- tr(Prec^-1) via Lt^-1 on GpSimd (parallel with stage 2) ----
        X1 = lin.tile([BL, N * N], F32, tag="X1")
        nc.gpsimd.memset(X1, 0.0)
        nc.gpsimd.memset(sub_ap(X1, 0, [[N + 1, N]]), 1.0)
        gSCR = lin.tile([BL, N * N], F32, tag="gSCR")
        emit_ltinv(nc.gpsimd, LT, X1, gSCR)
        trv = lin.tile([BL, 1], F32, tag="trv")
        nc.gpsimd.tensor_tensor(
            gSCR.rearrange("p (a b) -> p a b", b=N),
            X1.rearrange("p (a b) -> p a b", b=N),
            invD.unsqueeze(2).broadcast_to([BL, N, N]), Alu.mult)
        nc.gpsimd.tensor_tensor(gSCR, gSCR, X1, Alu.mult)
        # final free-axis reduce of gSCR into trv happens on vector at the tail

        # ---- z*-only reductions (vector, overlap AllGather) ----
        logs = lin.tile([BL, N], F32, tag="logs")
        ldv = lin.tile([BL, 1], F32, tag="ldv")
        nc.scalar.activation(logs, invD, Act.Ln)
        nc.vector.tensor_reduce(ldv, logs, mybir.AxisListType.X, Alu.add)  # -sum log D
        nc.vector.tensor_scalar(ldv, ldv, -0.5, None, Alu.mult)
        zsq = lin.tile([BL, N], F32, tag="zsq")
        latv = lin.tile([BL, 1], F32, tag="latv")
        nc.vector.tensor_tensor(zsq, zloc_sb, zloc_sb, Alu.mult)
        nc.vector.tensor_reduce(latv, zsq, mybir.AxisListType.X, Alu.add)
        # s2-dependent scalars (zs_loc known before AG returns)
        s2 = emit_sig(zs_loc, "s2")
        sq2 = lin.tile([BL, 2], F32, tag="sq2")
        nc.vector.tensor_tensor(sq2, s2, s2, Alu.mult)
        nc.vector.tensor_scalar(sq2, sq2, 2.0, None, Alu.mult)
        inv2 = lin.tile([BL, 2], F32, tag="inv2")
        nc.vector.reciprocal(inv2, sq2)     # [1/(2sp2^2), 1/(2sv2^2)]
        logs2 = lin.tile([BL, 2], F32, tag="logs2")
        logw = lin.tile([BL, 2], F32, tag="logw")
        nc.vector.memset(logw[:, 0:1], float(N))
        nc.vector.memset(logw[:, 1:2], float(D - N))
        nc.scalar.activation(logs2, s2, Act.Ln)
        logterm = lin.tile([BL, 1], F32, tag="logterm")
        junk2 = lin.tile([BL, 2], F32, tag="junk2")
        nc.vector.tensor_tensor(junk2, logs2, logw, Alu.mult)
        nc.vector.tensor_reduce(logterm, junk2, mybir.AxisListType.X, Alu.add)
        isub = lin.tile([BL, 1], F32, tag="isub")
        nc.vector.tensor_tensor(isub, inv2[:, 0:1], inv2[:, 1:2], Alu.subtract)

        # ---- stage 2 prep: h2T (all), m2T (local), A2T ----
        zsf_sb = work.tile([B, N], F32, tag="z_full2")
        nc.sync.dma_start(zsf_sb, zs_sh[:])
        zsf_bf = work.tile([B, N], BF16, tag="z_full2b")
        nc.vector.tensor_copy(zsf_bf, zsf_sb)
        zs_bf = lin.tile([BL, N], BF16, tag="zs_locb")
        nc.vector.tensor_copy(zs_bf, zs_loc)
        zsT_ps = psum.tile([N, B], BF16, tag="t_ps")
        pe_T(nc, zsT_ps, zsf_bf, identb)
        zsT_sb = work.tile([N, B], BF16, tag="zT2")
        nc.vector.tensor_copy(zsT_sb, zsT_ps)
        zslT_ps = psum.tile([N, BL], BF16, tag="t_ps")
        pe_T(nc, zslT_ps, zs_bf, identb)
        zslT_sb = work.tile([N, BL], BF16, tag="zlT2")
        nc.vector.tensor_copy(zslT_sb, zslT_ps)

        h2T_sb = res.tile([P, KT_H, B], BF16, tag="h2T")
        for mt in range(KT_H):
            aps = psum.tile([P, B], F32, tag="small_ps")
            nc.tensor.matmul(aps, w1_sb[:, mt * P:(mt + 1) * P],
                             zsT_sb, start=True, stop=True)
            nc.scalar.activation(h2T_sb[:, mt, :], aps, Act.Relu,
                                 bias=b1dcol[:, mt:mt + 1])

        m2T_sb = work.tile([P, KT_H, BL], BF16, tag="m2T")
        for mt in range(KT_H):
            aps = psum.tile([P, BL], F32, tag="small_ps")
            nc.tensor.matmul(aps, w1_sb[:, mt * P:(mt + 1) * P],
                             zslT_sb, start=True, stop=True)
            nc.vector.tensor_tensor(
                m2T_sb[:, mt, :], aps,
                nb1col[:, mt:mt + 1].broadcast_to([P, BL]), Alu.is_gt)
        AT2_sb = res.tile([P, KT_H, BL, N], BF16, tag="AT")   # reuse slot
        nc.vector.tensor_tensor(
            AT2_sb,
            w1T_sb.unsqueeze(2).broadcast_to([P, KT_H, BL, N]),
            m2T_sb.unsqueeze(3).broadcast_to([P, KT_H, BL, N]), Alu.mult)

        # ---- x_star slice, delta, d_sq, Wd (w2Ts from SBUF) ----
        d_sb = res.tile([B, DS], BF16, tag="d_sb")
        w2s_r = w2s[:].rearrange("(k p) ds -> p k ds", p=P)
        for nb in range(3):
            xmb_t = stream.tile([B, 512], BF16, tag="xmb_t")
            nc.sync.dma_start(xmb_t, xmb[:, nb * 512:(nb + 1) * 512])
            xs_ps = psum.tile([B, 512], F32, tag="big_ps")
            for kt in range(KT_H):
                wt = stream.tile([P, 512], BF16, tag="w2s_t")
                nc.sync.dma_start(wt, w2s_r[:, kt, nb * 512:(nb + 1) * 512])
                nc.tensor.matmul(xs_ps, h2T_sb[:, kt, :], wt,
                                 start=(kt == 0), stop=(kt == KT_H - 1))
            nc.vector.tensor_tensor(d_sb[:, nb * 512:(nb + 1) * 512], xmb_t,
                                    xs_ps, Alu.subtract)
        dT_sb = res.tile([P, KT_DS, B], BF16, tag="dT")
        for kt in range(KT_DS):
            tp = psum.tile([P, B], BF16, tag="t_ps")
            pe_T(nc, tp, d_sb[:, kt * P:(kt + 1) * P], identb)
            nc.vector.tensor_copy(dT_sb[:, kt, :], tp)
        dsq_sb = work.tile([B, 1], F32, tag="dsq")
        # d_sq = rowsum(delta^2); squares written in place (d_sb dead after dT)
        nc.scalar.activation(d_sb, d_sb, Act.Square, accum_out=dsq_sb)
        wd_sb = res.tile([B, H + 1], F32, tag="wd")
        for mb in range(4):
            wd_ps = psum.tile([B, 512], F32, tag="big_ps")
            for kt in range(KT_DS):
                nc.tensor.matmul(wd_ps, dT_sb[:, kt, :],
                                 w2Ts_sb[:, kt, mb * 512:(mb + 1) * 512],
                                 start=(kt == 0), stop=(kt == KT_DS - 1))
            nc.vector.tensor_copy(wd_sb[:, mb * 512:(mb + 1) * 512], wd_ps)
        nc.vector.tensor_copy(wd_sb[:, H:H + 1], dsq_sb)
        nc.sync.dma_start(wd_b[:], wd_sb)
        nc.gpsimd.collective_compute("ReduceScatter", Alu.add, replica_groups=RG,
                                     ins=[wd_b[:]], outs=[wds_b[:]])

        # ---- G2 on PE while Wd AllReduce runs ----
        g2_sb = emit_PG(AT2_sb, "2")
        Tm2 = lin.tile([BL, N * N], F32, tag="Tmat")   # reuse slot
        for s in range(BL):
            grp, sl = s // 4, s % 4
            nc.sync.dma_start(
                Tm2[s:s + 1, :],
                g2_sb[sl * N:(sl + 1) * N, grp, sl * N:(sl + 1) * N])

        # ---- LDLT2 (runs on vector during the ReduceScatter) ----
        invD2 = lin.tile([BL, N], F32, tag="invD2")
        emit_ldlt(nc, Tm2, SCR, invD2)
        LT2 = lin.tile([BL, N * N], F32, tag="LTmat")  # reuse slot
        nc.vector.tensor_tensor(
            LT2.rearrange("p (a b) -> p a b", b=N),
            Tm2.rearrange("p (a b) -> p a b", b=N),
            invD2.unsqueeze(1).broadcast_to([BL, N, N]), Alu.mult)

        # ---- local Wd/dsq arrive directly via ReduceScatter ----
        dsql = lin.tile([BL, 1], F32, tag="dsql")
        nc.sync.dma_start(dsql, wds_b[:, H:H + 1])
        wdl_bf = res.tile([BL, H], BF16, tag="wd_locb")
        nc.gpsimd.dma_start(wdl_bf, wds_b[:, 0:H])  # casting DMA f32->bf16
        wdlT_sb = work.tile([P, KT_H, BL], BF16, tag="wdlT")
        for kt in range(KT_H):
            tp2 = psum.tile([P, BL], BF16, tag="t_ps")
            pe_T(nc, tp2, wdl_bf[:, kt * P:(kt + 1) * P], identb)
            nc.vector.tensor_copy(wdlT_sb[:, kt, :], tp2)
        mwdT_sb = work.tile([P, KT_H, BL], BF16, tag="mwdT")
        nc.vector.tensor_tensor(mwdT_sb, wdlT_sb, m2T_sb, Alu.mult)
        # y[bl, n] = sum_h mwdT[h, bl] * w1T[h, n]  (t, already transposed)
        y_ps = psum.tile([BL, N], F32, tag="small_ps")
        for kt in range(KT_H):
            nc.tensor.matmul(y_ps, mwdT_sb[:, kt, :], w1T_sb[:, kt, :],
                             start=(kt == 0), stop=(kt == KT_H - 1))
        y = lin.tile([BL, N], F32, tag="y")
        nc.vector.tensor_copy(y, y_ps)
        emit_fwd_solve(nc, LT2, y, SCR)
        ysq = lin.tile([BL, N], F32, tag="ysq")
        yw = lin.tile([BL, N], F32, tag="yw")
        dproj = lin.tile([BL, 1], F32, tag="dproj")
        nc.vector.tensor_tensor(ysq, y, y, Alu.mult)
        nc.vector.tensor_tensor(yw, ysq, invD2, Alu.mult)
        nc.vector.tensor_reduce(dproj, yw, mybir.AxisListType.X, Alu.add)
        nc.vector.tensor_reduce(trv, gSCR, mybir.AxisListType.X, Alu.add)

        # ---- recon / output (scalars precomputed during stage 2) ----
        recon = lin.tile([BL, 1], F32, tag="recon")
        nc.vector.tensor_tensor(recon, dproj, isub, Alu.mult)
        p2t = lin.tile([BL, 1], F32, tag="p2t")
        nc.vector.tensor_tensor(p2t, dsql, inv2[:, 1:2], Alu.mult)
        nc.vector.tensor_tensor(recon, recon, p2t, Alu.add)
        nc.vector.tensor_tensor(recon, recon, logterm, Alu.add)
        ov = lin.tile([BL, 1], F32, tag="ov")
        nc.vector.tensor_tensor(ov, latv, trv, Alu.add)
        nc.vector.tensor_scalar(ov, ov, 0.5, None, Alu.mult)
        nc.vector.tensor_tensor(ov, ov, recon, Alu.add)
        nc.vector.tensor_tensor(ov, ov, ldv, Alu.add)
        nc.vector.tensor_scalar(ov, ov, 1.0 / D, None, Alu.mult)
        nc.sync.dma_start(out[:], ov)

    legalize_waits(nc)
    return nc


def shard_inputs(inputs):
    """Host-side prep: returns in_maps list for the 8 cores."""
    bf = ml_dtypes.bfloat16
    x = np.ascontiguousarray(np.asarray(inputs["x"], np.float32))
    eps = np.ascontiguousarray(np.asarray(inputs["eps"], np.float32))
    eW1 = np.ascontiguousarray(np.asarray(inputs["enc_W1"], np.float32))
    eb1 = np.asarray(inputs["enc_b1"], np.float32)
    eW2 = np.ascontiguousarray(np.asarray(inputs["enc_W2"], np.float32))
    eb2 = np.asarray(inputs["enc_b2"], np.float32)
    dW1 = np.ascontiguousarray(np.asarray(inputs["dec_W1"], np.float32))
    db1 = np.asarray(inputs["dec_b1"], np.float32)
    dW2 = np.ascontiguousarray(np.asarray(inputs["dec_W2"], np.float32))
    db2 = np.asarray(inputs["dec_b2"], np.float32)
    sW = np.asarray(inputs["sig_W"], np.float32)
    sb = np.asarray(inputs["sig_b"], np.float32)

    xT = np.ascontiguousarray(x.T).astype(bf)
    xTp = np.ascontiguousarray(
        xT.reshape(KT_D, P, B).transpose(1, 0, 2).reshape(P, KT_D * B))
    dW2T = np.ascontiguousarray(dW2.T)
    dW1T = np.ascontiguousarray(dW1.T).astype(bf)
    dW1b = dW1.astype(bf)
    sigv = np.zeros((1, 130), np.float32)
    sigv[0, 0:32] = sW[:, 0]
    sigv[0, 32:64] = sW[:, 1]
    sigv[0, 64:66] = sb
    sigv[0, 66:98] = sW[:, 0] * np.sqrt(N / 2.0)
    sigv[0, 98:130] = sW[:, 1] * np.sqrt((D - N) / 2.0)

    maps = []
    for k in range(NCORES):
        sel = np.zeros((B, BL), np.float32)
        for i in range(BL):
            sel[k * BL + i, i] = 1.0
        w1s = np.ascontiguousarray(eW1[:, k * HS:(k + 1) * HS]).astype(bf)
        maps.append({
            "xTp": xTp,
            "xmb": np.ascontiguousarray(
                x[:, k * DS:(k + 1) * DS]
                - db2[None, k * DS:(k + 1) * DS]).astype(bf),
            "w1esp": np.ascontiguousarray(
                w1s.reshape(KT_D, P, HS).transpose(1, 0, 2)
                .reshape(P, KT_D * HS)),
            "b1es": np.ascontiguousarray(eb1[None, k * HS:(k + 1) * HS]).astype(bf),
            "w2es": np.ascontiguousarray(eW2[k * HS:(k + 1) * HS, :]).astype(bf),
            "b2e": np.ascontiguousarray(eb2[None, :]).astype(bf),
            "w2Ts": np.ascontiguousarray(dW2T[k * DS:(k + 1) * DS, :]).astype(bf),
            "w2s": np.ascontiguousarray(dW2[:, k * DS:(k + 1) * DS]).astype(bf),
            "w1": dW1b,
            "w1Td": dW1T,
            "b1d": np.ascontiguousarray(db1[None, :]).astype(bf),
            "sigw": sigv,
            "sel8": sel.astype(bf),
            "epsin": np.ascontiguousarray(eps[k * BL:(k + 1) * BL, :]),
        })
    return maps


_NC_CACHE = None


def kernel(**inputs) -> np.ndarray:
    global _NC_CACHE
    from concourse.bass_utils import run_bass_kernel_spmd
    if _NC_CACHE is None:
        _NC_CACHE = build_nc()
    nc = _NC_CACHE
    maps = shard_inputs(inputs)
    res = run_bass_kernel_spmd(nc, maps, list(range(NCORES)))
    outs = [res.results[k]["out"].reshape(BL) for k in range(NCORES)]
    return np.concatenate(outs).astype(np.float32)


# revision 76
# speedup vs baseline: 1.1834x; 1.1834x over previous
"""Trainium2 Bass kernel for nn_EnergyAE (B=64, D=12288, N=32, H=2048) on 8 cores.

Hybrid sharding, bf16 matmuls (fp32 vector math):
  phase E  (model-parallel over H): encoder -> z* partial (bf16, folded into
           C-chunk0 AllReduce below)
  phase C  (contraction-parallel over D): upper-triangular 512-col blocks of
           Cpart = w2Ts @ w2Ts^T, AllReduced in 4 bf16 chunks pipelined with
           the build; mirrored to full C in SBUF locally after readback.
           C stays SBUF-resident for both PG stages.
  phase S1 (data-parallel, 8 samples/core): A1 = W1*m1, P1T = C@A1T,
           G = P1T^T A1T (batched 4 samples/matmul), Prec, LDLT, Lt^-1, dz,
           tr, logdet, z_s = z* + dz -> AllGather z_s (8KB)
  phase S2 (model-parallel over D): h2, x_star slice, delta, d_sq,
           Wd = delta@W2s^T (w2Ts reused from SBUF) -> AllReduce [Wd | d_sq]
  phase S3 (data-parallel): t = W1T^T(m2*Wd), G2 = A2 C A2^T, LDLT2,
           fwd solve, d_proj_sq, recon -> out (8 per core)

Identities replacing eigvalsh/cholesky/solve_triangular:
  Prec = Lt D Lt^T (unit-lower LDLT)
  sum(log eig)/2 = 0.5*sum(log D);   sum(1/eig) = ||D^-1/2 Lt^-1||_F^2
  U^-1 eps = Lt^-T (eps/sqrt(D));    t^T G2^-1 t = ||D2^-1/2 Lt2^-1 t||^2
  sig_term = (n w0 w0^T + (D-n) w1 w1^T)/2   (constant across batch)
"""
import sys

for _p in ("/opt/trn_rl_repo", "/root/.axon_site/_ro/trn_rl_repo"):
    if _p not in sys.path:
        sys.path.append(_p)

import numpy as np
import ml_dtypes
from contextlib import ExitStack

import concourse.bass as bass
import concourse.mybir as mybir
import concourse.tile as tile
from concourse.masks import make_identity

B, D, N, H = 64, 12288, 32, 2048
NCORES = 8
BL = B // NCORES          # 8 local samples
HS = H // NCORES          # 256
DS = D // NCORES          # 1536
KT_H = H // 128           # 16
KT_D = D // 128           # 96
KT_DS = DS // 128         # 12
P = 128
# upper-triangular C slab layout: col-block q holds block-rows 0..4q+3
CUP_OFF = [0, 4, 12, 24]          # slab index offset per col-block
CUP_NROW = [4, 8, 12, 16]         # slabs per col-block
CUP_TOT = 40
ZROWS = 4                         # z* partial occupies cup rows 0..3 (4x512)

F32 = mybir.dt.float32
BF16 = mybir.dt.bfloat16
FP8 = mybir.dt.float8e4
W2SC = 8.0            # fp8 scale on w2Ts; C/G carry W2SC^2, Wd carries W2SC
DR = mybir.MatmulPerfMode.DoubleRow
Alu = mybir.AluOpType
Act = mybir.ActivationFunctionType
RG = [list(range(NCORES))]


def sub_ap(t, extra_off, dims):
    """Custom free-dim AP on a [P, F] tile; dims = [[step,count],...] in elems."""
    base = t[:, 0:1]
    return bass.AP(base.tensor, base.offset + extra_off, [base.ap[0]] + dims)


def pe_T(nc, out_ps, in_ap, ident):
    """PE transpose: out_ps [f, p] = in_ap [p, f].T"""
    kp = in_ap.shape[0]
    nc.tensor.transpose(out_ps, in_ap, ident[0:kp, 0:kp])


def emit_ldlt(nc, T, OUT, invD, n=32):
    """In-place unit-lower LDLT of T [BL, n*n] (row-major per sample).
    After: strict lower of T holds unscaled columns u; diag holds D; invD = 1/D."""
    for j in range(n):
        nc.vector.reciprocal(invD[:, j:j + 1], T[:, (n + 1) * j:(n + 1) * j + 1])
        m = n - 1 - j
        if m == 0:
            break
        base = (j + 1) * n + j
        u_i = sub_ap(T, base, [[n, m], [0, m]])
        u_k = sub_ap(T, base, [[0, m], [n, m]])
        outer = sub_ap(OUT, 0, [[m, m], [1, m]])
        nc.vector.scalar_tensor_tensor(
            outer, u_i, invD[:, j:j + 1], u_k, Alu.mult, Alu.mult)
        trail = sub_ap(T, (j + 1) * (n + 1), [[n, m], [1, m]])
        nc.vector.tensor_tensor(trail, trail, outer, Alu.subtract)


def emit_ltinv(eng, LT, X, OUT, n=32):
    """X = LT^{-1} for unit-lower LT [BL, n*n]; X preset to I by caller.
    Uses only tensor_tensor (runs on GpSimd, whose ISA lacks STT)."""
    for k in range(n - 1):
        rows = n - 1 - k
        cols = k + 1
        lcol = sub_ap(LT, (k + 1) * n + k, [[n, rows], [0, cols]])
        xrow = sub_ap(X, k * n, [[0, rows], [1, cols]])
        prod = sub_ap(OUT, 0, [[cols, rows], [1, cols]])
        eng.tensor_tensor(prod, lcol, xrow, Alu.mult)
        xblk = sub_ap(X, (k + 1) * n, [[n, rows], [1, cols]])
        eng.tensor_tensor(xblk, xblk, prod, Alu.subtract)


def emit_bwd_solve(nc, LT, y, OUT, n=32):
    """y <- LT^{-T} y for unit-lower LT [BL, n*n], y [BL, n] in place."""
    for k in range(n - 1, 0, -1):
        lrow = sub_ap(LT, k * n, [[1, k]])
        nc.vector.scalar_tensor_tensor(
            OUT[:, 0:k], lrow, -1.0, y[:, k:k + 1].broadcast_to([BL, k]),
            Alu.mult, Alu.mult)
        nc.vector.tensor_tensor(y[:, 0:k], y[:, 0:k], OUT[:, 0:k], Alu.add)


def emit_fwd_solve(nc, LT, y, OUT, n=32):
    """y <- LT^{-1} y for unit-lower LT [BL, n*n], y [BL, n] in place."""
    for k in range(n - 1):
        rows = n - 1 - k
        lcol = sub_ap(LT, (k + 1) * n + k, [[n, rows]])
        nc.vector.scalar_tensor_tensor(
            OUT[:, 0:rows], lcol, -1.0, y[:, k:k + 1].broadcast_to([BL, rows]),
            Alu.mult, Alu.mult)
        nc.vector.tensor_tensor(y[:, k + 1:n], y[:, k + 1:n], OUT[:, 0:rows], Alu.add)


def legalize_waits(nc, maxw=1):
    """Split multi-wait sync_info into standalone EventSemaphore instructions."""
    for f in nc.m.functions:
        for bb in f.blocks:
            insts = list(bb.instructions)
            out = []
            changed = False
            for inst in insts:
                si = inst.sync_info
                if si is not None and si.on_wait and len(si.on_wait) > maxw:
                    waits = list(si.on_wait)
                    imm = [w for w in waits if w.uses_immediate]
                    reg = [w for w in waits if not w.uses_immediate]
                    keep = (reg + imm)[:maxw] if len(reg) <= maxw else reg
                    extra = [w for w in waits if w not in keep]
                    if len(keep) > maxw:
                        raise RuntimeError(f"{inst.name}: {len(keep)} register waits")
                    for w in extra:
                        ev = mybir.InstEventSemaphore(
                            name=nc.get_next_instruction_name(), ins=[], outs=[])
                        ev.engine = inst.engine
                        ev.sync_info = mybir.SyncInfo(on_wait=[w], on_update=[])
                        out.append(ev)
                    inst.sync_info = mybir.SyncInfo(
                        on_wait=keep, on_update=list(si.on_update or []))
                    changed = True
                out.append(inst)
            if changed:
                bb.instructions = out
    return nc


def build_nc():
    nc = bass.Bass()

    # ---- I/O ----
    # xTp/w1esp: pre-packed partition-major [(p) (k b)] so DMA descriptors are
    # per-partition contiguous (12KB / 6KB) instead of 128B/512B strided
    xTp = nc.dram_tensor("xTp", [P, KT_D * B], BF16, kind="ExternalInput")
    xmb = nc.dram_tensor("xmb", [B, DS], BF16, kind="ExternalInput")
    w1esp = nc.dram_tensor("w1esp", [P, KT_D * HS], BF16, kind="ExternalInput")
    b1es = nc.dram_tensor("b1es", [1, HS], BF16, kind="ExternalInput")
    w2es = nc.dram_tensor("w2es", [HS, N], BF16, kind="ExternalInput")
    b2e = nc.dram_tensor("b2e", [1, N], BF16, kind="ExternalInput")
    w2Ts = nc.dram_tensor("w2Ts", [DS, H], FP8, kind="ExternalInput")
    w2s = nc.dram_tensor("w2s", [H, DS], BF16, kind="ExternalInput")
    w1 = nc.dram_tensor("w1", [N, H], BF16, kind="ExternalInput")
    w1Td = nc.dram_tensor("w1Td", [H, N], BF16, kind="ExternalInput")
    b1d = nc.dram_tensor("b1d", [1, H], BF16, kind="ExternalInput")
    sigw = nc.dram_tensor("sigw", [1, 130], F32, kind="ExternalInput")
    sel8 = nc.dram_tensor("sel8", [B, BL], BF16, kind="ExternalInput")
    epsin = nc.dram_tensor("epsin", [BL, N], F32, kind="ExternalInput")
    out = nc.dram_tensor("out", [BL, 1], F32, kind="ExternalOutput")

    # ---- internal DRAM ----
    # chunk q: upper-C col-block q slabs (+ z* partial rows in chunk 0)
    cup = [nc.dram_tensor(f"cup{q}", [(ZROWS if q == 0 else 0) + CUP_NROW[q] * P, 512],
                          BF16) for q in range(4)]
    cup_sh = [nc.dram_tensor(f"cup_sh{q}", [(ZROWS if q == 0 else 0) + CUP_NROW[q] * P, 512],
                             BF16, addr_space="Shared") for q in range(4)]
    zs_b = nc.dram_tensor("zs_b", [BL, N], F32)
    zs_sh = nc.dram_tensor("zs_sh", [B, N], F32, addr_space="Shared")
    wd_b = nc.dram_tensor("wd_b", [B, H], BF16)
    wds_b = nc.dram_tensor("wds_b", [BL, H], BF16)
    dsq_b = nc.dram_tensor("dsq_b", [B, 1], F32)
    dsqs_b = nc.dram_tensor("dsqs_b", [BL, 1], F32)

    with tile.TileContext(nc) as tc, ExitStack() as ctx:
        consts = ctx.enter_context(tc.tile_pool(name="consts", bufs=1))
        work = ctx.enter_context(tc.tile_pool(name="work", bufs=2))
        stream = ctx.enter_context(tc.tile_pool(name="stream", bufs=3))
        psum = ctx.enter_context(tc.tile_pool(name="psum", bufs=2, space="PSUM"))
        psum_acc = ctx.enter_context(tc.tile_pool(name="psacc", bufs=1, space="PSUM"))
        lin = ctx.enter_context(tc.tile_pool(name="lin", bufs=1))
        res = ctx.enter_context(tc.tile_pool(name="res", bufs=1))
        encs = ctx.enter_context(tc.tile_pool(name="encs", bufs=2))
        w2sp = ctx.enter_context(tc.tile_pool(name="w2sp", bufs=6))

        # ---- constants / small loads ----
        identb = consts.tile([P, P], BF16)
        make_identity(nc, identb)
        ones1 = consts.tile([1, B], F32)
        nc.vector.memset(ones1, 1.0)
        onesb = consts.tile([1, B], BF16)
        nc.vector.memset(onesb, 1.0)
        sigw_sb = consts.tile([1, 130], F32)
        nc.sync.dma_start(sigw_sb, sigw[:])
        sigw_rep = consts.tile([BL, 130], F32)
        sigw_ps = psum.tile([BL, 130], F32, tag="small_ps")
        nc.tensor.matmul(sigw_ps, ones1[:, 0:BL], sigw_sb, start=True, stop=True)
        nc.vector.tensor_copy(sigw_rep, sigw_ps)
        sel8_sb = consts.tile([B, BL], BF16)
        nc.sync.dma_start(sel8_sb, sel8[:])
        eps_sb = consts.tile([BL, N], F32)
        nc.sync.dma_start(eps_sb, epsin[:])
        b1es_sb = consts.tile([1, HS], BF16)
        nc.sync.dma_start(b1es_sb, b1es[:])
        b2e_sb = consts.tile([1, N], BF16)
        nc.sync.dma_start(b2e_sb, b2e[:])
        # decoder bias as per-partition columns [P, KT_H] (+ negated copy)
        b1dcol = consts.tile([P, KT_H], BF16)
        nc.sync.dma_start(b1dcol, b1d[:].rearrange("o (k p) -> p (o k)", p=P))
        nb1col = consts.tile([P, KT_H], F32)
        nc.vector.tensor_scalar(nb1col, b1dcol, -1.0, None, Alu.mult)
        w1_sb = consts.tile([N, H], BF16)
        nc.sync.dma_start(w1_sb, w1[:])
        w1T_sb = consts.tile([P, KT_H, N], BF16)
        nc.sync.dma_start(w1T_sb, w1Td[:].rearrange("(k p) n -> p k n", p=P))

        # ---- resident weights: w2Ts fp8 x W2SC (used by phase C and Wd) ----
        w2Ts_sb = res.tile([P, KT_DS, H], FP8, tag="w2Ts")
        w2Ts_r = w2Ts[:].rearrange("(k p) h -> p k h", p=P)
        for kt in range(KT_DS):
            nc.sync.dma_start(w2Ts_sb[:, kt, :], w2Ts_r[:, kt, :])

        # ================= phase E: encoder (z* partial -> cup rows 0:4) ====
        a1_ps = psum_acc.tile([B, HS], F32, tag="acc")
        KSUP = 8  # k-tiles per packed super-chunk
        for kc in range(KT_D // KSUP):
            xp_t = encs.tile([P, KSUP, B], BF16, tag="xp_t")
            nc.sync.dma_start(
                xp_t, xTp[:, kc * KSUP * B:(kc + 1) * KSUP * B]
                .rearrange("p (k b) -> p k b", b=B))
            w1t = encs.tile([P, KSUP, HS], BF16, tag="w1es_t")
            nc.sync.dma_start(
                w1t, w1esp[:, kc * KSUP * HS:(kc + 1) * KSUP * HS]
                .rearrange("p (k h) -> p k h", h=HS))
            for kj in range(KSUP):
                kt = kc * KSUP + kj
                nc.tensor.matmul(a1_ps, xp_t[:, kj, :], w1t[:, kj, :],
                                 start=(kt == 0), stop=False)
        nc.tensor.matmul(a1_ps, onesb[:, 0:B], b1es_sb, start=False, stop=True)
        h1_sb = work.tile([B, HS], BF16, tag="h1")
        nc.vector.tensor_scalar(h1_sb, a1_ps, 0.0, None, Alu.max)
        h1T_sb = work.tile([P, 2, B], BF16, tag="h1T")
        for i in range(2):
            tp = psum.tile([P, B], BF16, tag="t_ps")
            pe_T(nc, tp, h1_sb[:, i * P:(i + 1) * P], identb)
            nc.vector.tensor_copy(h1T_sb[:, i, :], tp)
        w2es_sb = work.tile([P, 2, N], BF16, tag="w2es")
        nc.sync.dma_start(w2es_sb, w2es[:].rearrange("(k p) n -> p k n", p=P))
        zp_ps = psum.tile([B, N], F32, tag="small_ps")
        for i in range(2):
            nc.tensor.matmul(zp_ps, h1T_sb[:, i, :], w2es_sb[:, i, :],
                             start=(i == 0), stop=(i == 1))
        zp_sb = work.tile([B, N], BF16, tag="zstar_part")
        nc.vector.tensor_copy(zp_sb, zp_ps)
        nc.sync.dma_start(cup[0][0:ZROWS, :], zp_sb)

        # ========= phase C: upper C slabs + chunked AllReduce ==============
        # col-block q: cols [512q, 512q+512), block-rows j in 0..4q+3
        csb = res.tile([P, KT_H, H], BF16, tag="csb")
        for q in range(4):
            zr = ZROWS if q == 0 else 0
            for jg in range(q + 1):          # groups of 4 slabs
                cs = work.tile([P, 4, 512], BF16, tag="c_out")
                for jj in range(4):
                    j = jg * 4 + jj
                    cps = psum.tile([P, 512], F32, tag="big_ps")
                    for kd in range(KT_DS // 2):
                        nc.tensor.matmul(
                            cps,
                            w2Ts_sb[:, 2 * kd:2 * kd + 2, j * P:(j + 1) * P],
                            w2Ts_sb[:, 2 * kd:2 * kd + 2, 512 * q:512 * (q + 1)],
                            start=(kd == 0), stop=(kd == KT_DS // 2 - 1),
                            perf_mode=DR)
                    nc.scalar.copy(cs[:, jj, :], cps)
                r0 = zr + jg * 4 * P
                nc.sync.dma_start(
                    cup[q][r0:r0 + 4 * P, :].rearrange("(s p) c -> p s c", p=P),
                    cs)
            # AllReduce this chunk (chunk 0 also carries the z* partial)
            nc.gpsimd.collective_compute(
                "AllReduce", Alu.add, replica_groups=RG,
                ins=[cup[q][:]], outs=[cup_sh[q][:]])
            # readback on the gpsimd DMA queue right behind the AR so it
            # lands as soon as the chunk is reduced (SP queue is busy with
            # input streams / cup writes)
            if q == 0:
                zf_sb = work.tile([B, N], BF16, tag="z_full")
                nc.gpsimd.dma_start(zf_sb, cup_sh[0][0:ZROWS, :])
            zr_ = ZROWS if q == 0 else 0
            shr = cup_sh[q][zr_:, :].rearrange("(s p) c -> p s c", p=P)
            nc.gpsimd.dma_start(
                csb[:, 0:4 * q + 4, 512 * q:512 * (q + 1)], shr)

        # ---- z* post: local slice, sig1, masks, A1T (overlaps C build) ----
        zlT_ps = psum.tile([N, BL], F32, tag="small_ps")
        nc.tensor.matmul(zlT_ps, zf_sb, sel8_sb, start=True, stop=False)
        nc.tensor.matmul(zlT_ps, b2e_sb, onesb[:, 0:BL], start=False, stop=True)
        zlT_sb = work.tile([N, BL], BF16, tag="zlT")   # (z*loc + b2)^T
        nc.vector.tensor_copy(zlT_sb, zlT_ps)
        zloc_ps = psum.tile([BL, N], F32, tag="small_ps")
        nc.tensor.matmul(zloc_ps, sel8_sb, zf_sb, start=True, stop=False)
        nc.tensor.matmul(zloc_ps, onesb[:, 0:BL], b2e_sb, start=False, stop=True)
        zloc_sb = lin.tile([BL, N], F32, tag="z_loc")   # z* local + b2
        nc.vector.tensor_copy(zloc_sb, zloc_ps)

        def emit_sig(z_loc, name):
            lg = lin.tile([BL, 2, 32], F32, tag="sig_lg")
            nc.vector.tensor_tensor(
                lg, z_loc.unsqueeze(1).broadcast_to([BL, 2, 32]),
                sigw_rep[:, 0:64].rearrange("p (c n) -> p c n", c=2), Alu.mult)
            red = lin.tile([BL, 2], F32, tag=f"sig_red_{name}")
            nc.vector.tensor_reduce(red, lg, mybir.AxisListType.X, Alu.add)
            nc.vector.tensor_tensor(red, red, sigw_rep[:, 64:66], Alu.add)
            s = lin.tile([BL, 2], F32, tag=f"sig_s_{name}")
            nc.scalar.activation(s, red, Act.Exp)
            return s

        s1 = emit_sig(zloc_sb, "s1")
        invsp2 = lin.tile([BL, 1], F32, tag="invsp2")
        sp2t = lin.tile([BL, 1], F32, tag="sp2t")
        nc.vector.tensor_tensor(sp2t, s1[:, 0:1], s1[:, 0:1], Alu.mult)
        # G arrives scaled by W2SC^2 (fp8 weights); fold 1/W2SC^2 in here
        nc.vector.tensor_scalar(sp2t, sp2t, W2SC * W2SC, None, Alu.mult)
        nc.vector.reciprocal(invsp2, sp2t)

        # a1T (local) -> mask m1T [P, KT_H, BL] -> A1T  (mask: a1 > -b1)
        m1T_sb = work.tile([P, KT_H, BL], BF16, tag="m1T")
        for mt in range(KT_H):
            aps = psum.tile([P, BL], F32, tag="small_ps")
            nc.tensor.matmul(aps, w1_sb[:, mt * P:(mt + 1) * P],
                             zlT_sb, start=True, stop=True)
            nc.vector.tensor_tensor(
                m1T_sb[:, mt, :], aps,
                nb1col[:, mt:mt + 1].broadcast_to([P, BL]), Alu.is_gt)
        AT_sb = res.tile([P, KT_H, BL, N], BF16, tag="AT")
        nc.vector.tensor_tensor(
            AT_sb,
            w1T_sb.unsqueeze(2).broadcast_to([P, KT_H, BL, N]),
            m1T_sb.unsqueeze(3).broadcast_to([P, KT_H, BL, N]), Alu.mult)

        # ---- local mirror of lower blocks via PE transposes ----
        # mirror: block (i, j) with i//4 > j//4  <-  transpose of (j, i)
        for qi in range(1, 4):
            for i in range(4 * qi, 4 * qi + 4):
                for j in range(4 * qi):
                    tp = psum.tile([P, P], BF16, tag="t_ps")
                    pe_T(nc, tp, csb[:, j, i * P:(i + 1) * P], identb)
                    nc.scalar.copy(csb[:, i, j * P:(j + 1) * P], tp)

        # ---- P*T = C @ A*T ; G = P*T^T A*T  (C resident in SBUF) ----
        def emit_PG(AT, tag):
            PT_sb = res.tile([P, KT_H, BL * N], BF16, tag="PT")
            for mt in range(KT_H):
                pps = psum.tile([P, BL * N], F32, tag="big_ps")
                for kt in range(KT_H):
                    nc.tensor.matmul(
                        pps, csb[:, kt, mt * P:(mt + 1) * P], AT[:, kt, :, :],
                        start=(kt == 0), stop=(kt == KT_H - 1))
                nc.scalar.copy(PT_sb[:, mt, :], pps)
            g_sb = work.tile([P, 2, P], F32, tag="g_sb")
            for grp in range(2):
                g_ps = psum.tile([P, P], F32, tag="big_ps")
                for kt in range(KT_H):
                    nc.tensor.matmul(
                        g_ps,
                        PT_sb[:, kt, grp * P:(grp + 1) * P],
                        AT[:, kt, 4 * grp:4 * grp + 4, :],
                        start=(kt == 0), stop=(kt == KT_H - 1))
                nc.vector.tensor_copy(g_sb[:, grp, :], g_ps)
            return g_sb

        # ---- Prec = G*invsp2 + sig_term + I ----
        # preset Tm with replicated sig_term while PG runs
        st_ps = psum.tile([N, N], F32, tag="small_ps")
        nc.tensor.matmul(st_ps, sigw_sb[:, 66:98], sigw_sb[:, 66:98],
                         start=True, stop=False)
        nc.tensor.matmul(st_ps, sigw_sb[:, 98:130], sigw_sb[:, 98:130],
                         start=False, stop=True)
        st_sb = work.tile([N, N], F32, tag="st_sb")
        nc.vector.tensor_copy(st_sb, st_ps)
        Tm = lin.tile([BL, N * N], F32, tag="Tmat")
        for s in range(BL):
            nc.sync.dma_start(Tm[s:s + 1, :], st_sb)
        # add diag I up front
        diag1 = sub_ap(Tm, 0, [[N + 1, N]])
        nc.vector.tensor_scalar(diag1, diag1, 1.0, None, Alu.add)

        g_sb = emit_PG(AT_sb, "1")
        SCR = lin.tile([BL, N * N], F32, tag="scr")
        for s in range(BL):
            grp, sl = s // 4, s % 4
            nc.sync.dma_start(
                SCR[s:s + 1, :],
                g_sb[sl * N:(sl + 1) * N, grp, sl * N:(sl + 1) * N])
        nc.vector.scalar_tensor_tensor(Tm, SCR, invsp2, Tm, Alu.mult, Alu.add)

        # ---- LDLT, dz (backward solve; Lt^-1/tr deferred off critical path) ----
        invD = lin.tile([BL, N], F32, tag="invD")
        emit_ldlt(nc, Tm, SCR, invD)
        LT = lin.tile([BL, N * N], F32, tag="LTmat")
        nc.vector.tensor_tensor(
            LT.rearrange("p (a b) -> p a b", b=N),
            Tm.rearrange("p (a b) -> p a b", b=N),
            invD.unsqueeze(1).broadcast_to([BL, N, N]), Alu.mult)
        srD = lin.tile([BL, N], F32, tag="srD")
        nc.scalar.activation(srD, invD, Act.Sqrt)        # 1/sqrt(D)
        epss = lin.tile([BL, N], F32, tag="epss")
        nc.vector.tensor_tensor(epss, eps_sb, srD, Alu.mult)
        emit_bwd_solve(nc, LT, epss, SCR)                # epss <- Lt^-T epss = dz
        zs_loc = lin.tile([BL, N], F32, tag="zs_loc")
        nc.vector.tensor_tensor(zs_loc, zloc_sb, epss, Alu.add)
        nc.sync.dma_start(zs_b[:], zs_loc)
        nc.gpsimd.collective_compute("AllGather", Alu.bypass, replica_groups=RG,
                                     ins=[zs_b[:]], outs=[zs_sh[:]])

        # ---- tr(Prec^-1) via Lt^-1 on GpSimd (parallel with stage 2) ----
        X1 = lin.tile([BL, N * N], F32, tag="X1")
        nc.gpsimd.memset(X1, 0.0)
        nc.gpsimd.memset(sub_ap(X1, 0, [[N + 1, N]]), 1.0)
        gSCR = lin.tile([BL, N * N], F32, tag="gSCR")
        emit_ltinv(nc.gpsimd, LT, X1, gSCR)
        trv = lin.tile([BL, 1], F32, tag="trv")
        nc.gpsimd.tensor_tensor(
            gSCR.rearrange("p (a b) -> p a b", b=N),
            X1.rearrange("p (a b) -> p a b", b=N),
            invD.unsqueeze(2).broadcast_to([BL, N, N]), Alu.mult)
        nc.gpsimd.tensor_tensor(gSCR, gSCR, X1, Alu.mult)
        # final free-axis reduce of gSCR into trv happens on vector at the tail

        # ---- z*-only reductions (vector, overlap AllGather) ----
        logs = lin.tile([BL, N], F32, tag="logs")
        ldv = lin.tile([BL, 1], F32, tag="ldv")
        nc.scalar.activation(logs, invD, Act.Ln)
        nc.vector.tensor_reduce(ldv, logs, mybir.AxisListType.X, Alu.add)  # -sum log D
        nc.vector.tensor_scalar(ldv, ldv, -0.5, None, Alu.mult)
        zsq = lin.tile([BL, N], F32, tag="zsq")
        latv = lin.tile([BL, 1], F32, tag="latv")
        nc.vector.tensor_tensor(zsq, zloc_sb, zloc_sb, Alu.mult)
        nc.vector.tensor_reduce(latv, zsq, mybir.AxisListType.X, Alu.add)
        # s2-dependent scalars (zs_loc known before AG returns)
        s2 = emit_sig(zs_loc, "s2")
        sq2 = lin.tile([BL, 2], F32, tag="sq2")
        nc.vector.tensor_tensor(sq2, s2, s2, Alu.mult)
        nc.vector.tensor_scalar(sq2, sq2, 2.0, None, Alu.mult)
        inv2 = lin.tile([BL, 2], F32, tag="inv2")
        nc.vector.reciprocal(inv2, sq2)     # [1/(2sp2^2), 1/(2sv2^2)]
        logs2 = lin.tile([BL, 2], F32, tag="logs2")
        logw = lin.tile([BL, 2], F32, tag="logw")
        nc.vector.memset(logw[:, 0:1], float(N))
        nc.vector.memset(logw[:, 1:2], float(D - N))
        nc.scalar.activation(logs2, s2, Act.Ln)
        logterm = lin.tile([BL, 1], F32, tag="logterm")
        junk2 = lin.tile([BL, 2], F32, tag="junk2")
        nc.vector.tensor_tensor(junk2, logs2, logw, Alu.mult)
        nc.vector.tensor_reduce(logterm, junk2, mybir.AxisListType.X, Alu.add)
        isub = lin.tile([BL, 1], F32, tag="isub")
        nc.vector.tensor_tensor(isub, inv2[:, 0:1], inv2[:, 1:2], Alu.subtract)

        # ---- stage 2 prep: h2T (all), m2T (local), A2T ----
        zsf_sb = work.tile([B, N], F32, tag="z_full2")
        nc.sync.dma_start(zsf_sb, zs_sh[:])
        zsf_bf = work.tile([B, N], BF16, tag="z_full2b")
        nc.vector.tensor_copy(zsf_bf, zsf_sb)
        zs_bf = lin.tile([BL, N], BF16, tag="zs_locb")
        nc.vector.tensor_copy(zs_bf, zs_loc)
        zsT_ps = psum.tile([N, B], BF16, tag="t_ps")
        pe_T(nc, zsT_ps, zsf_bf, identb)
        zsT_sb = work.tile([N, B], BF16, tag="zT2")
        nc.vector.tensor_copy(zsT_sb, zsT_ps)
        zslT_ps = psum.tile([N, BL], BF16, tag="t_ps")
        pe_T(nc, zslT_ps, zs_bf, identb)
        zslT_sb = work.tile([N, BL], BF16, tag="zlT2")
        nc.vector.tensor_copy(zslT_sb, zslT_ps)

        h2T_sb = res.tile([P, KT_H, B], BF16, tag="h2T")
        for mt in range(KT_H):
            aps = psum.tile([P, B], F32, tag="small_ps")
            nc.tensor.matmul(aps, w1_sb[:, mt * P:(mt + 1) * P],
                             zsT_sb, start=True, stop=True)
            nc.scalar.activation(h2T_sb[:, mt, :], aps, Act.Relu,
                                 bias=b1dcol[:, mt:mt + 1])

        m2T_sb = work.tile([P, KT_H, BL], BF16, tag="m2T")
        for mt in range(KT_H):
            aps = psum.tile([P, BL], F32, tag="small_ps")
            nc.tensor.matmul(aps, w1_sb[:, mt * P:(mt + 1) * P],
                             zslT_sb, start=True, stop=True)
            nc.vector.tensor_tensor(
                m2T_sb[:, mt, :], aps,
                nb1col[:, mt:mt + 1].broadcast_to([P, BL]), Alu.is_gt)
        AT2_sb = res.tile([P, KT_H, BL, N], BF16, tag="AT")   # reuse slot
        nc.vector.tensor_tensor(
            AT2_sb,
            w1T_sb.unsqueeze(2).broadcast_to([P, KT_H, BL, N]),
            m2T_sb.unsqueeze(3).broadcast_to([P, KT_H, BL, N]), Alu.mult)

        # ---- G2 on PE first: fills PE while the w2s stream for x_star runs --
        g2_sb = emit_PG(AT2_sb, "2")
        Tm2 = lin.tile([BL, N * N], F32, tag="Tmat")   # reuse slot
        for s in range(BL):
            grp, sl = s // 4, s % 4
            nc.sync.dma_start(
                Tm2[s:s + 1, :],
                g2_sb[sl * N:(sl + 1) * N, grp, sl * N:(sl + 1) * N])
        # Jacobi weights for the Richardson solve (no factorization needed)
        dg2 = lin.tile([BL, N], F32, tag="dg2")
        nc.vector.tensor_copy(dg2, sub_ap(Tm2, 0, [[N + 1, N]]))
        widg = lin.tile([BL, N], F32, tag="widg")
        nc.vector.reciprocal(widg, dg2)
        nc.vector.tensor_scalar(widg, widg, 0.9, None, Alu.mult)

        # ---- x_star slice, delta, d_sq, Wd (w2Ts from SBUF) ----
        d_sb = res.tile([B, DS], BF16, tag="d_sb")
        w2s_r = w2s[:].rearrange("(k p) ds -> p k ds", p=P)
        for nb in range(3):
            xmb_t = stream.tile([B, 512], BF16, tag="xmb_t")
            nc.sync.dma_start(xmb_t, xmb[:, nb * 512:(nb + 1) * 512])
            xs_ps = psum.tile([B, 512], F32, tag="big_ps")
            for kt in range(KT_H):
                wt = w2sp.tile([P, 512], BF16, tag="w2s_t")
                nc.sync.dma_start(wt, w2s_r[:, kt, nb * 512:(nb + 1) * 512])
                nc.tensor.matmul(xs_ps, h2T_sb[:, kt, :], wt,
                                 start=(kt == 0), stop=(kt == KT_H - 1))
            nc.vector.tensor_tensor(d_sb[:, nb * 512:(nb + 1) * 512], xmb_t,
                                    xs_ps, Alu.subtract)
        dT_bf = res.tile([P, KT_DS, B], BF16, tag="dTb")
        for kt in range(KT_DS):
            tp = psum.tile([P, B], BF16, tag="t_ps")
            pe_T(nc, tp, d_sb[:, kt * P:(kt + 1) * P], identb)
            nc.vector.tensor_copy(dT_bf[:, kt, :], tp)
        dT8 = res.tile([P, KT_DS, B], FP8, tag="dT")
        nc.scalar.copy(dT8, dT_bf)
        dsq_sb = work.tile([B, 1], F32, tag="dsq")
        # d_sq = rowsum(delta^2); squares written in place (d_sb dead after d8)
        nc.scalar.activation(d_sb, d_sb, Act.Square, accum_out=dsq_sb)
        nc.sync.dma_start(dsq_b[:], dsq_sb)
        nc.gpsimd.collective_compute("ReduceScatter", Alu.add, replica_groups=RG,
                                     ins=[dsq_b[:]], outs=[dsqs_b[:]])
        wd_sb = res.tile([B, H], BF16, tag="wd")
        for mb in range(4):
            wd_ps = psum.tile([B, 512], F32, tag="big_ps")
            for kd in range(KT_DS // 2):
                nc.tensor.matmul(wd_ps, dT8[:, 2 * kd:2 * kd + 2, :],
                                 w2Ts_sb[:, 2 * kd:2 * kd + 2,
                                         mb * 512:(mb + 1) * 512],
                                 start=(kd == 0), stop=(kd == KT_DS // 2 - 1),
                                 perf_mode=DR)
            nc.vector.tensor_scalar(wd_sb[:, mb * 512:(mb + 1) * 512], wd_ps,
                                    1.0 / W2SC, None, Alu.mult)
        nc.sync.dma_start(wd_b[:], wd_sb)
        nc.gpsimd.collective_compute("ReduceScatter", Alu.add, replica_groups=RG,
                                     ins=[wd_b[:]], outs=[wds_b[:]])

        # ---- local Wd/dsq arrive directly via ReduceScatter ----
        dsql = lin.tile([BL, 1], F32, tag="dsql")
        nc.sync.dma_start(dsql, dsqs_b[:])
        wdl_bf = res.tile([BL, H], BF16, tag="wd_locb")
        nc.gpsimd.dma_start(wdl_bf, wds_b[:])
        wdlT_sb = work.tile([P, KT_H, BL], BF16, tag="wdlT")
        for kt in range(KT_H):
            tp2 = psum.tile([P, BL], BF16, tag="t_ps")
            pe_T(nc, tp2, wdl_bf[:, kt * P:(kt + 1) * P], identb)
            nc.vector.tensor_copy(wdlT_sb[:, kt, :], tp2)
        mwdT_sb = work.tile([P, KT_H, BL], BF16, tag="mwdT")
        nc.vector.tensor_tensor(mwdT_sb, wdlT_sb, m2T_sb, Alu.mult)
        # y[bl, n] = sum_h mwdT[h, bl] * w1T[h, n]  (t, already transposed)
        y_ps = psum.tile([BL, N], F32, tag="small_ps")
        for kt in range(KT_H):
            nc.tensor.matmul(y_ps, mwdT_sb[:, kt, :], w1T_sb[:, kt, :],
                             start=(kt == 0), stop=(kt == KT_H - 1))
        y = lin.tile([BL, N], F32, tag="y")
        nc.vector.tensor_copy(y, y_ps)
        # ---- solve G2 x = y by Jacobi-damped Richardson ----
        xs = lin.tile([BL, N], F32, tag="xs")
        gx = lin.tile([BL, N], F32, tag="gx")
        tmpv = lin.tile([BL, N], F32, tag="tmpv")
        nc.vector.tensor_tensor(xs, y, widg, Alu.mult)
        for _ in range(4):
            nc.vector.tensor_tensor(
                SCR.rearrange("p (a b) -> p a b", b=N),
                Tm2.rearrange("p (a b) -> p a b", b=N),
                xs.unsqueeze(1).broadcast_to([BL, N, N]), Alu.mult)
            nc.vector.tensor_reduce(
                gx, SCR.rearrange("p (a b) -> p a b", b=N),
                mybir.AxisListType.X, Alu.add)
            nc.vector.tensor_tensor(tmpv, y, gx, Alu.subtract)
            nc.vector.tensor_tensor(tmpv, tmpv, widg, Alu.mult)
            nc.vector.tensor_tensor(xs, xs, tmpv, Alu.add)
        yx = lin.tile([BL, N], F32, tag="yx")
        dproj = lin.tile([BL, 1], F32, tag="dproj")
        nc.vector.tensor_tensor(yx, y, xs, Alu.mult)
        nc.vector.tensor_reduce(dproj, yx, mybir.AxisListType.X, Alu.add)
        # Tm2 = W2SC^2 * G2, so x and hence dproj are 1/W2SC^2 scaled
        nc.vector.tensor_scalar(dproj, dproj, W2SC * W2SC, None, Alu.mult)
        nc.vector.tensor_reduce(trv, gSCR, mybir.AxisListType.X, Alu.add)

        # ---- recon / output (scalars precomputed during stage 2) ----
        recon = lin.tile([BL, 1], F32, tag="recon")
        nc.vector.tensor_tensor(recon, dproj, isub, Alu.mult)
        p2t = lin.tile([BL, 1], F32, tag="p2t")
        nc.vector.tensor_tensor(p2t, dsql, inv2[:, 1:2], Alu.mult)
        nc.vector.tensor_tensor(recon, recon, p2t, Alu.add)
        nc.vector.tensor_tensor(recon, recon, logterm, Alu.add)
        ov = lin.tile([BL, 1], F32, tag="ov")
        nc.vector.tensor_tensor(ov, latv, trv, Alu.add)
        nc.vector.tensor_scalar(ov, ov, 0.5, None, Alu.mult)
        nc.vector.tensor_tensor(ov, ov, recon, Alu.add)
        nc.vector.tensor_tensor(ov, ov, ldv, Alu.add)
        nc.vector.tensor_scalar(ov, ov, 1.0 / D, None, Alu.mult)
        nc.sync.dma_start(out[:], ov)

    legalize_waits(nc)
    return nc


def shard_inputs(inputs):
    """Host-side prep: returns in_maps list for the 8 cores."""
    bf = ml_dtypes.bfloat16
    x = np.ascontiguousarray(np.asarray(inputs["x"], np.float32))
    eps = np.ascontiguousarray(np.asarray(inputs["eps"], np.float32))
    eW1 = np.ascontiguousarray(np.asarray(inputs["enc_W1"], np.float32))
    eb1 = np.asarray(inputs["enc_b1"], np.float32)
    eW2 = np.ascontiguousarray(np.asarray(inputs["enc_W2"], np.float32))
    eb2 = np.asarray(inputs["enc_b2"], np.float32)
    dW1 = np.ascontiguousarray(np.asarray(inputs["dec_W1"], np.float32))
    db1 = np.asarray(inputs["dec_b1"], np.float32)
    dW2 = np.ascontiguousarray(np.asarray(inputs["dec_W2"], np.float32))
    db2 = np.asarray(inputs["dec_b2"], np.float32)
    sW = np.asarray(inputs["sig_W"], np.float32)
    sb = np.asarray(inputs["sig_b"], np.float32)

    xT = np.ascontiguousarray(x.T).astype(bf)
    xTp = np.ascontiguousarray(
        xT.reshape(KT_D, P, B).transpose(1, 0, 2).reshape(P, KT_D * B))
    dW2T = np.ascontiguousarray(dW2.T)
    dW1T = np.ascontiguousarray(dW1.T).astype(bf)
    dW1b = dW1.astype(bf)
    sigv = np.zeros((1, 130), np.float32)
    sigv[0, 0:32] = sW[:, 0]
    sigv[0, 32:64] = sW[:, 1]
    sigv[0, 64:66] = sb
    sigv[0, 66:98] = sW[:, 0] * np.sqrt(N / 2.0)
    sigv[0, 98:130] = sW[:, 1] * np.sqrt((D - N) / 2.0)

    maps = []
    for k in range(NCORES):
        sel = np.zeros((B, BL), np.float32)
        for i in range(BL):
            sel[k * BL + i, i] = 1.0
        w1s = np.ascontiguousarray(eW1[:, k * HS:(k + 1) * HS]).astype(bf)
        maps.append({
            "xTp": xTp,
            "xmb": np.ascontiguousarray(
                x[:, k * DS:(k + 1) * DS]
                - db2[None, k * DS:(k + 1) * DS]).astype(bf),
            "w1esp": np.ascontiguousarray(
                w1s.reshape(KT_D, P, HS).transpose(1, 0, 2)
                .reshape(P, KT_D * HS)),
            "b1es": np.ascontiguousarray(eb1[None, k * HS:(k + 1) * HS]).astype(bf),
            "w2es": np.ascontiguousarray(eW2[k * HS:(k + 1) * HS, :]).astype(bf),
            "b2e": np.ascontiguousarray(eb2[None, :]).astype(bf),
            "w2Ts": (np.ascontiguousarray(dW2T[k * DS:(k + 1) * DS, :]) * W2SC
                     ).astype(ml_dtypes.float8_e4m3fn),
            "w2s": np.ascontiguousarray(dW2[:, k * DS:(k + 1) * DS]).astype(bf),
            "w1": dW1b,
            "w1Td": dW1T,
            "b1d": np.ascontiguousarray(db1[None, :]).astype(bf),
            "sigw": sigv,
            "sel8": sel.astype(bf),
            "epsin": np.ascontiguousarray(eps[k * BL:(k + 1) * BL, :]),
        })
    return maps


_NC_CACHE = None


def kernel(**inputs) -> np.ndarray:
    global _NC_CACHE
    from concourse.bass_utils import run_bass_kernel_spmd
    if _NC_CACHE is None:
        _NC_CACHE = build_nc()
    nc = _NC_CACHE
    maps = shard_inputs(inputs)
    res = run_bass_kernel_spmd(nc, maps, list(range(NCORES)))
    outs = [res.results[k]["out"].reshape(BL) for k in range(NCORES)]
    return np.concatenate(outs).astype(np.float32)


# revision 82
# speedup vs baseline: 1.3150x; 1.1112x over previous
"""Trainium2 Bass kernel for nn_EnergyAE (B=64, D=12288, N=32, H=2048) on 8 cores.

Hybrid sharding, bf16 matmuls (fp32 vector math):
  phase E  (model-parallel over H): encoder -> z* partial (bf16, folded into
           C-chunk0 AllReduce below)
  phase C  (contraction-parallel over D): upper-triangular 512-col blocks of
           Cpart = w2Ts @ w2Ts^T, AllReduced in 4 bf16 chunks pipelined with
           the build; mirrored to full C in SBUF locally after readback.
           C stays SBUF-resident for both PG stages.
  phase S1 (data-parallel, 8 samples/core): A1 = W1*m1, P1T = C@A1T,
           G = P1T^T A1T (batched 4 samples/matmul), Prec, LDLT, Lt^-1, dz,
           tr, logdet, z_s = z* + dz -> AllGather z_s (8KB)
  phase S2 (model-parallel over D): h2, x_star slice, delta, d_sq,
           Wd = delta@W2s^T (w2Ts reused from SBUF) -> AllReduce [Wd | d_sq]
  phase S3 (data-parallel): t = W1T^T(m2*Wd), G2 = A2 C A2^T, LDLT2,
           fwd solve, d_proj_sq, recon -> out (8 per core)

Identities replacing eigvalsh/cholesky/solve_triangular:
  Prec = Lt D Lt^T (unit-lower LDLT)
  sum(log eig)/2 = 0.5*sum(log D);   sum(1/eig) = ||D^-1/2 Lt^-1||_F^2
  U^-1 eps = Lt^-T (eps/sqrt(D));    t^T G2^-1 t = ||D2^-1/2 Lt2^-1 t||^2
  sig_term = (n w0 w0^T + (D-n) w1 w1^T)/2   (constant across batch)
"""
import sys

for _p in ("/opt/trn_rl_repo", "/root/.axon_site/_ro/trn_rl_repo"):
    if _p not in sys.path:
        sys.path.append(_p)

import numpy as np
import ml_dtypes
from contextlib import ExitStack

import concourse.bass as bass
import concourse.mybir as mybir
import concourse.tile as tile
from concourse.masks import make_identity

B, D, N, H = 64, 12288, 32, 2048
NCORES = 8
BL = B // NCORES          # 8 local samples
HS = H // NCORES          # 256
DS = D // NCORES          # 1536
KT_H = H // 128           # 16
KT_D = D // 128           # 96
KT_DS = DS // 128         # 12
P = 128
# upper-triangular C slab layout: col-block q holds block-rows 0..4q+3
CUP_OFF = [0, 4, 12, 24]          # slab index offset per col-block
CUP_NROW = [4, 8, 12, 16]         # slabs per col-block
CUP_TOT = 40
ZROWS = 4                         # z* partial occupies cup rows 0..3 (4x512)

F32 = mybir.dt.float32
BF16 = mybir.dt.bfloat16
FP8 = mybir.dt.float8e4
W2SC = 8.0            # fp8 scale on w2Ts; C/G carry W2SC^2, Wd carries W2SC
DR = mybir.MatmulPerfMode.DoubleRow
Alu = mybir.AluOpType
Act = mybir.ActivationFunctionType
RG = [list(range(NCORES))]


def sub_ap(t, extra_off, dims):
    """Custom free-dim AP on a [P, F] tile; dims = [[step,count],...] in elems."""
    base = t[:, 0:1]
    return bass.AP(base.tensor, base.offset + extra_off, [base.ap[0]] + dims)


def pe_T(nc, out_ps, in_ap, ident):
    """PE transpose: out_ps [f, p] = in_ap [p, f].T"""
    kp = in_ap.shape[0]
    nc.tensor.transpose(out_ps, in_ap, ident[0:kp, 0:kp])


def emit_ldlt(nc, T, OUT, invD, n=32):
    """In-place unit-lower LDLT of T [BL, n*n] (row-major per sample).
    After: strict lower of T holds unscaled columns u; diag holds D; invD = 1/D."""
    for j in range(n):
        nc.vector.reciprocal(invD[:, j:j + 1], T[:, (n + 1) * j:(n + 1) * j + 1])
        m = n - 1 - j
        if m == 0:
            break
        base = (j + 1) * n + j
        u_i = sub_ap(T, base, [[n, m], [0, m]])
        u_k = sub_ap(T, base, [[0, m], [n, m]])
        outer = sub_ap(OUT, 0, [[m, m], [1, m]])
        nc.vector.scalar_tensor_tensor(
            outer, u_i, invD[:, j:j + 1], u_k, Alu.mult, Alu.mult)
        trail = sub_ap(T, (j + 1) * (n + 1), [[n, m], [1, m]])
        nc.vector.tensor_tensor(trail, trail, outer, Alu.subtract)


def emit_ltinv(eng, LT, X, OUT, n=32):
    """X = LT^{-1} for unit-lower LT [BL, n*n]; X preset to I by caller.
    Uses only tensor_tensor (runs on GpSimd, whose ISA lacks STT)."""
    for k in range(n - 1):
        rows = n - 1 - k
        cols = k + 1
        lcol = sub_ap(LT, (k + 1) * n + k, [[n, rows], [0, cols]])
        xrow = sub_ap(X, k * n, [[0, rows], [1, cols]])
        prod = sub_ap(OUT, 0, [[cols, rows], [1, cols]])
        eng.tensor_tensor(prod, lcol, xrow, Alu.mult)
        xblk = sub_ap(X, (k + 1) * n, [[n, rows], [1, cols]])
        eng.tensor_tensor(xblk, xblk, prod, Alu.subtract)


def emit_bwd_solve(nc, LT, y, OUT, n=32):
    """y <- LT^{-T} y for unit-lower LT [BL, n*n], y [BL, n] in place."""
    for k in range(n - 1, 0, -1):
        lrow = sub_ap(LT, k * n, [[1, k]])
        nc.vector.scalar_tensor_tensor(
            OUT[:, 0:k], lrow, -1.0, y[:, k:k + 1].broadcast_to([BL, k]),
            Alu.mult, Alu.mult)
        nc.vector.tensor_tensor(y[:, 0:k], y[:, 0:k], OUT[:, 0:k], Alu.add)


def emit_fwd_solve(nc, LT, y, OUT, n=32):
    """y <- LT^{-1} y for unit-lower LT [BL, n*n], y [BL, n] in place."""
    for k in range(n - 1):
        rows = n - 1 - k
        lcol = sub_ap(LT, (k + 1) * n + k, [[n, rows]])
        nc.vector.scalar_tensor_tensor(
            OUT[:, 0:rows], lcol, -1.0, y[:, k:k + 1].broadcast_to([BL, rows]),
            Alu.mult, Alu.mult)
        nc.vector.tensor_tensor(y[:, k + 1:n], y[:, k + 1:n], OUT[:, 0:rows], Alu.add)


def legalize_waits(nc, maxw=1):
    """Split multi-wait sync_info into standalone EventSemaphore instructions."""
    for f in nc.m.functions:
        for bb in f.blocks:
            insts = list(bb.instructions)
            out = []
            changed = False
            for inst in insts:
                si = inst.sync_info
                if si is not None and si.on_wait and len(si.on_wait) > maxw:
                    waits = list(si.on_wait)
                    imm = [w for w in waits if w.uses_immediate]
                    reg = [w for w in waits if not w.uses_immediate]
                    keep = (reg + imm)[:maxw] if len(reg) <= maxw else reg
                    extra = [w for w in waits if w not in keep]
                    if len(keep) > maxw:
                        raise RuntimeError(f"{inst.name}: {len(keep)} register waits")
                    for w in extra:
                        ev = mybir.InstEventSemaphore(
                            name=nc.get_next_instruction_name(), ins=[], outs=[])
                        ev.engine = inst.engine
                        ev.sync_info = mybir.SyncInfo(on_wait=[w], on_update=[])
                        out.append(ev)
                    inst.sync_info = mybir.SyncInfo(
                        on_wait=keep, on_update=list(si.on_update or []))
                    changed = True
                out.append(inst)
            if changed:
                bb.instructions = out
    return nc


def build_nc():
    nc = bass.Bass()

    # ---- I/O ----
    # xTp/w1esp: pre-packed partition-major [(p) (k b)] so DMA descriptors are
    # per-partition contiguous (12KB / 6KB) instead of 128B/512B strided
    xTp = nc.dram_tensor("xTp", [P, KT_D * B], BF16, kind="ExternalInput")
    xmb = nc.dram_tensor("xmb", [B, DS], BF16, kind="ExternalInput")
    w1esp = nc.dram_tensor("w1esp", [P, KT_D * HS], BF16, kind="ExternalInput")
    b1es = nc.dram_tensor("b1es", [1, HS], BF16, kind="ExternalInput")
    w2es = nc.dram_tensor("w2es", [HS, N], BF16, kind="ExternalInput")
    b2e = nc.dram_tensor("b2e", [1, N], BF16, kind="ExternalInput")
    w2Ts = nc.dram_tensor("w2Ts", [DS, H], FP8, kind="ExternalInput")
    w2s = nc.dram_tensor("w2s", [H, DS], BF16, kind="ExternalInput")
    w1 = nc.dram_tensor("w1", [N, H], BF16, kind="ExternalInput")
    w1Td = nc.dram_tensor("w1Td", [H, N], BF16, kind="ExternalInput")
    b1d = nc.dram_tensor("b1d", [1, H], BF16, kind="ExternalInput")
    sigw = nc.dram_tensor("sigw", [1, 130], F32, kind="ExternalInput")
    sel8 = nc.dram_tensor("sel8", [B, BL], BF16, kind="ExternalInput")
    epsin = nc.dram_tensor("epsin", [BL, N], F32, kind="ExternalInput")
    out = nc.dram_tensor("out", [BL, 1], F32, kind="ExternalOutput")

    # ---- internal DRAM ----
    # chunk q: upper-C col-block q slabs (+ z* partial rows in chunk 0).
    # all chunks bf16 (fp8 collectives produce NaN on this stack)
    CUPDT = [BF16, BF16, BF16, BF16]
    cup = [nc.dram_tensor(f"cup{q}", [(ZROWS if q == 0 else 0) + CUP_NROW[q] * P, 512],
                          CUPDT[q]) for q in range(4)]
    cup_sh = [nc.dram_tensor(f"cup_sh{q}", [(ZROWS if q == 0 else 0) + CUP_NROW[q] * P, 512],
                             CUPDT[q], addr_space="Shared") for q in range(4)]
    zs_b = nc.dram_tensor("zs_b", [BL, N], F32)
    zs_sh = nc.dram_tensor("zs_sh", [B, N], F32, addr_space="Shared")
    wd_b = nc.dram_tensor("wd_b", [B, H], BF16)
    wds_b = nc.dram_tensor("wds_b", [BL, H], BF16)
    dsq_b = nc.dram_tensor("dsq_b", [B, 1], F32)
    dsqs_b = nc.dram_tensor("dsqs_b", [BL, 1], F32)

    with tile.TileContext(nc) as tc, ExitStack() as ctx:
        consts = ctx.enter_context(tc.tile_pool(name="consts", bufs=1))
        work = ctx.enter_context(tc.tile_pool(name="work", bufs=2))
        stream = ctx.enter_context(tc.tile_pool(name="stream", bufs=3))
        psum = ctx.enter_context(tc.tile_pool(name="psum", bufs=2, space="PSUM"))
        psum_acc = ctx.enter_context(tc.tile_pool(name="psacc", bufs=1, space="PSUM"))
        lin = ctx.enter_context(tc.tile_pool(name="lin", bufs=1))
        res = ctx.enter_context(tc.tile_pool(name="res", bufs=1))
        encs = ctx.enter_context(tc.tile_pool(name="encs", bufs=2))
        w2sp = ctx.enter_context(tc.tile_pool(name="w2sp", bufs=6))

        # ---- constants / small loads ----
        identb = consts.tile([P, P], BF16)
        make_identity(nc, identb)
        ones1 = consts.tile([1, B], F32)
        nc.vector.memset(ones1, 1.0)
        onesb = consts.tile([1, B], BF16)
        nc.vector.memset(onesb, 1.0)
        sigw_sb = consts.tile([1, 130], F32)
        nc.sync.dma_start(sigw_sb, sigw[:])
        sigw_rep = consts.tile([BL, 130], F32)
        sigw_ps = psum.tile([BL, 130], F32, tag="small_ps")
        nc.tensor.matmul(sigw_ps, ones1[:, 0:BL], sigw_sb, start=True, stop=True)
        nc.vector.tensor_copy(sigw_rep, sigw_ps)
        sel8_sb = consts.tile([B, BL], BF16)
        nc.sync.dma_start(sel8_sb, sel8[:])
        eps_sb = consts.tile([BL, N], F32)
        nc.sync.dma_start(eps_sb, epsin[:])
        b1es_sb = consts.tile([1, HS], BF16)
        nc.sync.dma_start(b1es_sb, b1es[:])
        b2e_sb = consts.tile([1, N], BF16)
        nc.sync.dma_start(b2e_sb, b2e[:])
        # decoder bias as per-partition columns [P, KT_H] (+ negated copy)
        b1dcol = consts.tile([P, KT_H], BF16)
        nc.sync.dma_start(b1dcol, b1d[:].rearrange("o (k p) -> p (o k)", p=P))
        nb1col = consts.tile([P, KT_H], F32)
        nc.vector.tensor_scalar(nb1col, b1dcol, -1.0, None, Alu.mult)
        w1_sb = consts.tile([N, H], BF16)
        nc.sync.dma_start(w1_sb, w1[:])
        w1T_sb = consts.tile([P, KT_H, N], BF16)
        nc.sync.dma_start(w1T_sb, w1Td[:].rearrange("(k p) n -> p k n", p=P))

        # ---- resident weights: w2Ts fp8 x W2SC (used by phase C and Wd) ----
        w2Ts_sb = res.tile([P, KT_DS, H], FP8, tag="w2Ts")
        w2Ts_r = w2Ts[:].rearrange("(k p) h -> p k h", p=P)
        for kt in range(KT_DS):
            nc.sync.dma_start(w2Ts_sb[:, kt, :], w2Ts_r[:, kt, :])

        # ================= phase E: encoder (z* partial -> cup rows 0:4) ====
        a1_ps = psum_acc.tile([B, HS], F32, tag="acc")
        KSUP = 8  # k-tiles per packed super-chunk
        for kc in range(KT_D // KSUP):
            xp_t = encs.tile([P, KSUP, B], BF16, tag="xp_t")
            nc.sync.dma_start(
                xp_t, xTp[:, kc * KSUP * B:(kc + 1) * KSUP * B]
                .rearrange("p (k b) -> p k b", b=B))
            w1t = encs.tile([P, KSUP, HS], BF16, tag="w1es_t")
            nc.sync.dma_start(
                w1t, w1esp[:, kc * KSUP * HS:(kc + 1) * KSUP * HS]
                .rearrange("p (k h) -> p k h", h=HS))
            for kj in range(KSUP):
                kt = kc * KSUP + kj
                nc.tensor.matmul(a1_ps, xp_t[:, kj, :], w1t[:, kj, :],
                                 start=(kt == 0), stop=False)
        nc.tensor.matmul(a1_ps, onesb[:, 0:B], b1es_sb, start=False, stop=True)
        h1_sb = work.tile([B, HS], BF16, tag="h1")
        nc.vector.tensor_scalar(h1_sb, a1_ps, 0.0, None, Alu.max)
        h1T_sb = work.tile([P, 2, B], BF16, tag="h1T")
        for i in range(2):
            tp = psum.tile([P, B], BF16, tag="t_ps")
            pe_T(nc, tp, h1_sb[:, i * P:(i + 1) * P], identb)
            nc.vector.tensor_copy(h1T_sb[:, i, :], tp)
        w2es_sb = work.tile([P, 2, N], BF16, tag="w2es")
        nc.sync.dma_start(w2es_sb, w2es[:].rearrange("(k p) n -> p k n", p=P))
        zp_ps = psum.tile([B, N], F32, tag="small_ps")
        for i in range(2):
            nc.tensor.matmul(zp_ps, h1T_sb[:, i, :], w2es_sb[:, i, :],
                             start=(i == 0), stop=(i == 1))
        zp_sb = work.tile([B, N], BF16, tag="zstar_part")
        nc.vector.tensor_copy(zp_sb, zp_ps)
        nc.sync.dma_start(cup[0][0:ZROWS, :], zp_sb)

        # ========= phase C: upper C slabs + chunked AllReduce ==============
        # col-block q: cols [512q, 512q+512), block-rows j in 0..4q+3
        csb = res.tile([P, KT_H, H], BF16, tag="csb")
        for q in range(4):
            zr = ZROWS if q == 0 else 0
            for jg in range(q + 1):          # groups of 4 slabs
                cs = work.tile([P, 4, 512], CUPDT[q],
                               tag="c_out" if q == 0 else "c_out8")
                for jj in range(4):
                    j = jg * 4 + jj
                    cps = psum.tile([P, 512], F32, tag="big_ps")
                    for kd in range(KT_DS // 2):
                        nc.tensor.matmul(
                            cps,
                            w2Ts_sb[:, 2 * kd:2 * kd + 2, j * P:(j + 1) * P],
                            w2Ts_sb[:, 2 * kd:2 * kd + 2, 512 * q:512 * (q + 1)],
                            start=(kd == 0), stop=(kd == KT_DS // 2 - 1),
                            perf_mode=DR)
                    nc.scalar.copy(cs[:, jj, :], cps)
                r0 = zr + jg * 4 * P
                nc.sync.dma_start(
                    cup[q][r0:r0 + 4 * P, :].rearrange("(s p) c -> p s c", p=P),
                    cs)
            # AllReduce this chunk (chunk 0 also carries the z* partial)
            nc.gpsimd.collective_compute(
                "AllReduce", Alu.add, replica_groups=RG,
                ins=[cup[q][:]], outs=[cup_sh[q][:]])
            # readback on the gpsimd DMA queue right behind the AR so it
            # lands as soon as the chunk is reduced (SP queue is busy with
            # input streams / cup writes)
            if q == 0:
                zf_sb = work.tile([B, N], BF16, tag="z_full")
                nc.gpsimd.dma_start(zf_sb, cup_sh[0][0:ZROWS, :])
            zr_ = ZROWS if q == 0 else 0
            shr = cup_sh[q][zr_:, :].rearrange("(s p) c -> p s c", p=P)
            nc.gpsimd.dma_start(
                csb[:, 0:4 * q + 4, 512 * q:512 * (q + 1)], shr)

        # ---- z* post: local slice, sig1, masks, A1T (overlaps C build) ----
        zlT_ps = psum.tile([N, BL], F32, tag="small_ps")
        nc.tensor.matmul(zlT_ps, zf_sb, sel8_sb, start=True, stop=False)
        nc.tensor.matmul(zlT_ps, b2e_sb, onesb[:, 0:BL], start=False, stop=True)
        zlT_sb = work.tile([N, BL], BF16, tag="zlT")   # (z*loc + b2)^T
        nc.vector.tensor_copy(zlT_sb, zlT_ps)
        zloc_ps = psum.tile([BL, N], F32, tag="small_ps")
        nc.tensor.matmul(zloc_ps, sel8_sb, zf_sb, start=True, stop=False)
        nc.tensor.matmul(zloc_ps, onesb[:, 0:BL], b2e_sb, start=False, stop=True)
        zloc_sb = lin.tile([BL, N], F32, tag="z_loc")   # z* local + b2
        nc.vector.tensor_copy(zloc_sb, zloc_ps)

        def emit_sig(z_loc, name):
            lg = lin.tile([BL, 2, 32], F32, tag="sig_lg")
            nc.vector.tensor_tensor(
                lg, z_loc.unsqueeze(1).broadcast_to([BL, 2, 32]),
                sigw_rep[:, 0:64].rearrange("p (c n) -> p c n", c=2), Alu.mult)
            red = lin.tile([BL, 2], F32, tag=f"sig_red_{name}")
            nc.vector.tensor_reduce(red, lg, mybir.AxisListType.X, Alu.add)
            nc.vector.tensor_tensor(red, red, sigw_rep[:, 64:66], Alu.add)
            s = lin.tile([BL, 2], F32, tag=f"sig_s_{name}")
            nc.scalar.activation(s, red, Act.Exp)
            return s

        s1 = emit_sig(zloc_sb, "s1")
        invsp2 = lin.tile([BL, 1], F32, tag="invsp2")
        sp2t = lin.tile([BL, 1], F32, tag="sp2t")
        nc.vector.tensor_tensor(sp2t, s1[:, 0:1], s1[:, 0:1], Alu.mult)
        # G arrives scaled by W2SC^2 (fp8 weights); fold 1/W2SC^2 in here
        nc.vector.tensor_scalar(sp2t, sp2t, W2SC * W2SC, None, Alu.mult)
        nc.vector.reciprocal(invsp2, sp2t)

        # a1T (local) -> mask m1T [P, KT_H, BL] -> A1T  (mask: a1 > -b1)
        m1T_sb = work.tile([P, KT_H, BL], BF16, tag="m1T")
        for mt in range(KT_H):
            aps = psum.tile([P, BL], F32, tag="small_ps")
            nc.tensor.matmul(aps, w1_sb[:, mt * P:(mt + 1) * P],
                             zlT_sb, start=True, stop=True)
            nc.vector.tensor_tensor(
                m1T_sb[:, mt, :], aps,
                nb1col[:, mt:mt + 1].broadcast_to([P, BL]), Alu.is_gt)
        AT_sb = res.tile([P, KT_H, BL, N], BF16, tag="AT")
        nc.vector.tensor_tensor(
            AT_sb,
            w1T_sb.unsqueeze(2).broadcast_to([P, KT_H, BL, N]),
            m1T_sb.unsqueeze(3).broadcast_to([P, KT_H, BL, N]), Alu.mult)

        # ---- chunk-pipelined P1T = C @ A1T: each col-chunk's mirrors and PT
        # contributions run as soon as its AllReduce lands (PE overlaps ARs) --
        PT1 = res.tile([P, KT_H, BL * N], BF16, tag="PT")
        for q in range(4):
            # mirrors sourced from chunk q: targets (i in stripe q, j < 4q)
            for i in range(4 * q, 4 * q + 4):
                for j in range(4 * q):
                    tp = psum.tile([P, P], BF16, tag="t_ps")
                    pe_T(nc, tp, csb[:, j, i * P:(i + 1) * P], identb)
                    nc.scalar.copy(csb[:, i, j * P:(j + 1) * P], tp)
            # (a) fold the new kt-stripe q into PT[mt] for mt < 4q
            for mt in range(4 * q):
                pps = psum.tile([P, BL * N], F32, tag="big_ps")
                for kt in range(4 * q, 4 * q + 4):
                    nc.tensor.matmul(pps, csb[:, kt, mt * P:(mt + 1) * P],
                                     AT_sb[:, kt, :, :],
                                     start=(kt == 4 * q), stop=(kt == 4 * q + 3))
                nc.vector.tensor_tensor(PT1[:, mt, :], PT1[:, mt, :], pps,
                                        Alu.add)
            # (b) initialize PT[mt] for mt in stripe q (kt 0..4q+3 available)
            for mt in range(4 * q, 4 * q + 4):
                pps = psum.tile([P, BL * N], F32, tag="big_ps")
                for kt in range(4 * q + 4):
                    nc.tensor.matmul(pps, csb[:, kt, mt * P:(mt + 1) * P],
                                     AT_sb[:, kt, :, :],
                                     start=(kt == 0), stop=(kt == 4 * q + 3))
                nc.scalar.copy(PT1[:, mt, :], pps)



        # ---- P*T = C @ A*T ; G = P*T^T A*T  (C resident in SBUF) ----
        def emit_PG(AT, tag):
            PT_sb = res.tile([P, KT_H, BL * N], BF16, tag="PT")
            for mt in range(KT_H):
                pps = psum.tile([P, BL * N], F32, tag="big_ps")
                for kt in range(KT_H):
                    nc.tensor.matmul(
                        pps, csb[:, kt, mt * P:(mt + 1) * P], AT[:, kt, :, :],
                        start=(kt == 0), stop=(kt == KT_H - 1))
                nc.scalar.copy(PT_sb[:, mt, :], pps)
            g_sb = work.tile([P, 2, P], F32, tag="g_sb")
            for grp in range(2):
                g_ps = psum.tile([P, P], F32, tag="big_ps")
                for kt in range(KT_H):
                    nc.tensor.matmul(
                        g_ps,
                        PT_sb[:, kt, grp * P:(grp + 1) * P],
                        AT[:, kt, 4 * grp:4 * grp + 4, :],
                        start=(kt == 0), stop=(kt == KT_H - 1))
                nc.vector.tensor_copy(g_sb[:, grp, :], g_ps)
            return g_sb

        # ---- Prec = G*invsp2 + sig_term + I ----
        # preset Tm with replicated sig_term while PG runs
        st_ps = psum.tile([N, N], F32, tag="small_ps")
        nc.tensor.matmul(st_ps, sigw_sb[:, 66:98], sigw_sb[:, 66:98],
                         start=True, stop=False)
        nc.tensor.matmul(st_ps, sigw_sb[:, 98:130], sigw_sb[:, 98:130],
                         start=False, stop=True)
        st_sb = work.tile([N, N], F32, tag="st_sb")
        nc.vector.tensor_copy(st_sb, st_ps)
        Tm = lin.tile([BL, N * N], F32, tag="Tmat")
        for s in range(BL):
            nc.sync.dma_start(Tm[s:s + 1, :], st_sb)
        # add diag I up front
        diag1 = sub_ap(Tm, 0, [[N + 1, N]])
        nc.vector.tensor_scalar(diag1, diag1, 1.0, None, Alu.add)

        g_sb = work.tile([P, 2, P], F32, tag="g_sb")
        for grp in range(2):
            g_ps = psum.tile([P, P], F32, tag="big_ps")
            for kt in range(KT_H):
                nc.tensor.matmul(
                    g_ps, PT1[:, kt, grp * P:(grp + 1) * P],
                    AT_sb[:, kt, 4 * grp:4 * grp + 4, :],
                    start=(kt == 0), stop=(kt == KT_H - 1))
            nc.vector.tensor_copy(g_sb[:, grp, :], g_ps)
        SCR = lin.tile([BL, N * N], F32, tag="scr")
        for s in range(BL):
            grp, sl = s // 4, s % 4
            nc.sync.dma_start(
                SCR[s:s + 1, :],
                g_sb[sl * N:(sl + 1) * N, grp, sl * N:(sl + 1) * N])
        nc.vector.scalar_tensor_tensor(Tm, SCR, invsp2, Tm, Alu.mult, Alu.add)

        # ---- LDLT, dz (backward solve; Lt^-1/tr deferred off critical path) ----
        invD = lin.tile([BL, N], F32, tag="invD")
        emit_ldlt(nc, Tm, SCR, invD)
        LT = lin.tile([BL, N * N], F32, tag="LTmat")
        nc.vector.tensor_tensor(
            LT.rearrange("p (a b) -> p a b", b=N),
            Tm.rearrange("p (a b) -> p a b", b=N),
            invD.unsqueeze(1).broadcast_to([BL, N, N]), Alu.mult)
        srD = lin.tile([BL, N], F32, tag="srD")
        nc.scalar.activation(srD, invD, Act.Sqrt)        # 1/sqrt(D)
        epss = lin.tile([BL, N], F32, tag="epss")
        nc.vector.tensor_tensor(epss, eps_sb, srD, Alu.mult)
        emit_bwd_solve(nc, LT, epss, SCR)                # epss <- Lt^-T epss = dz
        zs_loc = lin.tile([BL, N], F32, tag="zs_loc")
        nc.vector.tensor_tensor(zs_loc, zloc_sb, epss, Alu.add)
        nc.sync.dma_start(zs_b[:], zs_loc)
        nc.gpsimd.collective_compute("AllGather", Alu.bypass, replica_groups=RG,
                                     ins=[zs_b[:]], outs=[zs_sh[:]])

        # ---- tr(Prec^-1) via Lt^-1 on GpSimd (parallel with stage 2) ----
        X1 = lin.tile([BL, N * N], F32, tag="X1")
        nc.gpsimd.memset(X1, 0.0)
        nc.gpsimd.memset(sub_ap(X1, 0, [[N + 1, N]]), 1.0)
        gSCR = lin.tile([BL, N * N], F32, tag="gSCR")
        emit_ltinv(nc.gpsimd, LT, X1, gSCR)
        trv = lin.tile([BL, 1], F32, tag="trv")
        nc.gpsimd.tensor_tensor(
            gSCR.rearrange("p (a b) -> p a b", b=N),
            X1.rearrange("p (a b) -> p a b", b=N),
            invD.unsqueeze(2).broadcast_to([BL, N, N]), Alu.mult)
        nc.gpsimd.tensor_tensor(gSCR, gSCR, X1, Alu.mult)
        # final free-axis reduce of gSCR into trv happens on vector at the tail

        # ---- z*-only reductions (vector, overlap AllGather) ----
        logs = lin.tile([BL, N], F32, tag="logs")
        ldv = lin.tile([BL, 1], F32, tag="ldv")
        nc.scalar.activation(logs, invD, Act.Ln)
        nc.vector.tensor_reduce(ldv, logs, mybir.AxisListType.X, Alu.add)  # -sum log D
        nc.vector.tensor_scalar(ldv, ldv, -0.5, None, Alu.mult)
        zsq = lin.tile([BL, N], F32, tag="zsq")
        latv = lin.tile([BL, 1], F32, tag="latv")
        nc.vector.tensor_tensor(zsq, zloc_sb, zloc_sb, Alu.mult)
        nc.vector.tensor_reduce(latv, zsq, mybir.AxisListType.X, Alu.add)
        # s2-dependent scalars (zs_loc known before AG returns)
        s2 = emit_sig(zs_loc, "s2")
        sq2 = lin.tile([BL, 2], F32, tag="sq2")
        nc.vector.tensor_tensor(sq2, s2, s2, Alu.mult)
        nc.vector.tensor_scalar(sq2, sq2, 2.0, None, Alu.mult)
        inv2 = lin.tile([BL, 2], F32, tag="inv2")
        nc.vector.reciprocal(inv2, sq2)     # [1/(2sp2^2), 1/(2sv2^2)]
        logs2 = lin.tile([BL, 2], F32, tag="logs2")
        logw = lin.tile([BL, 2], F32, tag="logw")
        nc.vector.memset(logw[:, 0:1], float(N))
        nc.vector.memset(logw[:, 1:2], float(D - N))
        nc.scalar.activation(logs2, s2, Act.Ln)
        logterm = lin.tile([BL, 1], F32, tag="logterm")
        junk2 = lin.tile([BL, 2], F32, tag="junk2")
        nc.vector.tensor_tensor(junk2, logs2, logw, Alu.mult)
        nc.vector.tensor_reduce(logterm, junk2, mybir.AxisListType.X, Alu.add)
        isub = lin.tile([BL, 1], F32, tag="isub")
        nc.vector.tensor_tensor(isub, inv2[:, 0:1], inv2[:, 1:2], Alu.subtract)

        # ---- stage 2 prep: h2T (all), m2T (local), A2T ----
        zsf_sb = work.tile([B, N], F32, tag="z_full2")
        nc.sync.dma_start(zsf_sb, zs_sh[:])
        zsf_bf = work.tile([B, N], BF16, tag="z_full2b")
        nc.vector.tensor_copy(zsf_bf, zsf_sb)
        zs_bf = lin.tile([BL, N], BF16, tag="zs_locb")
        nc.vector.tensor_copy(zs_bf, zs_loc)
        zsT_ps = psum.tile([N, B], BF16, tag="t_ps")
        pe_T(nc, zsT_ps, zsf_bf, identb)
        zsT_sb = work.tile([N, B], BF16, tag="zT2")
        nc.vector.tensor_copy(zsT_sb, zsT_ps)
        zslT_ps = psum.tile([N, BL], BF16, tag="t_ps")
        pe_T(nc, zslT_ps, zs_bf, identb)
        zslT_sb = work.tile([N, BL], BF16, tag="zlT2")
        nc.vector.tensor_copy(zslT_sb, zslT_ps)

        h2T_sb = res.tile([P, KT_H, B], BF16, tag="h2T")
        for mt in range(KT_H):
            aps = psum.tile([P, B], F32, tag="small_ps")
            nc.tensor.matmul(aps, w1_sb[:, mt * P:(mt + 1) * P],
                             zsT_sb, start=True, stop=True)
            nc.scalar.activation(h2T_sb[:, mt, :], aps, Act.Relu,
                                 bias=b1dcol[:, mt:mt + 1])

        m2T_sb = work.tile([P, KT_H, BL], BF16, tag="m2T")
        for mt in range(KT_H):
            aps = psum.tile([P, BL], F32, tag="small_ps")
            nc.tensor.matmul(aps, w1_sb[:, mt * P:(mt + 1) * P],
                             zslT_sb, start=True, stop=True)
            nc.vector.tensor_tensor(
                m2T_sb[:, mt, :], aps,
                nb1col[:, mt:mt + 1].broadcast_to([P, BL]), Alu.is_gt)
        AT2_sb = res.tile([P, KT_H, BL, N], BF16, tag="AT")   # reuse slot
        nc.vector.tensor_tensor(
            AT2_sb,
            w1T_sb.unsqueeze(2).broadcast_to([P, KT_H, BL, N]),
            m2T_sb.unsqueeze(3).broadcast_to([P, KT_H, BL, N]), Alu.mult)

        # ---- G2 on PE first: fills PE while the w2s stream for x_star runs --
        g2_sb = emit_PG(AT2_sb, "2")
        Tm2 = lin.tile([BL, N * N], F32, tag="Tmat")   # reuse slot
        for s in range(BL):
            grp, sl = s // 4, s % 4
            nc.sync.dma_start(
                Tm2[s:s + 1, :],
                g2_sb[sl * N:(sl + 1) * N, grp, sl * N:(sl + 1) * N])
        # Jacobi weights for the Richardson solve (no factorization needed)
        dg2 = lin.tile([BL, N], F32, tag="dg2")
        nc.vector.tensor_copy(dg2, sub_ap(Tm2, 0, [[N + 1, N]]))
        widg = lin.tile([BL, N], F32, tag="widg")
        nc.vector.reciprocal(widg, dg2)
        nc.vector.tensor_scalar(widg, widg, 0.9, None, Alu.mult)

        # ---- x_star slice, delta, d_sq, Wd (w2Ts from SBUF) ----
        d_sb = res.tile([B, DS], BF16, tag="d_sb")
        w2s_r = w2s[:].rearrange("(k p) ds -> p k ds", p=P)
        for nb in range(3):
            xmb_t = stream.tile([B, 512], BF16, tag="xmb_t")
            nc.sync.dma_start(xmb_t, xmb[:, nb * 512:(nb + 1) * 512])
            xs_ps = psum.tile([B, 512], F32, tag="big_ps")
            for kt in range(KT_H):
                wt = w2sp.tile([P, 512], BF16, tag="w2s_t")
                nc.sync.dma_start(wt, w2s_r[:, kt, nb * 512:(nb + 1) * 512])
                nc.tensor.matmul(xs_ps, h2T_sb[:, kt, :], wt,
                                 start=(kt == 0), stop=(kt == KT_H - 1))
            nc.vector.tensor_tensor(d_sb[:, nb * 512:(nb + 1) * 512], xmb_t,
                                    xs_ps, Alu.subtract)
        dT_bf = res.tile([P, KT_DS, B], BF16, tag="dTb")
        for kt in range(KT_DS):
            tp = psum.tile([P, B], BF16, tag="t_ps")
            pe_T(nc, tp, d_sb[:, kt * P:(kt + 1) * P], identb)
            nc.vector.tensor_copy(dT_bf[:, kt, :], tp)
        dT8 = res.tile([P, KT_DS, B], FP8, tag="dT")
        nc.scalar.copy(dT8, dT_bf)
        dsq_sb = work.tile([B, 1], F32, tag="dsq")
        # d_sq = rowsum(delta^2); squares written in place (d_sb dead after d8)
        nc.scalar.activation(d_sb, d_sb, Act.Square, accum_out=dsq_sb)
        nc.sync.dma_start(dsq_b[:], dsq_sb)
        nc.gpsimd.collective_compute("ReduceScatter", Alu.add, replica_groups=RG,
                                     ins=[dsq_b[:]], outs=[dsqs_b[:]])
        wd_sb = res.tile([B, H], BF16, tag="wd")
        for mb in range(4):
            wd_ps = psum.tile([B, 512], F32, tag="big_ps")
            for kd in range(KT_DS // 2):
                nc.tensor.matmul(wd_ps, dT8[:, 2 * kd:2 * kd + 2, :],
                                 w2Ts_sb[:, 2 * kd:2 * kd + 2,
                                         mb * 512:(mb + 1) * 512],
                                 start=(kd == 0), stop=(kd == KT_DS // 2 - 1),
                                 perf_mode=DR)
            nc.vector.tensor_scalar(wd_sb[:, mb * 512:(mb + 1) * 512], wd_ps,
                                    1.0 / W2SC, None, Alu.mult)
        nc.sync.dma_start(wd_b[:], wd_sb)
        nc.gpsimd.collective_compute("ReduceScatter", Alu.add, replica_groups=RG,
                                     ins=[wd_b[:]], outs=[wds_b[:]])

        # ---- local Wd/dsq arrive directly via ReduceScatter ----
        dsql = lin.tile([BL, 1], F32, tag="dsql")
        nc.sync.dma_start(dsql, dsqs_b[:])
        wdl_bf = res.tile([BL, H], BF16, tag="wd_locb")
        nc.gpsimd.dma_start(wdl_bf, wds_b[:])
        wdlT_sb = work.tile([P, KT_H, BL], BF16, tag="wdlT")
        for kt in range(KT_H):
            tp2 = psum.tile([P, BL], BF16, tag="t_ps")
            pe_T(nc, tp2, wdl_bf[:, kt * P:(kt + 1) * P], identb)
            nc.vector.tensor_copy(wdlT_sb[:, kt, :], tp2)
        mwdT_sb = work.tile([P, KT_H, BL], BF16, tag="mwdT")
        nc.vector.tensor_tensor(mwdT_sb, wdlT_sb, m2T_sb, Alu.mult)
        # y[bl, n] = sum_h mwdT[h, bl] * w1T[h, n]  (t, already transposed)
        y_ps = psum.tile([BL, N], F32, tag="small_ps")
        for kt in range(KT_H):
            nc.tensor.matmul(y_ps, mwdT_sb[:, kt, :], w1T_sb[:, kt, :],
                             start=(kt == 0), stop=(kt == KT_H - 1))
        y = lin.tile([BL, N], F32, tag="y")
        nc.vector.tensor_copy(y, y_ps)
        # ---- solve G2 x = y by Jacobi-damped Richardson ----
        xs = lin.tile([BL, N], F32, tag="xs")
        gx = lin.tile([BL, N], F32, tag="gx")
        tmpv = lin.tile([BL, N], F32, tag="tmpv")
        nc.vector.tensor_tensor(xs, y, widg, Alu.mult)
        for _ in range(4):
            nc.vector.tensor_tensor(
                SCR.rearrange("p (a b) -> p a b", b=N),
                Tm2.rearrange("p (a b) -> p a b", b=N),
                xs.unsqueeze(1).broadcast_to([BL, N, N]), Alu.mult)
            nc.vector.tensor_reduce(
                gx, SCR.rearrange("p (a b) -> p a b", b=N),
                mybir.AxisListType.X, Alu.add)
            nc.vector.tensor_tensor(tmpv, y, gx, Alu.subtract)
            nc.vector.tensor_tensor(tmpv, tmpv, widg, Alu.mult)
            nc.vector.tensor_tensor(xs, xs, tmpv, Alu.add)
        yx = lin.tile([BL, N], F32, tag="yx")
        dproj = lin.tile([BL, 1], F32, tag="dproj")
        nc.vector.tensor_tensor(yx, y, xs, Alu.mult)
        nc.vector.tensor_reduce(dproj, yx, mybir.AxisListType.X, Alu.add)
        # Tm2 = W2SC^2 * G2, so x and hence dproj are 1/W2SC^2 scaled
        nc.vector.tensor_scalar(dproj, dproj, W2SC * W2SC, None, Alu.mult)
        nc.vector.tensor_reduce(trv, gSCR, mybir.AxisListType.X, Alu.add)

        # ---- recon / output (scalars precomputed during stage 2) ----
        recon = lin.tile([BL, 1], F32, tag="recon")
        nc.vector.tensor_tensor(recon, dproj, isub, Alu.mult)
        p2t = lin.tile([BL, 1], F32, tag="p2t")
        nc.vector.tensor_tensor(p2t, dsql, inv2[:, 1:2], Alu.mult)
        nc.vector.tensor_tensor(recon, recon, p2t, Alu.add)
        nc.vector.tensor_tensor(recon, recon, logterm, Alu.add)
        ov = lin.tile([BL, 1], F32, tag="ov")
        nc.vector.tensor_tensor(ov, latv, trv, Alu.add)
        nc.vector.tensor_scalar(ov, ov, 0.5, None, Alu.mult)
        nc.vector.tensor_tensor(ov, ov, recon, Alu.add)
        nc.vector.tensor_tensor(ov, ov, ldv, Alu.add)
        nc.vector.tensor_scalar(ov, ov, 1.0 / D, None, Alu.mult)
        nc.sync.dma_start(out[:], ov)

    legalize_waits(nc)
    return nc


def shard_inputs(inputs):
    """Host-side prep: returns in_maps list for the 8 cores."""
    bf = ml_dtypes.bfloat16
    x = np.ascontiguousarray(np.asarray(inputs["x"], np.float32))
    eps = np.ascontiguousarray(np.asarray(inputs["eps"], np.float32))
    eW1 = np.ascontiguousarray(np.asarray(inputs["enc_W1"], np.float32))
    eb1 = np.asarray(inputs["enc_b1"], np.float32)
    eW2 = np.ascontiguousarray(np.asarray(inputs["enc_W2"], np.float32))
    eb2 = np.asarray(inputs["enc_b2"], np.float32)
    dW1 = np.ascontiguousarray(np.asarray(inputs["dec_W1"], np.float32))
    db1 = np.asarray(inputs["dec_b1"], np.float32)
    dW2 = np.ascontiguousarray(np.asarray(inputs["dec_W2"], np.float32))
    db2 = np.asarray(inputs["dec_b2"], np.float32)
    sW = np.asarray(inputs["sig_W"], np.float32)
    sb = np.asarray(inputs["sig_b"], np.float32)

    xT = np.ascontiguousarray(x.T).astype(bf)
    xTp = np.ascontiguousarray(
        xT.reshape(KT_D, P, B).transpose(1, 0, 2).reshape(P, KT_D * B))
    dW2T = np.ascontiguousarray(dW2.T)
    dW1T = np.ascontiguousarray(dW1.T).astype(bf)
    dW1b = dW1.astype(bf)
    sigv = np.zeros((1, 130), np.float32)
    sigv[0, 0:32] = sW[:, 0]
    sigv[0, 32:64] = sW[:, 1]
    sigv[0, 64:66] = sb
    sigv[0, 66:98] = sW[:, 0] * np.sqrt(N / 2.0)
    sigv[0, 98:130] = sW[:, 1] * np.sqrt((D - N) / 2.0)

    maps = []
    for k in range(NCORES):
        sel = np.zeros((B, BL), np.float32)
        for i in range(BL):
            sel[k * BL + i, i] = 1.0
        w1s = np.ascontiguousarray(eW1[:, k * HS:(k + 1) * HS]).astype(bf)
        maps.append({
            "xTp": xTp,
            "xmb": np.ascontiguousarray(
                x[:, k * DS:(k + 1) * DS]
                - db2[None, k * DS:(k + 1) * DS]).astype(bf),
            "w1esp": np.ascontiguousarray(
                w1s.reshape(KT_D, P, HS).transpose(1, 0, 2)
                .reshape(P, KT_D * HS)),
            "b1es": np.ascontiguousarray(eb1[None, k * HS:(k + 1) * HS]).astype(bf),
            "w2es": np.ascontiguousarray(eW2[k * HS:(k + 1) * HS, :]).astype(bf),
            "b2e": np.ascontiguousarray(eb2[None, :]).astype(bf),
            "w2Ts": (np.ascontiguousarray(dW2T[k * DS:(k + 1) * DS, :]) * W2SC
                     ).astype(ml_dtypes.float8_e4m3fn),
            "w2s": np.ascontiguousarray(dW2[:, k * DS:(k + 1) * DS]).astype(bf),
            "w1": dW1b,
            "w1Td": dW1T,
            "b1d": np.ascontiguousarray(db1[None, :]).astype(bf),
            "sigw": sigv,
            "sel8": sel.astype(bf),
            "epsin": np.ascontiguousarray(eps[k * BL:(k + 1) * BL, :]),
        })
    return maps


_NC_CACHE = None


def kernel(**inputs) -> np.ndarray:
    global _NC_CACHE
    from concourse.bass_utils import run_bass_kernel_spmd
    if _NC_CACHE is None:
        _NC_CACHE = build_nc()
    nc = _NC_CACHE
    maps = shard_inputs(inputs)
    res = run_bass_kernel_spmd(nc, maps, list(range(NCORES)))
    outs = [res.results[k]["out"].reshape(BL) for k in range(NCORES)]
    return np.concatenate(outs).astype(np.float32)


# revision 84
# speedup vs baseline: 1.3153x; 1.0003x over previous
"""Trainium2 Bass kernel for nn_EnergyAE (B=64, D=12288, N=32, H=2048) on 8 cores.

Hybrid sharding, bf16 matmuls (fp32 vector math):
  phase E  (model-parallel over H): encoder -> z* partial (bf16, folded into
           C-chunk0 AllReduce below)
  phase C  (contraction-parallel over D): upper-triangular 512-col blocks of
           Cpart = w2Ts @ w2Ts^T, AllReduced in 4 bf16 chunks pipelined with
           the build; mirrored to full C in SBUF locally after readback.
           C stays SBUF-resident for both PG stages.
  phase S1 (data-parallel, 8 samples/core): A1 = W1*m1, P1T = C@A1T,
           G = P1T^T A1T (batched 4 samples/matmul), Prec, LDLT, Lt^-1, dz,
           tr, logdet, z_s = z* + dz -> AllGather z_s (8KB)
  phase S2 (model-parallel over D): h2, x_star slice, delta, d_sq,
           Wd = delta@W2s^T (w2Ts reused from SBUF) -> AllReduce [Wd | d_sq]
  phase S3 (data-parallel): t = W1T^T(m2*Wd), G2 = A2 C A2^T, LDLT2,
           fwd solve, d_proj_sq, recon -> out (8 per core)

Identities replacing eigvalsh/cholesky/solve_triangular:
  Prec = Lt D Lt^T (unit-lower LDLT)
  sum(log eig)/2 = 0.5*sum(log D);   sum(1/eig) = ||D^-1/2 Lt^-1||_F^2
  U^-1 eps = Lt^-T (eps/sqrt(D));    t^T G2^-1 t = ||D2^-1/2 Lt2^-1 t||^2
  sig_term = (n w0 w0^T + (D-n) w1 w1^T)/2   (constant across batch)
"""
import sys

for _p in ("/opt/trn_rl_repo", "/root/.axon_site/_ro/trn_rl_repo"):
    if _p not in sys.path:
        sys.path.append(_p)

import numpy as np
import ml_dtypes
from contextlib import ExitStack

import concourse.bass as bass
import concourse.mybir as mybir
import concourse.tile as tile
from concourse.masks import make_identity

B, D, N, H = 64, 12288, 32, 2048
NCORES = 8
BL = B // NCORES          # 8 local samples
HS = H // NCORES          # 256
DS = D // NCORES          # 1536
KT_H = H // 128           # 16
KT_D = D // 128           # 96
KT_DS = DS // 128         # 12
P = 128
# upper-triangular C slab layout: col-block q holds block-rows 0..4q+3
CUP_OFF = [0, 4, 12, 24]          # slab index offset per col-block
CUP_NROW = [4, 8, 12, 16]         # slabs per col-block
CUP_TOT = 40
ZROWS = 4                         # z* partial occupies cup rows 0..3 (4x512)

F32 = mybir.dt.float32
BF16 = mybir.dt.bfloat16
FP8 = mybir.dt.float8e4
W2SC = 8.0            # fp8 scale on w2Ts; C/G carry W2SC^2, Wd carries W2SC
DR = mybir.MatmulPerfMode.DoubleRow
Alu = mybir.AluOpType
Act = mybir.ActivationFunctionType
RG = [list(range(NCORES))]


def sub_ap(t, extra_off, dims):
    """Custom free-dim AP on a [P, F] tile; dims = [[step,count],...] in elems."""
    base = t[:, 0:1]
    return bass.AP(base.tensor, base.offset + extra_off, [base.ap[0]] + dims)


def pe_T(nc, out_ps, in_ap, ident):
    """PE transpose: out_ps [f, p] = in_ap [p, f].T"""
    kp = in_ap.shape[0]
    nc.tensor.transpose(out_ps, in_ap, ident[0:kp, 0:kp])


def emit_ldlt(nc, T, OUT, invD, n=32):
    """In-place unit-lower LDLT of T [BL, n*n] (row-major per sample).
    After: strict lower of T holds unscaled columns u; diag holds D; invD = 1/D."""
    for j in range(n):
        nc.vector.reciprocal(invD[:, j:j + 1], T[:, (n + 1) * j:(n + 1) * j + 1])
        m = n - 1 - j
        if m == 0:
            break
        base = (j + 1) * n + j
        u_i = sub_ap(T, base, [[n, m], [0, m]])
        u_k = sub_ap(T, base, [[0, m], [n, m]])
        outer = sub_ap(OUT, 0, [[m, m], [1, m]])
        nc.vector.scalar_tensor_tensor(
            outer, u_i, invD[:, j:j + 1], u_k, Alu.mult, Alu.mult)
        trail = sub_ap(T, (j + 1) * (n + 1), [[n, m], [1, m]])
        nc.vector.tensor_tensor(trail, trail, outer, Alu.subtract)


def emit_ltinv(eng, LT, X, OUT, n=32):
    """X = LT^{-1} for unit-lower LT [BL, n*n]; X preset to I by caller.
    Uses only tensor_tensor (runs on GpSimd, whose ISA lacks STT)."""
    for k in range(n - 1):
        rows = n - 1 - k
        cols = k + 1
        lcol = sub_ap(LT, (k + 1) * n + k, [[n, rows], [0, cols]])
        xrow = sub_ap(X, k * n, [[0, rows], [1, cols]])
        prod = sub_ap(OUT, 0, [[cols, rows], [1, cols]])
        eng.tensor_tensor(prod, lcol, xrow, Alu.mult)
        xblk = sub_ap(X, (k + 1) * n, [[n, rows], [1, cols]])
        eng.tensor_tensor(xblk, xblk, prod, Alu.subtract)


def emit_bwd_solve(nc, LT, y, OUT, n=32):
    """y <- LT^{-T} y for unit-lower LT [BL, n*n], y [BL, n] in place."""
    for k in range(n - 1, 0, -1):
        lrow = sub_ap(LT, k * n, [[1, k]])
        nc.vector.scalar_tensor_tensor(
            OUT[:, 0:k], lrow, -1.0, y[:, k:k + 1].broadcast_to([BL, k]),
            Alu.mult, Alu.mult)
        nc.vector.tensor_tensor(y[:, 0:k], y[:, 0:k], OUT[:, 0:k], Alu.add)


def emit_fwd_solve(nc, LT, y, OUT, n=32):
    """y <- LT^{-1} y for unit-lower LT [BL, n*n], y [BL, n] in place."""
    for k in range(n - 1):
        rows = n - 1 - k
        lcol = sub_ap(LT, (k + 1) * n + k, [[n, rows]])
        nc.vector.scalar_tensor_tensor(
            OUT[:, 0:rows], lcol, -1.0, y[:, k:k + 1].broadcast_to([BL, rows]),
            Alu.mult, Alu.mult)
        nc.vector.tensor_tensor(y[:, k + 1:n], y[:, k + 1:n], OUT[:, 0:rows], Alu.add)


def legalize_waits(nc, maxw=1):
    """Split multi-wait sync_info into standalone EventSemaphore instructions."""
    for f in nc.m.functions:
        for bb in f.blocks:
            insts = list(bb.instructions)
            out = []
            changed = False
            for inst in insts:
                si = inst.sync_info
                if si is not None and si.on_wait and len(si.on_wait) > maxw:
                    waits = list(si.on_wait)
                    imm = [w for w in waits if w.uses_immediate]
                    reg = [w for w in waits if not w.uses_immediate]
                    keep = (reg + imm)[:maxw] if len(reg) <= maxw else reg
                    extra = [w for w in waits if w not in keep]
                    if len(keep) > maxw:
                        raise RuntimeError(f"{inst.name}: {len(keep)} register waits")
                    for w in extra:
                        ev = mybir.InstEventSemaphore(
                            name=nc.get_next_instruction_name(), ins=[], outs=[])
                        ev.engine = inst.engine
                        ev.sync_info = mybir.SyncInfo(on_wait=[w], on_update=[])
                        out.append(ev)
                    inst.sync_info = mybir.SyncInfo(
                        on_wait=keep, on_update=list(si.on_update or []))
                    changed = True
                out.append(inst)
            if changed:
                bb.instructions = out
    return nc


def build_nc():
    nc = bass.Bass()

    # ---- I/O ----
    # xTp/w1esp: pre-packed partition-major [(p) (k b)] so DMA descriptors are
    # per-partition contiguous (12KB / 6KB) instead of 128B/512B strided
    xTp = nc.dram_tensor("xTp", [P, KT_D * B], BF16, kind="ExternalInput")
    xmb = nc.dram_tensor("xmb", [B, DS], BF16, kind="ExternalInput")
    w1esp = nc.dram_tensor("w1esp", [P, KT_D * HS], BF16, kind="ExternalInput")
    b1es = nc.dram_tensor("b1es", [1, HS], BF16, kind="ExternalInput")
    w2es = nc.dram_tensor("w2es", [HS, N], BF16, kind="ExternalInput")
    b2e = nc.dram_tensor("b2e", [1, N], BF16, kind="ExternalInput")
    w2Ts = nc.dram_tensor("w2Ts", [DS, H], FP8, kind="ExternalInput")
    w2s = nc.dram_tensor("w2s", [H, DS], BF16, kind="ExternalInput")
    w1 = nc.dram_tensor("w1", [N, H], BF16, kind="ExternalInput")
    w1Td = nc.dram_tensor("w1Td", [H, N], BF16, kind="ExternalInput")
    b1d = nc.dram_tensor("b1d", [1, H], BF16, kind="ExternalInput")
    sigw = nc.dram_tensor("sigw", [1, 130], F32, kind="ExternalInput")
    sel8 = nc.dram_tensor("sel8", [B, BL], BF16, kind="ExternalInput")
    epsin = nc.dram_tensor("epsin", [BL, N], F32, kind="ExternalInput")
    out = nc.dram_tensor("out", [BL, 1], F32, kind="ExternalOutput")

    # ---- internal DRAM ----
    # chunk q: upper-C col-block q slabs (+ z* partial rows in chunk 0).
    # all chunks bf16 (fp8 collectives produce NaN on this stack)
    CUPDT = [BF16, BF16, BF16, BF16]
    cup = [nc.dram_tensor(f"cup{q}", [(ZROWS if q == 0 else 0) + CUP_NROW[q] * P, 512],
                          CUPDT[q]) for q in range(4)]
    cup_sh = [nc.dram_tensor(f"cup_sh{q}", [(ZROWS if q == 0 else 0) + CUP_NROW[q] * P, 512],
                             CUPDT[q], addr_space="Shared") for q in range(4)]
    zs_b = nc.dram_tensor("zs_b", [BL, N], F32)
    zs_sh = nc.dram_tensor("zs_sh", [B, N], F32, addr_space="Shared")
    wd_b = nc.dram_tensor("wd_b", [B, H], BF16)
    wds_b = nc.dram_tensor("wds_b", [BL, H], BF16)
    dsq_b = nc.dram_tensor("dsq_b", [B, 1], F32)
    dsqs_b = nc.dram_tensor("dsqs_b", [BL, 1], F32)

    with tile.TileContext(nc) as tc, ExitStack() as ctx:
        consts = ctx.enter_context(tc.tile_pool(name="consts", bufs=1))
        work = ctx.enter_context(tc.tile_pool(name="work", bufs=2))
        stream = ctx.enter_context(tc.tile_pool(name="stream", bufs=3))
        psum = ctx.enter_context(tc.tile_pool(name="psum", bufs=2, space="PSUM"))
        psum_acc = ctx.enter_context(tc.tile_pool(name="psacc", bufs=1, space="PSUM"))
        lin = ctx.enter_context(tc.tile_pool(name="lin", bufs=1))
        res = ctx.enter_context(tc.tile_pool(name="res", bufs=1))
        encs = ctx.enter_context(tc.tile_pool(name="encs", bufs=2))
        w2sp = ctx.enter_context(tc.tile_pool(name="w2sp", bufs=6))

        # ---- resident weights first: w2Ts fp8 x W2SC (phase C + Wd) ----
        # emitted ahead of the consts so the SP DMA queue starts on them
        w2Ts_sb = res.tile([P, KT_DS, H], FP8, tag="w2Ts")
        w2Ts_r = w2Ts[:].rearrange("(k p) h -> p k h", p=P)
        for kt in range(KT_DS):
            nc.sync.dma_start(w2Ts_sb[:, kt, :], w2Ts_r[:, kt, :])

        # ---- constants / small loads ----
        identb = consts.tile([P, P], BF16)
        make_identity(nc, identb)
        ones1 = consts.tile([1, B], F32)
        nc.vector.memset(ones1, 1.0)
        onesb = consts.tile([1, B], BF16)
        nc.vector.memset(onesb, 1.0)
        sigw_sb = consts.tile([1, 130], F32)
        nc.sync.dma_start(sigw_sb, sigw[:])
        sigw_rep = consts.tile([BL, 130], F32)
        sigw_ps = psum.tile([BL, 130], F32, tag="small_ps")
        nc.tensor.matmul(sigw_ps, ones1[:, 0:BL], sigw_sb, start=True, stop=True)
        nc.vector.tensor_copy(sigw_rep, sigw_ps)
        sel8_sb = consts.tile([B, BL], BF16)
        nc.sync.dma_start(sel8_sb, sel8[:])
        eps_sb = consts.tile([BL, N], F32)
        nc.sync.dma_start(eps_sb, epsin[:])
        b1es_sb = consts.tile([1, HS], BF16)
        nc.sync.dma_start(b1es_sb, b1es[:])
        b2e_sb = consts.tile([1, N], BF16)
        nc.sync.dma_start(b2e_sb, b2e[:])
        # decoder bias as per-partition columns [P, KT_H] (+ negated copy)
        b1dcol = consts.tile([P, KT_H], BF16)
        nc.sync.dma_start(b1dcol, b1d[:].rearrange("o (k p) -> p (o k)", p=P))
        nb1col = consts.tile([P, KT_H], F32)
        nc.vector.tensor_scalar(nb1col, b1dcol, -1.0, None, Alu.mult)
        w1_sb = consts.tile([N, H], BF16)
        nc.sync.dma_start(w1_sb, w1[:])
        w1T_sb = consts.tile([P, KT_H, N], BF16)
        nc.sync.dma_start(w1T_sb, w1Td[:].rearrange("(k p) n -> p k n", p=P))

        # ================= phase E: encoder (z* partial -> cup rows 0:4) ====
        a1_ps = psum_acc.tile([B, HS], F32, tag="acc")
        KSUP = 8  # k-tiles per packed super-chunk
        for kc in range(KT_D // KSUP):
            xp_t = encs.tile([P, KSUP, B], BF16, tag="xp_t")
            nc.sync.dma_start(
                xp_t, xTp[:, kc * KSUP * B:(kc + 1) * KSUP * B]
                .rearrange("p (k b) -> p k b", b=B))
            w1t = encs.tile([P, KSUP, HS], BF16, tag="w1es_t")
            nc.sync.dma_start(
                w1t, w1esp[:, kc * KSUP * HS:(kc + 1) * KSUP * HS]
                .rearrange("p (k h) -> p k h", h=HS))
            for kj in range(KSUP):
                kt = kc * KSUP + kj
                nc.tensor.matmul(a1_ps, xp_t[:, kj, :], w1t[:, kj, :],
                                 start=(kt == 0), stop=False)
        nc.tensor.matmul(a1_ps, onesb[:, 0:B], b1es_sb, start=False, stop=True)
        h1_sb = work.tile([B, HS], BF16, tag="h1")
        nc.vector.tensor_scalar(h1_sb, a1_ps, 0.0, None, Alu.max)
        h1T_sb = work.tile([P, 2, B], BF16, tag="h1T")
        for i in range(2):
            tp = psum.tile([P, B], BF16, tag="t_ps")
            pe_T(nc, tp, h1_sb[:, i * P:(i + 1) * P], identb)
            nc.vector.tensor_copy(h1T_sb[:, i, :], tp)
        w2es_sb = work.tile([P, 2, N], BF16, tag="w2es")
        nc.sync.dma_start(w2es_sb, w2es[:].rearrange("(k p) n -> p k n", p=P))
        zp_ps = psum.tile([B, N], F32, tag="small_ps")
        for i in range(2):
            nc.tensor.matmul(zp_ps, h1T_sb[:, i, :], w2es_sb[:, i, :],
                             start=(i == 0), stop=(i == 1))
        zp_sb = work.tile([B, N], BF16, tag="zstar_part")
        nc.vector.tensor_copy(zp_sb, zp_ps)
        nc.sync.dma_start(cup[0][0:ZROWS, :], zp_sb)

        # ========= phase C: upper C slabs + chunked AllReduce ==============
        # col-block q: cols [512q, 512q+512), block-rows j in 0..4q+3
        csb = res.tile([P, KT_H, H], BF16, tag="csb")
        for q in range(4):
            zr = ZROWS if q == 0 else 0
            for jg in range(q + 1):          # groups of 4 slabs
                cs = work.tile([P, 4, 512], CUPDT[q],
                               tag="c_out" if q == 0 else "c_out8")
                for jj in range(4):
                    j = jg * 4 + jj
                    cps = psum.tile([P, 512], F32, tag="big_ps")
                    for kd in range(KT_DS // 2):
                        nc.tensor.matmul(
                            cps,
                            w2Ts_sb[:, 2 * kd:2 * kd + 2, j * P:(j + 1) * P],
                            w2Ts_sb[:, 2 * kd:2 * kd + 2, 512 * q:512 * (q + 1)],
                            start=(kd == 0), stop=(kd == KT_DS // 2 - 1),
                            perf_mode=DR)
                    nc.scalar.copy(cs[:, jj, :], cps)
                r0 = zr + jg * 4 * P
                nc.sync.dma_start(
                    cup[q][r0:r0 + 4 * P, :].rearrange("(s p) c -> p s c", p=P),
                    cs)
            # AllReduce this chunk (chunk 0 also carries the z* partial)
            nc.gpsimd.collective_compute(
                "AllReduce", Alu.add, replica_groups=RG,
                ins=[cup[q][:]], outs=[cup_sh[q][:]])
            # readback on the gpsimd DMA queue right behind the AR so it
            # lands as soon as the chunk is reduced (SP queue is busy with
            # input streams / cup writes)
            if q == 0:
                zf_sb = work.tile([B, N], BF16, tag="z_full")
                nc.gpsimd.dma_start(zf_sb, cup_sh[0][0:ZROWS, :])
            zr_ = ZROWS if q == 0 else 0
            shr = cup_sh[q][zr_:, :].rearrange("(s p) c -> p s c", p=P)
            nc.gpsimd.dma_start(
                csb[:, 0:4 * q + 4, 512 * q:512 * (q + 1)], shr)

        # ---- z* post: local slice, sig1, masks, A1T (overlaps C build) ----
        zlT_ps = psum.tile([N, BL], F32, tag="small_ps")
        nc.tensor.matmul(zlT_ps, zf_sb, sel8_sb, start=True, stop=False)
        nc.tensor.matmul(zlT_ps, b2e_sb, onesb[:, 0:BL], start=False, stop=True)
        zlT_sb = work.tile([N, BL], BF16, tag="zlT")   # (z*loc + b2)^T
        nc.vector.tensor_copy(zlT_sb, zlT_ps)
        zloc_ps = psum.tile([BL, N], F32, tag="small_ps")
        nc.tensor.matmul(zloc_ps, sel8_sb, zf_sb, start=True, stop=False)
        nc.tensor.matmul(zloc_ps, onesb[:, 0:BL], b2e_sb, start=False, stop=True)
        zloc_sb = lin.tile([BL, N], F32, tag="z_loc")   # z* local + b2
        nc.vector.tensor_copy(zloc_sb, zloc_ps)

        def emit_sig(z_loc, name):
            lg = lin.tile([BL, 2, 32], F32, tag="sig_lg")
            nc.vector.tensor_tensor(
                lg, z_loc.unsqueeze(1).broadcast_to([BL, 2, 32]),
                sigw_rep[:, 0:64].rearrange("p (c n) -> p c n", c=2), Alu.mult)
            red = lin.tile([BL, 2], F32, tag=f"sig_red_{name}")
            nc.vector.tensor_reduce(red, lg, mybir.AxisListType.X, Alu.add)
            nc.vector.tensor_tensor(red, red, sigw_rep[:, 64:66], Alu.add)
            s = lin.tile([BL, 2], F32, tag=f"sig_s_{name}")
            nc.scalar.activation(s, red, Act.Exp)
            return s

        s1 = emit_sig(zloc_sb, "s1")
        invsp2 = lin.tile([BL, 1], F32, tag="invsp2")
        sp2t = lin.tile([BL, 1], F32, tag="sp2t")
        nc.vector.tensor_tensor(sp2t, s1[:, 0:1], s1[:, 0:1], Alu.mult)
        # G arrives scaled by W2SC^2 (fp8 weights); fold 1/W2SC^2 in here
        nc.vector.tensor_scalar(sp2t, sp2t, W2SC * W2SC, None, Alu.mult)
        nc.vector.reciprocal(invsp2, sp2t)

        # a1T (local) -> mask m1T [P, KT_H, BL] -> A1T  (mask: a1 > -b1)
        m1T_sb = work.tile([P, KT_H, BL], BF16, tag="m1T")
        for mt in range(KT_H):
            aps = psum.tile([P, BL], F32, tag="small_ps")
            nc.tensor.matmul(aps, w1_sb[:, mt * P:(mt + 1) * P],
                             zlT_sb, start=True, stop=True)
            nc.vector.tensor_tensor(
                m1T_sb[:, mt, :], aps,
                nb1col[:, mt:mt + 1].broadcast_to([P, BL]), Alu.is_gt)
        AT_sb = res.tile([P, KT_H, BL, N], BF16, tag="AT")
        nc.vector.tensor_tensor(
            AT_sb,
            w1T_sb.unsqueeze(2).broadcast_to([P, KT_H, BL, N]),
            m1T_sb.unsqueeze(3).broadcast_to([P, KT_H, BL, N]), Alu.mult)

        # ---- chunk-pipelined P1T = C @ A1T: each col-chunk's mirrors and PT
        # contributions run as soon as its AllReduce lands (PE overlaps ARs) --
        PT1 = res.tile([P, KT_H, BL * N], BF16, tag="PT")
        for q in range(4):
            # mirrors sourced from chunk q: targets (i in stripe q, j < 4q)
            for i in range(4 * q, 4 * q + 4):
                for j in range(4 * q):
                    tp = psum.tile([P, P], BF16, tag="t_ps")
                    pe_T(nc, tp, csb[:, j, i * P:(i + 1) * P], identb)
                    nc.scalar.copy(csb[:, i, j * P:(j + 1) * P], tp)
            # (a) fold the new kt-stripe q into PT[mt] for mt < 4q
            for mt in range(4 * q):
                pps = psum.tile([P, BL * N], F32, tag="big_ps")
                for kt in range(4 * q, 4 * q + 4):
                    nc.tensor.matmul(pps, csb[:, kt, mt * P:(mt + 1) * P],
                                     AT_sb[:, kt, :, :],
                                     start=(kt == 4 * q), stop=(kt == 4 * q + 3))
                nc.vector.tensor_tensor(PT1[:, mt, :], PT1[:, mt, :], pps,
                                        Alu.add)
            # (b) initialize PT[mt] for mt in stripe q (kt 0..4q+3 available)
            for mt in range(4 * q, 4 * q + 4):
                pps = psum.tile([P, BL * N], F32, tag="big_ps")
                for kt in range(4 * q + 4):
                    nc.tensor.matmul(pps, csb[:, kt, mt * P:(mt + 1) * P],
                                     AT_sb[:, kt, :, :],
                                     start=(kt == 0), stop=(kt == 4 * q + 3))
                nc.scalar.copy(PT1[:, mt, :], pps)



        # ---- P*T = C @ A*T ; G = P*T^T A*T  (C resident in SBUF) ----
        def emit_PG(AT, tag):
            PT_sb = res.tile([P, KT_H, BL * N], BF16, tag="PT")
            for mt in range(KT_H):
                pps = psum.tile([P, BL * N], F32, tag="big_ps")
                for kt in range(KT_H):
                    nc.tensor.matmul(
                        pps, csb[:, kt, mt * P:(mt + 1) * P], AT[:, kt, :, :],
                        start=(kt == 0), stop=(kt == KT_H - 1))
                nc.scalar.copy(PT_sb[:, mt, :], pps)
            g_sb = work.tile([P, 2, P], F32, tag="g_sb")
            for grp in range(2):
                g_ps = psum.tile([P, P], F32, tag="big_ps")
                for kt in range(KT_H):
                    nc.tensor.matmul(
                        g_ps,
                        PT_sb[:, kt, grp * P:(grp + 1) * P],
                        AT[:, kt, 4 * grp:4 * grp + 4, :],
                        start=(kt == 0), stop=(kt == KT_H - 1))
                nc.vector.tensor_copy(g_sb[:, grp, :], g_ps)
            return g_sb

        # ---- Prec = G*invsp2 + sig_term + I ----
        # preset Tm with replicated sig_term while PG runs
        st_ps = psum.tile([N, N], F32, tag="small_ps")
        nc.tensor.matmul(st_ps, sigw_sb[:, 66:98], sigw_sb[:, 66:98],
                         start=True, stop=False)
        nc.tensor.matmul(st_ps, sigw_sb[:, 98:130], sigw_sb[:, 98:130],
                         start=False, stop=True)
        st_sb = work.tile([N, N], F32, tag="st_sb")
        nc.vector.tensor_copy(st_sb, st_ps)
        Tm = lin.tile([BL, N * N], F32, tag="Tmat")
        for s in range(BL):
            nc.sync.dma_start(Tm[s:s + 1, :], st_sb)
        # add diag I up front
        diag1 = sub_ap(Tm, 0, [[N + 1, N]])
        nc.vector.tensor_scalar(diag1, diag1, 1.0, None, Alu.add)

        g_sb = work.tile([P, 2, P], F32, tag="g_sb")
        for grp in range(2):
            g_ps = psum.tile([P, P], F32, tag="big_ps")
            for kt in range(KT_H):
                nc.tensor.matmul(
                    g_ps, PT1[:, kt, grp * P:(grp + 1) * P],
                    AT_sb[:, kt, 4 * grp:4 * grp + 4, :],
                    start=(kt == 0), stop=(kt == KT_H - 1))
            nc.vector.tensor_copy(g_sb[:, grp, :], g_ps)
        SCR = lin.tile([BL, N * N], F32, tag="scr")
        for s in range(BL):
            grp, sl = s // 4, s % 4
            nc.sync.dma_start(
                SCR[s:s + 1, :],
                g_sb[sl * N:(sl + 1) * N, grp, sl * N:(sl + 1) * N])
        nc.vector.scalar_tensor_tensor(Tm, SCR, invsp2, Tm, Alu.mult, Alu.add)

        # ---- LDLT, dz (backward solve; Lt^-1/tr deferred off critical path) ----
        invD = lin.tile([BL, N], F32, tag="invD")
        emit_ldlt(nc, Tm, SCR, invD)
        LT = lin.tile([BL, N * N], F32, tag="LTmat")
        nc.vector.tensor_tensor(
            LT.rearrange("p (a b) -> p a b", b=N),
            Tm.rearrange("p (a b) -> p a b", b=N),
            invD.unsqueeze(1).broadcast_to([BL, N, N]), Alu.mult)
        srD = lin.tile([BL, N], F32, tag="srD")
        nc.scalar.activation(srD, invD, Act.Sqrt)        # 1/sqrt(D)
        epss = lin.tile([BL, N], F32, tag="epss")
        nc.vector.tensor_tensor(epss, eps_sb, srD, Alu.mult)
        emit_bwd_solve(nc, LT, epss, SCR)                # epss <- Lt^-T epss = dz
        zs_loc = lin.tile([BL, N], F32, tag="zs_loc")
        nc.vector.tensor_tensor(zs_loc, zloc_sb, epss, Alu.add)
        nc.sync.dma_start(zs_b[:], zs_loc)
        nc.gpsimd.collective_compute("AllGather", Alu.bypass, replica_groups=RG,
                                     ins=[zs_b[:]], outs=[zs_sh[:]])

        # ---- tr(Prec^-1) via Lt^-1 on GpSimd (parallel with stage 2) ----
        X1 = lin.tile([BL, N * N], F32, tag="X1")
        nc.gpsimd.memset(X1, 0.0)
        nc.gpsimd.memset(sub_ap(X1, 0, [[N + 1, N]]), 1.0)
        gSCR = lin.tile([BL, N * N], F32, tag="gSCR")
        emit_ltinv(nc.gpsimd, LT, X1, gSCR)
        trv = lin.tile([BL, 1], F32, tag="trv")
        nc.gpsimd.tensor_tensor(
            gSCR.rearrange("p (a b) -> p a b", b=N),
            X1.rearrange("p (a b) -> p a b", b=N),
            invD.unsqueeze(2).broadcast_to([BL, N, N]), Alu.mult)
        nc.gpsimd.tensor_tensor(gSCR, gSCR, X1, Alu.mult)
        # final free-axis reduce of gSCR into trv happens on vector at the tail

        # ---- z*-only reductions (vector, overlap AllGather) ----
        logs = lin.tile([BL, N], F32, tag="logs")
        ldv = lin.tile([BL, 1], F32, tag="ldv")
        nc.scalar.activation(logs, invD, Act.Ln)
        nc.vector.tensor_reduce(ldv, logs, mybir.AxisListType.X, Alu.add)  # -sum log D
        nc.vector.tensor_scalar(ldv, ldv, -0.5, None, Alu.mult)
        zsq = lin.tile([BL, N], F32, tag="zsq")
        latv = lin.tile([BL, 1], F32, tag="latv")
        nc.vector.tensor_tensor(zsq, zloc_sb, zloc_sb, Alu.mult)
        nc.vector.tensor_reduce(latv, zsq, mybir.AxisListType.X, Alu.add)
        # s2-dependent scalars (zs_loc known before AG returns)
        s2 = emit_sig(zs_loc, "s2")
        sq2 = lin.tile([BL, 2], F32, tag="sq2")
        nc.vector.tensor_tensor(sq2, s2, s2, Alu.mult)
        nc.vector.tensor_scalar(sq2, sq2, 2.0, None, Alu.mult)
        inv2 = lin.tile([BL, 2], F32, tag="inv2")
        nc.vector.reciprocal(inv2, sq2)     # [1/(2sp2^2), 1/(2sv2^2)]
        logs2 = lin.tile([BL, 2], F32, tag="logs2")
        logw = lin.tile([BL, 2], F32, tag="logw")
        nc.vector.memset(logw[:, 0:1], float(N))
        nc.vector.memset(logw[:, 1:2], float(D - N))
        nc.scalar.activation(logs2, s2, Act.Ln)
        logterm = lin.tile([BL, 1], F32, tag="logterm")
        junk2 = lin.tile([BL, 2], F32, tag="junk2")
        nc.vector.tensor_tensor(junk2, logs2, logw, Alu.mult)
        nc.vector.tensor_reduce(logterm, junk2, mybir.AxisListType.X, Alu.add)
        isub = lin.tile([BL, 1], F32, tag="isub")
        nc.vector.tensor_tensor(isub, inv2[:, 0:1], inv2[:, 1:2], Alu.subtract)

        # ---- stage 2 prep: h2T (all), m2T (local), A2T ----
        zsf_sb = work.tile([B, N], F32, tag="z_full2")
        nc.sync.dma_start(zsf_sb, zs_sh[:])
        zsf_bf = work.tile([B, N], BF16, tag="z_full2b")
        nc.vector.tensor_copy(zsf_bf, zsf_sb)
        zs_bf = lin.tile([BL, N], BF16, tag="zs_locb")
        nc.vector.tensor_copy(zs_bf, zs_loc)
        zsT_ps = psum.tile([N, B], BF16, tag="t_ps")
        pe_T(nc, zsT_ps, zsf_bf, identb)
        zsT_sb = work.tile([N, B], BF16, tag="zT2")
        nc.vector.tensor_copy(zsT_sb, zsT_ps)
        zslT_ps = psum.tile([N, BL], BF16, tag="t_ps")
        pe_T(nc, zslT_ps, zs_bf, identb)
        zslT_sb = work.tile([N, BL], BF16, tag="zlT2")
        nc.vector.tensor_copy(zslT_sb, zslT_ps)

        h2T_sb = res.tile([P, KT_H, B], BF16, tag="h2T")
        for mt in range(KT_H):
            aps = psum.tile([P, B], F32, tag="small_ps")
            nc.tensor.matmul(aps, w1_sb[:, mt * P:(mt + 1) * P],
                             zsT_sb, start=True, stop=True)
            nc.scalar.activation(h2T_sb[:, mt, :], aps, Act.Relu,
                                 bias=b1dcol[:, mt:mt + 1])

        m2T_sb = work.tile([P, KT_H, BL], BF16, tag="m2T")
        for mt in range(KT_H):
            aps = psum.tile([P, BL], F32, tag="small_ps")
            nc.tensor.matmul(aps, w1_sb[:, mt * P:(mt + 1) * P],
                             zslT_sb, start=True, stop=True)
            nc.vector.tensor_tensor(
                m2T_sb[:, mt, :], aps,
                nb1col[:, mt:mt + 1].broadcast_to([P, BL]), Alu.is_gt)
        AT2_sb = res.tile([P, KT_H, BL, N], BF16, tag="AT")   # reuse slot
        nc.vector.tensor_tensor(
            AT2_sb,
            w1T_sb.unsqueeze(2).broadcast_to([P, KT_H, BL, N]),
            m2T_sb.unsqueeze(3).broadcast_to([P, KT_H, BL, N]), Alu.mult)

        # ---- G2 on PE first: fills PE while the w2s stream for x_star runs --
        g2_sb = emit_PG(AT2_sb, "2")
        Tm2 = lin.tile([BL, N * N], F32, tag="Tmat")   # reuse slot
        for s in range(BL):
            grp, sl = s // 4, s % 4
            nc.sync.dma_start(
                Tm2[s:s + 1, :],
                g2_sb[sl * N:(sl + 1) * N, grp, sl * N:(sl + 1) * N])
        # Jacobi weights for the Richardson solve (no factorization needed)
        dg2 = lin.tile([BL, N], F32, tag="dg2")
        nc.vector.tensor_copy(dg2, sub_ap(Tm2, 0, [[N + 1, N]]))
        widg = lin.tile([BL, N], F32, tag="widg")
        nc.vector.reciprocal(widg, dg2)
        nc.vector.tensor_scalar(widg, widg, 0.9, None, Alu.mult)

        # ---- x_star slice, delta, d_sq, Wd (w2Ts from SBUF) ----
        d_sb = res.tile([B, DS], BF16, tag="d_sb")
        w2s_r = w2s[:].rearrange("(k p) ds -> p k ds", p=P)
        for nb in range(3):
            xmb_t = stream.tile([B, 512], BF16, tag="xmb_t")
            nc.sync.dma_start(xmb_t, xmb[:, nb * 512:(nb + 1) * 512])
            xs_ps = psum.tile([B, 512], F32, tag="big_ps")
            for kt in range(KT_H):
                wt = w2sp.tile([P, 512], BF16, tag="w2s_t")
                nc.sync.dma_start(wt, w2s_r[:, kt, nb * 512:(nb + 1) * 512])
                nc.tensor.matmul(xs_ps, h2T_sb[:, kt, :], wt,
                                 start=(kt == 0), stop=(kt == KT_H - 1))
            nc.vector.tensor_tensor(d_sb[:, nb * 512:(nb + 1) * 512], xmb_t,
                                    xs_ps, Alu.subtract)
        dT_bf = res.tile([P, KT_DS, B], BF16, tag="dTb")
        for kt in range(KT_DS):
            tp = psum.tile([P, B], BF16, tag="t_ps")
            pe_T(nc, tp, d_sb[:, kt * P:(kt + 1) * P], identb)
            nc.vector.tensor_copy(dT_bf[:, kt, :], tp)
        dT8 = res.tile([P, KT_DS, B], FP8, tag="dT")
        nc.scalar.copy(dT8, dT_bf)
        dsq_sb = work.tile([B, 1], F32, tag="dsq")
        # d_sq = rowsum(delta^2); squares written in place (d_sb dead after d8)
        nc.scalar.activation(d_sb, d_sb, Act.Square, accum_out=dsq_sb)
        nc.sync.dma_start(dsq_b[:], dsq_sb)
        nc.gpsimd.collective_compute("ReduceScatter", Alu.add, replica_groups=RG,
                                     ins=[dsq_b[:]], outs=[dsqs_b[:]])
        wd_sb = res.tile([B, H], BF16, tag="wd")
        for mb in range(4):
            wd_ps = psum.tile([B, 512], F32, tag="big_ps")
            for kd in range(KT_DS // 2):
                nc.tensor.matmul(wd_ps, dT8[:, 2 * kd:2 * kd + 2, :],
                                 w2Ts_sb[:, 2 * kd:2 * kd + 2,
                                         mb * 512:(mb + 1) * 512],
                                 start=(kd == 0), stop=(kd == KT_DS // 2 - 1),
                                 perf_mode=DR)
            nc.vector.tensor_scalar(wd_sb[:, mb * 512:(mb + 1) * 512], wd_ps,
                                    1.0 / W2SC, None, Alu.mult)
        nc.sync.dma_start(wd_b[:], wd_sb)
        nc.gpsimd.collective_compute("ReduceScatter", Alu.add, replica_groups=RG,
                                     ins=[wd_b[:]], outs=[wds_b[:]])

        # ---- local Wd/dsq arrive directly via ReduceScatter ----
        dsql = lin.tile([BL, 1], F32, tag="dsql")
        nc.sync.dma_start(dsql, dsqs_b[:])
        wdl_bf = res.tile([BL, H], BF16, tag="wd_locb")
        nc.gpsimd.dma_start(wdl_bf, wds_b[:])
        wdlT_sb = work.tile([P, KT_H, BL], BF16, tag="wdlT")
        for kt in range(KT_H):
            tp2 = psum.tile([P, BL], BF16, tag="t_ps")
            pe_T(nc, tp2, wdl_bf[:, kt * P:(kt + 1) * P], identb)
            nc.vector.tensor_copy(wdlT_sb[:, kt, :], tp2)
        mwdT_sb = work.tile([P, KT_H, BL], BF16, tag="mwdT")
        nc.vector.tensor_tensor(mwdT_sb, wdlT_sb, m2T_sb, Alu.mult)
        # y[bl, n] = sum_h mwdT[h, bl] * w1T[h, n]  (t, already transposed)
        y_ps = psum.tile([BL, N], F32, tag="small_ps")
        for kt in range(KT_H):
            nc.tensor.matmul(y_ps, mwdT_sb[:, kt, :], w1T_sb[:, kt, :],
                             start=(kt == 0), stop=(kt == KT_H - 1))
        y = lin.tile([BL, N], F32, tag="y")
        nc.vector.tensor_copy(y, y_ps)
        # ---- solve G2 x = y by Jacobi-damped Richardson ----
        xs = lin.tile([BL, N], F32, tag="xs")
        gx = lin.tile([BL, N], F32, tag="gx")
        tmpv = lin.tile([BL, N], F32, tag="tmpv")
        nc.vector.tensor_tensor(xs, y, widg, Alu.mult)
        for _ in range(4):
            nc.vector.tensor_tensor(
                SCR.rearrange("p (a b) -> p a b", b=N),
                Tm2.rearrange("p (a b) -> p a b", b=N),
                xs.unsqueeze(1).broadcast_to([BL, N, N]), Alu.mult)
            nc.vector.tensor_reduce(
                gx, SCR.rearrange("p (a b) -> p a b", b=N),
                mybir.AxisListType.X, Alu.add)
            nc.vector.tensor_tensor(tmpv, y, gx, Alu.subtract)
            nc.vector.tensor_tensor(tmpv, tmpv, widg, Alu.mult)
            nc.vector.tensor_tensor(xs, xs, tmpv, Alu.add)
        yx = lin.tile([BL, N], F32, tag="yx")
        dproj = lin.tile([BL, 1], F32, tag="dproj")
        nc.vector.tensor_tensor(yx, y, xs, Alu.mult)
        nc.vector.tensor_reduce(dproj, yx, mybir.AxisListType.X, Alu.add)
        # Tm2 = W2SC^2 * G2, so x and hence dproj are 1/W2SC^2 scaled
        nc.vector.tensor_scalar(dproj, dproj, W2SC * W2SC, None, Alu.mult)
        nc.vector.tensor_reduce(trv, gSCR, mybir.AxisListType.X, Alu.add)

        # ---- recon / output (scalars precomputed during stage 2) ----
        recon = lin.tile([BL, 1], F32, tag="recon")
        nc.vector.tensor_tensor(recon, dproj, isub, Alu.mult)
        p2t = lin.tile([BL, 1], F32, tag="p2t")
        nc.vector.tensor_tensor(p2t, dsql, inv2[:, 1:2], Alu.mult)
        nc.vector.tensor_tensor(recon, recon, p2t, Alu.add)
        nc.vector.tensor_tensor(recon, recon, logterm, Alu.add)
        ov = lin.tile([BL, 1], F32, tag="ov")
        nc.vector.tensor_tensor(ov, latv, trv, Alu.add)
        nc.vector.tensor_scalar(ov, ov, 0.5, None, Alu.mult)
        nc.vector.tensor_tensor(ov, ov, recon, Alu.add)
        nc.vector.tensor_tensor(ov, ov, ldv, Alu.add)
        nc.vector.tensor_scalar(ov, ov, 1.0 / D, None, Alu.mult)
        nc.sync.dma_start(out[:], ov)

    legalize_waits(nc)
    return nc


def shard_inputs(inputs):
    """Host-side prep: returns in_maps list for the 8 cores."""
    bf = ml_dtypes.bfloat16
    x = np.ascontiguousarray(np.asarray(inputs["x"], np.float32))
    eps = np.ascontiguousarray(np.asarray(inputs["eps"], np.float32))
    eW1 = np.ascontiguousarray(np.asarray(inputs["enc_W1"], np.float32))
    eb1 = np.asarray(inputs["enc_b1"], np.float32)
    eW2 = np.ascontiguousarray(np.asarray(inputs["enc_W2"], np.float32))
    eb2 = np.asarray(inputs["enc_b2"], np.float32)
    dW1 = np.ascontiguousarray(np.asarray(inputs["dec_W1"], np.float32))
    db1 = np.asarray(inputs["dec_b1"], np.float32)
    dW2 = np.ascontiguousarray(np.asarray(inputs["dec_W2"], np.float32))
    db2 = np.asarray(inputs["dec_b2"], np.float32)
    sW = np.asarray(inputs["sig_W"], np.float32)
    sb = np.asarray(inputs["sig_b"], np.float32)

    xT = np.ascontiguousarray(x.T).astype(bf)
    xTp = np.ascontiguousarray(
        xT.reshape(KT_D, P, B).transpose(1, 0, 2).reshape(P, KT_D * B))
    dW2T = np.ascontiguousarray(dW2.T)
    dW1T = np.ascontiguousarray(dW1.T).astype(bf)
    dW1b = dW1.astype(bf)
    sigv = np.zeros((1, 130), np.float32)
    sigv[0, 0:32] = sW[:, 0]
    sigv[0, 32:64] = sW[:, 1]
    sigv[0, 64:66] = sb
    sigv[0, 66:98] = sW[:, 0] * np.sqrt(N / 2.0)
    sigv[0, 98:130] = sW[:, 1] * np.sqrt((D - N) / 2.0)

    maps = []
    for k in range(NCORES):
        sel = np.zeros((B, BL), np.float32)
        for i in range(BL):
            sel[k * BL + i, i] = 1.0
        w1s = np.ascontiguousarray(eW1[:, k * HS:(k + 1) * HS]).astype(bf)
        maps.append({
            "xTp": xTp,
            "xmb": np.ascontiguousarray(
                x[:, k * DS:(k + 1) * DS]
                - db2[None, k * DS:(k + 1) * DS]).astype(bf),
            "w1esp": np.ascontiguousarray(
                w1s.reshape(KT_D, P, HS).transpose(1, 0, 2)
                .reshape(P, KT_D * HS)),
            "b1es": np.ascontiguousarray(eb1[None, k * HS:(k + 1) * HS]).astype(bf),
            "w2es": np.ascontiguousarray(eW2[k * HS:(k + 1) * HS, :]).astype(bf),
            "b2e": np.ascontiguousarray(eb2[None, :]).astype(bf),
            "w2Ts": (np.ascontiguousarray(dW2T[k * DS:(k + 1) * DS, :]) * W2SC
                     ).astype(ml_dtypes.float8_e4m3fn),
            "w2s": np.ascontiguousarray(dW2[:, k * DS:(k + 1) * DS]).astype(bf),
            "w1": dW1b,
            "w1Td": dW1T,
            "b1d": np.ascontiguousarray(db1[None, :]).astype(bf),
            "sigw": sigv,
            "sel8": sel.astype(bf),
            "epsin": np.ascontiguousarray(eps[k * BL:(k + 1) * BL, :]),
        })
    return maps


_NC_CACHE = None


def kernel(**inputs) -> np.ndarray:
    global _NC_CACHE
    from concourse.bass_utils import run_bass_kernel_spmd
    if _NC_CACHE is None:
        _NC_CACHE = build_nc()
    nc = _NC_CACHE
    maps = shard_inputs(inputs)
    res = run_bass_kernel_spmd(nc, maps, list(range(NCORES)))
    outs = [res.results[k]["out"].reshape(BL) for k in range(NCORES)]
    return np.concatenate(outs).astype(np.float32)


# revision 89
# speedup vs baseline: 1.3962x; 1.0615x over previous
"""Trainium2 Bass kernel for nn_EnergyAE (B=64, D=12288, N=32, H=2048) on 8 cores.

Hybrid sharding, bf16 matmuls (fp32 vector math):
  phase E  (model-parallel over H): encoder -> z* partial (bf16, folded into
           C-chunk0 AllReduce below)
  phase C  (contraction-parallel over D): upper-triangular 512-col blocks of
           Cpart = w2Ts @ w2Ts^T, AllReduced in 4 bf16 chunks pipelined with
           the build; mirrored to full C in SBUF locally after readback.
           C stays SBUF-resident for both PG stages.
  phase S1 (data-parallel, 8 samples/core): A1 = W1*m1, P1T = C@A1T,
           G = P1T^T A1T (batched 4 samples/matmul), Prec, LDLT, Lt^-1, dz,
           tr, logdet, z_s = z* + dz -> AllGather z_s (8KB)
  phase S2 (model-parallel over D): h2, x_star slice, delta, d_sq,
           Wd = delta@W2s^T (w2Ts reused from SBUF) -> AllReduce [Wd | d_sq]
  phase S3 (data-parallel): t = W1T^T(m2*Wd), G2 = A2 C A2^T, LDLT2,
           fwd solve, d_proj_sq, recon -> out (8 per core)

Identities replacing eigvalsh/cholesky/solve_triangular:
  Prec = Lt D Lt^T (unit-lower LDLT)
  sum(log eig)/2 = 0.5*sum(log D);   sum(1/eig) = ||D^-1/2 Lt^-1||_F^2
  U^-1 eps = Lt^-T (eps/sqrt(D));    t^T G2^-1 t = ||D2^-1/2 Lt2^-1 t||^2
  sig_term = (n w0 w0^T + (D-n) w1 w1^T)/2   (constant across batch)
"""
import sys

for _p in ("/opt/trn_rl_repo", "/root/.axon_site/_ro/trn_rl_repo"):
    if _p not in sys.path:
        sys.path.append(_p)

import numpy as np
import ml_dtypes
from contextlib import ExitStack

import concourse.bass as bass
import concourse.mybir as mybir
import concourse.tile as tile
from concourse.masks import make_identity

B, D, N, H = 64, 12288, 32, 2048
NCORES = 8
BL = B // NCORES          # 8 local samples
HS = H // NCORES          # 256
DS = D // NCORES          # 1536
KT_H = H // 128           # 16
KT_D = D // 128           # 96
KT_DS = DS // 128         # 12
P = 128
# upper-triangular C slab layout: col-block q holds block-rows 0..4q+3
CUP_OFF = [0, 4, 12, 24]          # slab index offset per col-block
CUP_NROW = [4, 8, 12, 16]         # slabs per col-block
CUP_TOT = 40
ZROWS = 4                         # z* partial occupies cup rows 0..3 (4x512)

F32 = mybir.dt.float32
BF16 = mybir.dt.bfloat16
FP8 = mybir.dt.float8e4
W2SC = 8.0            # fp8 scale on w2Ts; C/G carry W2SC^2, Wd carries W2SC
DR = mybir.MatmulPerfMode.DoubleRow
Alu = mybir.AluOpType
Act = mybir.ActivationFunctionType
RG = [list(range(NCORES))]


def sub_ap(t, extra_off, dims):
    """Custom free-dim AP on a [P, F] tile; dims = [[step,count],...] in elems."""
    base = t[:, 0:1]
    return bass.AP(base.tensor, base.offset + extra_off, [base.ap[0]] + dims)


def pe_T(nc, out_ps, in_ap, ident):
    """PE transpose: out_ps [f, p] = in_ap [p, f].T"""
    kp = in_ap.shape[0]
    nc.tensor.transpose(out_ps, in_ap, ident[0:kp, 0:kp])


def emit_ldlt(nc, T, OUT, invD, n=32):
    """In-place unit-lower LDLT of T [BL, n*n] (row-major per sample).
    After: strict lower of T holds unscaled columns u; diag holds D; invD = 1/D."""
    for j in range(n):
        nc.vector.reciprocal(invD[:, j:j + 1], T[:, (n + 1) * j:(n + 1) * j + 1])
        m = n - 1 - j
        if m == 0:
            break
        base = (j + 1) * n + j
        u_i = sub_ap(T, base, [[n, m], [0, m]])
        u_k = sub_ap(T, base, [[0, m], [n, m]])
        outer = sub_ap(OUT, 0, [[m, m], [1, m]])
        nc.vector.scalar_tensor_tensor(
            outer, u_i, invD[:, j:j + 1], u_k, Alu.mult, Alu.mult)
        trail = sub_ap(T, (j + 1) * (n + 1), [[n, m], [1, m]])
        nc.vector.tensor_tensor(trail, trail, outer, Alu.subtract)


def emit_ltinv(eng, LT, X, OUT, n=32):
    """X = LT^{-1} for unit-lower LT [BL, n*n]; X preset to I by caller.
    Uses only tensor_tensor (runs on GpSimd, whose ISA lacks STT)."""
    for k in range(n - 1):
        rows = n - 1 - k
        cols = k + 1
        lcol = sub_ap(LT, (k + 1) * n + k, [[n, rows], [0, cols]])
        xrow = sub_ap(X, k * n, [[0, rows], [1, cols]])
        prod = sub_ap(OUT, 0, [[cols, rows], [1, cols]])
        eng.tensor_tensor(prod, lcol, xrow, Alu.mult)
        xblk = sub_ap(X, (k + 1) * n, [[n, rows], [1, cols]])
        eng.tensor_tensor(xblk, xblk, prod, Alu.subtract)


def emit_bwd_solve(nc, LT, y, OUT, n=32):
    """y <- LT^{-T} y for unit-lower LT [BL, n*n], y [BL, n] in place."""
    for k in range(n - 1, 0, -1):
        lrow = sub_ap(LT, k * n, [[1, k]])
        nc.vector.scalar_tensor_tensor(
            OUT[:, 0:k], lrow, -1.0, y[:, k:k + 1].broadcast_to([BL, k]),
            Alu.mult, Alu.mult)
        nc.vector.tensor_tensor(y[:, 0:k], y[:, 0:k], OUT[:, 0:k], Alu.add)


def emit_fwd_solve(nc, LT, y, OUT, n=32):
    """y <- LT^{-1} y for unit-lower LT [BL, n*n], y [BL, n] in place."""
    for k in range(n - 1):
        rows = n - 1 - k
        lcol = sub_ap(LT, (k + 1) * n + k, [[n, rows]])
        nc.vector.scalar_tensor_tensor(
            OUT[:, 0:rows], lcol, -1.0, y[:, k:k + 1].broadcast_to([BL, rows]),
            Alu.mult, Alu.mult)
        nc.vector.tensor_tensor(y[:, k + 1:n], y[:, k + 1:n], OUT[:, 0:rows], Alu.add)


def legalize_waits(nc, maxw=1):
    """Split multi-wait sync_info into standalone EventSemaphore instructions."""
    for f in nc.m.functions:
        for bb in f.blocks:
            insts = list(bb.instructions)
            out = []
            changed = False
            for inst in insts:
                si = inst.sync_info
                if si is not None and si.on_wait and len(si.on_wait) > maxw:
                    waits = list(si.on_wait)
                    imm = [w for w in waits if w.uses_immediate]
                    reg = [w for w in waits if not w.uses_immediate]
                    keep = (reg + imm)[:maxw] if len(reg) <= maxw else reg
                    extra = [w for w in waits if w not in keep]
                    if len(keep) > maxw:
                        raise RuntimeError(f"{inst.name}: {len(keep)} register waits")
                    for w in extra:
                        ev = mybir.InstEventSemaphore(
                            name=nc.get_next_instruction_name(), ins=[], outs=[])
                        ev.engine = inst.engine
                        ev.sync_info = mybir.SyncInfo(on_wait=[w], on_update=[])
                        out.append(ev)
                    inst.sync_info = mybir.SyncInfo(
                        on_wait=keep, on_update=list(si.on_update or []))
                    changed = True
                out.append(inst)
            if changed:
                bb.instructions = out
    return nc


def build_nc():
    nc = bass.Bass()

    # ---- I/O ----
    # xTp/w1esp: pre-packed partition-major [(p) (k b)] so DMA descriptors are
    # per-partition contiguous (12KB / 6KB) instead of 128B/512B strided
    xTp = nc.dram_tensor("xTp", [P, KT_D * B], BF16, kind="ExternalInput")
    xmb = nc.dram_tensor("xmb", [B, DS], BF16, kind="ExternalInput")
    w1esp = nc.dram_tensor("w1esp", [P, KT_D * HS], BF16, kind="ExternalInput")
    b1es = nc.dram_tensor("b1es", [1, HS], BF16, kind="ExternalInput")
    w2es = nc.dram_tensor("w2es", [HS, N], BF16, kind="ExternalInput")
    b2e = nc.dram_tensor("b2e", [1, N], BF16, kind="ExternalInput")
    w2Ts = nc.dram_tensor("w2Ts", [DS, H], FP8, kind="ExternalInput")
    w2s = nc.dram_tensor("w2s", [H, DS], FP8, kind="ExternalInput")
    w1 = nc.dram_tensor("w1", [N, H], BF16, kind="ExternalInput")
    w1Td = nc.dram_tensor("w1Td", [H, N], BF16, kind="ExternalInput")
    b1d = nc.dram_tensor("b1d", [1, H], BF16, kind="ExternalInput")
    sigw = nc.dram_tensor("sigw", [1, 130], F32, kind="ExternalInput")
    sel8 = nc.dram_tensor("sel8", [B, BL], BF16, kind="ExternalInput")
    epsin = nc.dram_tensor("epsin", [BL, N], F32, kind="ExternalInput")
    out = nc.dram_tensor("out", [BL, 1], F32, kind="ExternalOutput")

    # ---- internal DRAM ----
    # chunk q: upper-C col-block q slabs (+ z* partial rows in chunk 0).
    # all chunks bf16 (fp8 collectives produce NaN on this stack)
    CUPDT = [BF16, BF16, BF16, BF16]
    cup = [nc.dram_tensor(f"cup{q}", [(ZROWS if q == 0 else 0) + CUP_NROW[q] * P, 512],
                          CUPDT[q]) for q in range(4)]
    cup_sh = [nc.dram_tensor(f"cup_sh{q}", [(ZROWS if q == 0 else 0) + CUP_NROW[q] * P, 512],
                             CUPDT[q], addr_space="Shared") for q in range(4)]
    zs_b = nc.dram_tensor("zs_b", [BL, N], F32)
    zs_sh = nc.dram_tensor("zs_sh", [B, N], F32, addr_space="Shared")
    wd_b = nc.dram_tensor("wd_b", [B, H], BF16)
    wds_b = nc.dram_tensor("wds_b", [BL, H], BF16)
    dsq_b = nc.dram_tensor("dsq_b", [B, 1], F32)
    dsqs_b = nc.dram_tensor("dsqs_b", [BL, 1], F32)

    with tile.TileContext(nc) as tc, ExitStack() as ctx:
        consts = ctx.enter_context(tc.tile_pool(name="consts", bufs=1))
        work = ctx.enter_context(tc.tile_pool(name="work", bufs=2))
        stream = ctx.enter_context(tc.tile_pool(name="stream", bufs=3))
        psum = ctx.enter_context(tc.tile_pool(name="psum", bufs=2, space="PSUM"))
        psum_acc = ctx.enter_context(tc.tile_pool(name="psacc", bufs=1, space="PSUM"))
        lin = ctx.enter_context(tc.tile_pool(name="lin", bufs=1))
        res = ctx.enter_context(tc.tile_pool(name="res", bufs=1))
        encs = ctx.enter_context(tc.tile_pool(name="encs", bufs=2))
        w2sp = ctx.enter_context(tc.tile_pool(name="w2sp", bufs=6))

        # ---- resident weights first: w2Ts fp8 x W2SC (phase C + Wd) ----
        # emitted ahead of the consts so the SP DMA queue starts on them
        w2Ts_sb = res.tile([P, KT_DS, H], FP8, tag="w2Ts")
        w2Ts_r = w2Ts[:].rearrange("(k p) h -> p k h", p=P)
        for kt in range(KT_DS):
            nc.sync.dma_start(w2Ts_sb[:, kt, :], w2Ts_r[:, kt, :])

        # ---- constants / small loads ----
        identb = consts.tile([P, P], BF16)
        make_identity(nc, identb)
        ones1 = consts.tile([1, B], F32)
        nc.vector.memset(ones1, 1.0)
        onesb = consts.tile([1, B], BF16)
        nc.vector.memset(onesb, 1.0)
        sigw_sb = consts.tile([1, 130], F32)
        nc.sync.dma_start(sigw_sb, sigw[:])
        sigw_rep = consts.tile([BL, 130], F32)
        sigw_ps = psum.tile([BL, 130], F32, tag="small_ps")
        nc.tensor.matmul(sigw_ps, ones1[:, 0:BL], sigw_sb, start=True, stop=True)
        nc.vector.tensor_copy(sigw_rep, sigw_ps)
        sel8_sb = consts.tile([B, BL], BF16)
        nc.sync.dma_start(sel8_sb, sel8[:])
        eps_sb = consts.tile([BL, N], F32)
        nc.sync.dma_start(eps_sb, epsin[:])
        b1es_sb = consts.tile([1, HS], BF16)
        nc.sync.dma_start(b1es_sb, b1es[:])
        b2e_sb = consts.tile([1, N], BF16)
        nc.sync.dma_start(b2e_sb, b2e[:])
        # decoder bias as per-partition columns [P, KT_H] (+ negated copy)
        b1dcol = consts.tile([P, KT_H], BF16)
        nc.sync.dma_start(b1dcol, b1d[:].rearrange("o (k p) -> p (o k)", p=P))
        nb1col = consts.tile([P, KT_H], F32)
        nc.vector.tensor_scalar(nb1col, b1dcol, -1.0, None, Alu.mult)
        w1_sb = consts.tile([N, H], BF16)
        nc.sync.dma_start(w1_sb, w1[:])
        w1T_sb = consts.tile([P, KT_H, N], BF16)
        nc.sync.dma_start(w1T_sb, w1Td[:].rearrange("(k p) n -> p k n", p=P))

        # ================= phase E: encoder (z* partial -> cup rows 0:4) ====
        a1_ps = psum_acc.tile([B, HS], F32, tag="acc")
        KSUP = 8  # k-tiles per packed super-chunk
        for kc in range(KT_D // KSUP):
            xp_t = encs.tile([P, KSUP, B], BF16, tag="xp_t")
            nc.sync.dma_start(
                xp_t, xTp[:, kc * KSUP * B:(kc + 1) * KSUP * B]
                .rearrange("p (k b) -> p k b", b=B))
            w1t = encs.tile([P, KSUP, HS], BF16, tag="w1es_t")
            nc.sync.dma_start(
                w1t, w1esp[:, kc * KSUP * HS:(kc + 1) * KSUP * HS]
                .rearrange("p (k h) -> p k h", h=HS))
            for kj in range(KSUP):
                kt = kc * KSUP + kj
                nc.tensor.matmul(a1_ps, xp_t[:, kj, :], w1t[:, kj, :],
                                 start=(kt == 0), stop=False)
        nc.tensor.matmul(a1_ps, onesb[:, 0:B], b1es_sb, start=False, stop=True)
        h1_sb = work.tile([B, HS], BF16, tag="h1")
        nc.vector.tensor_scalar(h1_sb, a1_ps, 0.0, None, Alu.max)
        h1T_sb = work.tile([P, 2, B], BF16, tag="h1T")
        for i in range(2):
            tp = psum.tile([P, B], BF16, tag="t_ps")
            pe_T(nc, tp, h1_sb[:, i * P:(i + 1) * P], identb)
            nc.vector.tensor_copy(h1T_sb[:, i, :], tp)
        w2es_sb = work.tile([P, 2, N], BF16, tag="w2es")
        nc.sync.dma_start(w2es_sb, w2es[:].rearrange("(k p) n -> p k n", p=P))
        zp_ps = psum.tile([B, N], F32, tag="small_ps")
        for i in range(2):
            nc.tensor.matmul(zp_ps, h1T_sb[:, i, :], w2es_sb[:, i, :],
                             start=(i == 0), stop=(i == 1))
        zp_sb = work.tile([B, N], BF16, tag="zstar_part")
        nc.vector.tensor_copy(zp_sb, zp_ps)
        nc.sync.dma_start(cup[0][0:ZROWS, :], zp_sb)

        # ========= phase C: upper C slabs + chunked AllReduce ==============
        # col-block q: cols [512q, 512q+512), block-rows j in 0..4q+3
        csb = res.tile([P, KT_H, H], BF16, tag="csb")
        for q in range(4):
            zr = ZROWS if q == 0 else 0
            for jg in range(q + 1):          # groups of 4 slabs
                cs = work.tile([P, 4, 512], CUPDT[q],
                               tag="c_out" if q == 0 else "c_out8")
                for jj in range(4):
                    j = jg * 4 + jj
                    cps = psum.tile([P, 512], F32, tag="big_ps")
                    for kd in range(KT_DS // 2):
                        nc.tensor.matmul(
                            cps,
                            w2Ts_sb[:, 2 * kd:2 * kd + 2, j * P:(j + 1) * P],
                            w2Ts_sb[:, 2 * kd:2 * kd + 2, 512 * q:512 * (q + 1)],
                            start=(kd == 0), stop=(kd == KT_DS // 2 - 1),
                            perf_mode=DR)
                    nc.scalar.copy(cs[:, jj, :], cps)
                r0 = zr + jg * 4 * P
                nc.sync.dma_start(
                    cup[q][r0:r0 + 4 * P, :].rearrange("(s p) c -> p s c", p=P),
                    cs)
            # AllReduce this chunk (chunk 0 also carries the z* partial)
            nc.gpsimd.collective_compute(
                "AllReduce", Alu.add, replica_groups=RG,
                ins=[cup[q][:]], outs=[cup_sh[q][:]])
            # readback on the gpsimd DMA queue right behind the AR so it
            # lands as soon as the chunk is reduced (SP queue is busy with
            # input streams / cup writes)
            if q == 0:
                zf_sb = work.tile([B, N], BF16, tag="z_full")
                nc.gpsimd.dma_start(zf_sb, cup_sh[0][0:ZROWS, :])
            zr_ = ZROWS if q == 0 else 0
            shr = cup_sh[q][zr_:, :].rearrange("(s p) c -> p s c", p=P)
            nc.gpsimd.dma_start(
                csb[:, 0:4 * q + 4, 512 * q:512 * (q + 1)], shr)

        # ---- z* post: local slice, sig1, masks, A1T (overlaps C build) ----
        zlT_ps = psum.tile([N, BL], F32, tag="small_ps")
        nc.tensor.matmul(zlT_ps, zf_sb, sel8_sb, start=True, stop=False)
        nc.tensor.matmul(zlT_ps, b2e_sb, onesb[:, 0:BL], start=False, stop=True)
        zlT_sb = work.tile([N, BL], BF16, tag="zlT")   # (z*loc + b2)^T
        nc.vector.tensor_copy(zlT_sb, zlT_ps)
        zloc_ps = psum.tile([BL, N], F32, tag="small_ps")
        nc.tensor.matmul(zloc_ps, sel8_sb, zf_sb, start=True, stop=False)
        nc.tensor.matmul(zloc_ps, onesb[:, 0:BL], b2e_sb, start=False, stop=True)
        zloc_sb = lin.tile([BL, N], F32, tag="z_loc")   # z* local + b2
        nc.vector.tensor_copy(zloc_sb, zloc_ps)

        def emit_sig(z_loc, name):
            lg = lin.tile([BL, 2, 32], F32, tag="sig_lg")
            nc.vector.tensor_tensor(
                lg, z_loc.unsqueeze(1).broadcast_to([BL, 2, 32]),
                sigw_rep[:, 0:64].rearrange("p (c n) -> p c n", c=2), Alu.mult)
            red = lin.tile([BL, 2], F32, tag=f"sig_red_{name}")
            nc.vector.tensor_reduce(red, lg, mybir.AxisListType.X, Alu.add)
            nc.vector.tensor_tensor(red, red, sigw_rep[:, 64:66], Alu.add)
            s = lin.tile([BL, 2], F32, tag=f"sig_s_{name}")
            nc.scalar.activation(s, red, Act.Exp)
            return s

        s1 = emit_sig(zloc_sb, "s1")
        invsp2 = lin.tile([BL, 1], F32, tag="invsp2")
        sp2t = lin.tile([BL, 1], F32, tag="sp2t")
        nc.vector.tensor_tensor(sp2t, s1[:, 0:1], s1[:, 0:1], Alu.mult)
        # G arrives scaled by W2SC^2 (fp8 weights); fold 1/W2SC^2 in here
        nc.vector.tensor_scalar(sp2t, sp2t, W2SC * W2SC, None, Alu.mult)
        nc.vector.reciprocal(invsp2, sp2t)

        # a1T (local) -> mask m1T [P, KT_H, BL] -> A1T  (mask: a1 > -b1)
        m1T_sb = work.tile([P, KT_H, BL], BF16, tag="m1T")
        for mt in range(KT_H):
            aps = psum.tile([P, BL], F32, tag="small_ps")
            nc.tensor.matmul(aps, w1_sb[:, mt * P:(mt + 1) * P],
                             zlT_sb, start=True, stop=True)
            nc.vector.tensor_tensor(
                m1T_sb[:, mt, :], aps,
                nb1col[:, mt:mt + 1].broadcast_to([P, BL]), Alu.is_gt)
        AT_sb = res.tile([P, KT_H, BL, N], BF16, tag="AT")
        nc.vector.tensor_tensor(
            AT_sb,
            w1T_sb.unsqueeze(2).broadcast_to([P, KT_H, BL, N]),
            m1T_sb.unsqueeze(3).broadcast_to([P, KT_H, BL, N]), Alu.mult)

        # ---- chunk-pipelined P1T = C @ A1T: each col-chunk's mirrors and PT
        # contributions run as soon as its AllReduce lands (PE overlaps ARs) --
        PT1 = res.tile([P, KT_H, BL * N], BF16, tag="PT")
        for q in range(4):
            # mirrors sourced from chunk q: targets (i in stripe q, j < 4q)
            for i in range(4 * q, 4 * q + 4):
                for j in range(4 * q):
                    tp = psum.tile([P, P], BF16, tag="t_ps")
                    pe_T(nc, tp, csb[:, j, i * P:(i + 1) * P], identb)
                    nc.scalar.copy(csb[:, i, j * P:(j + 1) * P], tp)
            # (a) fold the new kt-stripe q into PT[mt] for mt < 4q
            for mt in range(4 * q):
                pps = psum.tile([P, BL * N], F32, tag="big_ps")
                for kt in range(4 * q, 4 * q + 4):
                    nc.tensor.matmul(pps, csb[:, kt, mt * P:(mt + 1) * P],
                                     AT_sb[:, kt, :, :],
                                     start=(kt == 4 * q), stop=(kt == 4 * q + 3))
                nc.vector.tensor_tensor(PT1[:, mt, :], PT1[:, mt, :], pps,
                                        Alu.add)
            # (b) initialize PT[mt] for mt in stripe q (kt 0..4q+3 available)
            for mt in range(4 * q, 4 * q + 4):
                pps = psum.tile([P, BL * N], F32, tag="big_ps")
                for kt in range(4 * q + 4):
                    nc.tensor.matmul(pps, csb[:, kt, mt * P:(mt + 1) * P],
                                     AT_sb[:, kt, :, :],
                                     start=(kt == 0), stop=(kt == 4 * q + 3))
                nc.scalar.copy(PT1[:, mt, :], pps)



        # ---- P*T = C @ A*T ; G = P*T^T A*T  (C resident in SBUF) ----
        def emit_PG(AT, tag):
            PT_sb = res.tile([P, KT_H, BL * N], BF16, tag="PT")
            for mt in range(KT_H):
                pps = psum.tile([P, BL * N], F32, tag="big_ps")
                for kt in range(KT_H):
                    nc.tensor.matmul(
                        pps, csb[:, kt, mt * P:(mt + 1) * P], AT[:, kt, :, :],
                        start=(kt == 0), stop=(kt == KT_H - 1))
                nc.scalar.copy(PT_sb[:, mt, :], pps)
            g_sb = work.tile([P, 2, P], F32, tag="g_sb")
            for grp in range(2):
                g_ps = psum.tile([P, P], F32, tag="big_ps")
                for kt in range(KT_H):
                    nc.tensor.matmul(
                        g_ps,
                        PT_sb[:, kt, grp * P:(grp + 1) * P],
                        AT[:, kt, 4 * grp:4 * grp + 4, :],
                        start=(kt == 0), stop=(kt == KT_H - 1))
                nc.vector.tensor_copy(g_sb[:, grp, :], g_ps)
            return g_sb

        # ---- Prec = G*invsp2 + sig_term + I ----
        # preset Tm with replicated sig_term while PG runs
        st_ps = psum.tile([N, N], F32, tag="small_ps")
        nc.tensor.matmul(st_ps, sigw_sb[:, 66:98], sigw_sb[:, 66:98],
                         start=True, stop=False)
        nc.tensor.matmul(st_ps, sigw_sb[:, 98:130], sigw_sb[:, 98:130],
                         start=False, stop=True)
        st_sb = work.tile([N, N], F32, tag="st_sb")
        nc.vector.tensor_copy(st_sb, st_ps)
        Tm = lin.tile([BL, N * N], F32, tag="Tmat")
        for s in range(BL):
            nc.sync.dma_start(Tm[s:s + 1, :], st_sb)
        # add diag I up front
        diag1 = sub_ap(Tm, 0, [[N + 1, N]])
        nc.vector.tensor_scalar(diag1, diag1, 1.0, None, Alu.add)

        g_sb = work.tile([P, 2, P], F32, tag="g_sb")
        for grp in range(2):
            g_ps = psum.tile([P, P], F32, tag="big_ps")
            for kt in range(KT_H):
                nc.tensor.matmul(
                    g_ps, PT1[:, kt, grp * P:(grp + 1) * P],
                    AT_sb[:, kt, 4 * grp:4 * grp + 4, :],
                    start=(kt == 0), stop=(kt == KT_H - 1))
            nc.vector.tensor_copy(g_sb[:, grp, :], g_ps)
        SCR = lin.tile([BL, N * N], F32, tag="scr")
        for s in range(BL):
            grp, sl = s // 4, s % 4
            nc.sync.dma_start(
                SCR[s:s + 1, :],
                g_sb[sl * N:(sl + 1) * N, grp, sl * N:(sl + 1) * N])
        nc.vector.scalar_tensor_tensor(Tm, SCR, invsp2, Tm, Alu.mult, Alu.add)

        # ---- LDLT, dz (backward solve; Lt^-1/tr deferred off critical path) ----
        invD = lin.tile([BL, N], F32, tag="invD")
        emit_ldlt(nc, Tm, SCR, invD)
        LT = lin.tile([BL, N * N], F32, tag="LTmat")
        nc.vector.tensor_tensor(
            LT.rearrange("p (a b) -> p a b", b=N),
            Tm.rearrange("p (a b) -> p a b", b=N),
            invD.unsqueeze(1).broadcast_to([BL, N, N]), Alu.mult)
        srD = lin.tile([BL, N], F32, tag="srD")
        nc.scalar.activation(srD, invD, Act.Sqrt)        # 1/sqrt(D)
        epss = lin.tile([BL, N], F32, tag="epss")
        nc.vector.tensor_tensor(epss, eps_sb, srD, Alu.mult)
        emit_bwd_solve(nc, LT, epss, SCR)                # epss <- Lt^-T epss = dz
        zs_loc = lin.tile([BL, N], F32, tag="zs_loc")
        nc.vector.tensor_tensor(zs_loc, zloc_sb, epss, Alu.add)
        nc.sync.dma_start(zs_b[:], zs_loc)
        nc.gpsimd.collective_compute("AllGather", Alu.bypass, replica_groups=RG,
                                     ins=[zs_b[:]], outs=[zs_sh[:]])

        # ---- tr(Prec^-1) via Lt^-1 on GpSimd (parallel with stage 2) ----
        X1 = lin.tile([BL, N * N], F32, tag="X1")
        nc.gpsimd.memset(X1, 0.0)
        nc.gpsimd.memset(sub_ap(X1, 0, [[N + 1, N]]), 1.0)
        gSCR = lin.tile([BL, N * N], F32, tag="gSCR")
        emit_ltinv(nc.gpsimd, LT, X1, gSCR)
        trv = lin.tile([BL, 1], F32, tag="trv")
        nc.gpsimd.tensor_tensor(
            gSCR.rearrange("p (a b) -> p a b", b=N),
            X1.rearrange("p (a b) -> p a b", b=N),
            invD.unsqueeze(2).broadcast_to([BL, N, N]), Alu.mult)
        nc.gpsimd.tensor_tensor(gSCR, gSCR, X1, Alu.mult)
        # final free-axis reduce of gSCR into trv happens on vector at the tail

        # ---- z*-only reductions (vector, overlap AllGather) ----
        logs = lin.tile([BL, N], F32, tag="logs")
        ldv = lin.tile([BL, 1], F32, tag="ldv")
        nc.scalar.activation(logs, invD, Act.Ln)
        nc.vector.tensor_reduce(ldv, logs, mybir.AxisListType.X, Alu.add)  # -sum log D
        nc.vector.tensor_scalar(ldv, ldv, -0.5, None, Alu.mult)
        zsq = lin.tile([BL, N], F32, tag="zsq")
        latv = lin.tile([BL, 1], F32, tag="latv")
        nc.vector.tensor_tensor(zsq, zloc_sb, zloc_sb, Alu.mult)
        nc.vector.tensor_reduce(latv, zsq, mybir.AxisListType.X, Alu.add)
        # s2-dependent scalars (zs_loc known before AG returns)
        s2 = emit_sig(zs_loc, "s2")
        sq2 = lin.tile([BL, 2], F32, tag="sq2")
        nc.vector.tensor_tensor(sq2, s2, s2, Alu.mult)
        nc.vector.tensor_scalar(sq2, sq2, 2.0, None, Alu.mult)
        inv2 = lin.tile([BL, 2], F32, tag="inv2")
        nc.vector.reciprocal(inv2, sq2)     # [1/(2sp2^2), 1/(2sv2^2)]
        logs2 = lin.tile([BL, 2], F32, tag="logs2")
        logw = lin.tile([BL, 2], F32, tag="logw")
        nc.vector.memset(logw[:, 0:1], float(N))
        nc.vector.memset(logw[:, 1:2], float(D - N))
        nc.scalar.activation(logs2, s2, Act.Ln)
        logterm = lin.tile([BL, 1], F32, tag="logterm")
        junk2 = lin.tile([BL, 2], F32, tag="junk2")
        nc.vector.tensor_tensor(junk2, logs2, logw, Alu.mult)
        nc.vector.tensor_reduce(logterm, junk2, mybir.AxisListType.X, Alu.add)
        isub = lin.tile([BL, 1], F32, tag="isub")
        nc.vector.tensor_tensor(isub, inv2[:, 0:1], inv2[:, 1:2], Alu.subtract)

        # ---- stage 2 prep: h2T (all), m2T (local), A2T ----
        zsf_sb = work.tile([B, N], F32, tag="z_full2")
        nc.sync.dma_start(zsf_sb, zs_sh[:])
        zsf_bf = work.tile([B, N], BF16, tag="z_full2b")
        nc.vector.tensor_copy(zsf_bf, zsf_sb)
        zs_bf = lin.tile([BL, N], BF16, tag="zs_locb")
        nc.vector.tensor_copy(zs_bf, zs_loc)
        zsT_ps = psum.tile([N, B], BF16, tag="t_ps")
        pe_T(nc, zsT_ps, zsf_bf, identb)
        zsT_sb = work.tile([N, B], BF16, tag="zT2")
        nc.vector.tensor_copy(zsT_sb, zsT_ps)
        zslT_ps = psum.tile([N, BL], BF16, tag="t_ps")
        pe_T(nc, zslT_ps, zs_bf, identb)
        zslT_sb = work.tile([N, BL], BF16, tag="zlT2")
        nc.vector.tensor_copy(zslT_sb, zslT_ps)

        h2T_sb = res.tile([P, KT_H, B], FP8, tag="h2T")
        for mt in range(KT_H):
            aps = psum.tile([P, B], F32, tag="small_ps")
            nc.tensor.matmul(aps, w1_sb[:, mt * P:(mt + 1) * P],
                             zsT_sb, start=True, stop=True)
            nc.scalar.activation(h2T_sb[:, mt, :], aps, Act.Relu,
                                 bias=b1dcol[:, mt:mt + 1])

        m2T_sb = work.tile([P, KT_H, BL], BF16, tag="m2T")
        for mt in range(KT_H):
            aps = psum.tile([P, BL], F32, tag="small_ps")
            nc.tensor.matmul(aps, w1_sb[:, mt * P:(mt + 1) * P],
                             zslT_sb, start=True, stop=True)
            nc.vector.tensor_tensor(
                m2T_sb[:, mt, :], aps,
                nb1col[:, mt:mt + 1].broadcast_to([P, BL]), Alu.is_gt)
        AT2_sb = res.tile([P, KT_H, BL, N], BF16, tag="AT")   # reuse slot
        nc.vector.tensor_tensor(
            AT2_sb,
            w1T_sb.unsqueeze(2).broadcast_to([P, KT_H, BL, N]),
            m2T_sb.unsqueeze(3).broadcast_to([P, KT_H, BL, N]), Alu.mult)

        # ---- G2 on PE first: fills PE while the w2s stream for x_star runs --
        g2_sb = emit_PG(AT2_sb, "2")
        Tm2 = lin.tile([BL, N * N], F32, tag="Tmat")   # reuse slot
        for s in range(BL):
            grp, sl = s // 4, s % 4
            nc.sync.dma_start(
                Tm2[s:s + 1, :],
                g2_sb[sl * N:(sl + 1) * N, grp, sl * N:(sl + 1) * N])
        # Jacobi weights for the Richardson solve (no factorization needed)
        dg2 = lin.tile([BL, N], F32, tag="dg2")
        nc.vector.tensor_copy(dg2, sub_ap(Tm2, 0, [[N + 1, N]]))
        widg = lin.tile([BL, N], F32, tag="widg")
        nc.vector.reciprocal(widg, dg2)
        nc.vector.tensor_scalar(widg, widg, 0.9, None, Alu.mult)

        # ---- x_star slice, delta, d_sq, Wd (fp8 DoubleRow; d_sb holds
        # NEGATED delta x W2SC-descale folded in -- delta only enters
        # quadratically (d_sq, t^T G^-1 t) so the sign is irrelevant ----
        d_sb = res.tile([B, DS], BF16, tag="d_sb")
        w2s_r = w2s[:].rearrange("(k p) ds -> p k ds", p=P)
        for nb in range(3):
            xmb_t = stream.tile([B, 512], BF16, tag="xmb_t")
            nc.sync.dma_start(xmb_t, xmb[:, nb * 512:(nb + 1) * 512])
            xs_ps = psum.tile([B, 512], F32, tag="big_ps")
            for kd in range(KT_H // 2):
                wt = w2sp.tile([P, 2, 512], FP8, tag="w2s_t")
                nc.sync.dma_start(
                    wt, w2s_r[:, 2 * kd:2 * kd + 2, nb * 512:(nb + 1) * 512])
                nc.tensor.matmul(xs_ps, h2T_sb[:, 2 * kd:2 * kd + 2, :], wt,
                                 start=(kd == 0), stop=(kd == KT_H // 2 - 1),
                                 perf_mode=DR)
            nc.vector.scalar_tensor_tensor(
                d_sb[:, nb * 512:(nb + 1) * 512], xs_ps, 1.0 / W2SC, xmb_t,
                Alu.mult, Alu.subtract)
        dsq_sb = work.tile([B, 1], F32, tag="dsq")
        dsqf = res.tile([B, DS], BF16, tag="dsqf")
        nc.scalar.activation(dsqf, d_sb, Act.Square, accum_out=dsq_sb)
        nc.sync.dma_start(dsq_b[:], dsq_sb)
        nc.gpsimd.collective_compute("ReduceScatter", Alu.add, replica_groups=RG,
                                     ins=[dsq_b[:]], outs=[dsqs_b[:]])
        dT_bf = res.tile([P, KT_DS, B], BF16, tag="dTb")
        for kt in range(KT_DS):
            tp = psum.tile([P, B], BF16, tag="t_ps")
            pe_T(nc, tp, d_sb[:, kt * P:(kt + 1) * P], identb)
            nc.vector.tensor_copy(dT_bf[:, kt, :], tp)
        dT8 = res.tile([P, KT_DS, B], FP8, tag="dT")
        nc.scalar.copy(dT8, dT_bf)
        wd_sb = res.tile([B, H], BF16, tag="wd")
        for mb in range(4):
            wd_ps = psum.tile([B, 512], F32, tag="big_ps")
            for kd in range(KT_DS // 2):
                nc.tensor.matmul(wd_ps, dT8[:, 2 * kd:2 * kd + 2, :],
                                 w2Ts_sb[:, 2 * kd:2 * kd + 2,
                                         mb * 512:(mb + 1) * 512],
                                 start=(kd == 0), stop=(kd == KT_DS // 2 - 1),
                                 perf_mode=DR)
            nc.vector.tensor_scalar(wd_sb[:, mb * 512:(mb + 1) * 512], wd_ps,
                                    1.0 / W2SC, None, Alu.mult)
        nc.sync.dma_start(wd_b[:], wd_sb)
        nc.gpsimd.collective_compute("ReduceScatter", Alu.add, replica_groups=RG,
                                     ins=[wd_b[:]], outs=[wds_b[:]])

        # ---- local Wd/dsq arrive directly via ReduceScatter ----
        dsql = lin.tile([BL, 1], F32, tag="dsql")
        nc.sync.dma_start(dsql, dsqs_b[:])
        wdl_bf = res.tile([BL, H], BF16, tag="wd_locb")
        nc.gpsimd.dma_start(wdl_bf, wds_b[:])
        wdlT_sb = work.tile([P, KT_H, BL], BF16, tag="wdlT")
        for kt in range(KT_H):
            tp2 = psum.tile([P, BL], BF16, tag="t_ps")
            pe_T(nc, tp2, wdl_bf[:, kt * P:(kt + 1) * P], identb)
            nc.vector.tensor_copy(wdlT_sb[:, kt, :], tp2)
        mwdT_sb = work.tile([P, KT_H, BL], BF16, tag="mwdT")
        nc.vector.tensor_tensor(mwdT_sb, wdlT_sb, m2T_sb, Alu.mult)
        # y[bl, n] = sum_h mwdT[h, bl] * w1T[h, n]  (t, already transposed)
        y_ps = psum.tile([BL, N], F32, tag="small_ps")
        for kt in range(KT_H):
            nc.tensor.matmul(y_ps, mwdT_sb[:, kt, :], w1T_sb[:, kt, :],
                             start=(kt == 0), stop=(kt == KT_H - 1))
        y = lin.tile([BL, N], F32, tag="y")
        nc.vector.tensor_copy(y, y_ps)
        # ---- solve G2 x = y by Jacobi-damped Richardson ----
        xs = lin.tile([BL, N], F32, tag="xs")
        gx = lin.tile([BL, N], F32, tag="gx")
        tmpv = lin.tile([BL, N], F32, tag="tmpv")
        nc.vector.tensor_tensor(xs, y, widg, Alu.mult)
        for _ in range(3):
            nc.vector.tensor_tensor(
                SCR.rearrange("p (a b) -> p a b", b=N),
                Tm2.rearrange("p (a b) -> p a b", b=N),
                xs.unsqueeze(1).broadcast_to([BL, N, N]), Alu.mult)
            nc.vector.tensor_reduce(
                gx, SCR.rearrange("p (a b) -> p a b", b=N),
                mybir.AxisListType.X, Alu.add)
            nc.vector.tensor_tensor(tmpv, y, gx, Alu.subtract)
            nc.vector.tensor_tensor(tmpv, tmpv, widg, Alu.mult)
            nc.vector.tensor_tensor(xs, xs, tmpv, Alu.add)
        yx = lin.tile([BL, N], F32, tag="yx")
        dproj = lin.tile([BL, 1], F32, tag="dproj")
        nc.vector.tensor_tensor(yx, y, xs, Alu.mult)
        nc.vector.tensor_reduce(dproj, yx, mybir.AxisListType.X, Alu.add)
        # Tm2 = W2SC^2 * G2, so x and hence dproj are 1/W2SC^2 scaled
        nc.vector.tensor_scalar(dproj, dproj, W2SC * W2SC, None, Alu.mult)
        nc.vector.tensor_reduce(trv, gSCR, mybir.AxisListType.X, Alu.add)

        # ---- recon / output (scalars precomputed during stage 2) ----
        recon = lin.tile([BL, 1], F32, tag="recon")
        nc.vector.tensor_tensor(recon, dproj, isub, Alu.mult)
        p2t = lin.tile([BL, 1], F32, tag="p2t")
        nc.vector.tensor_tensor(p2t, dsql, inv2[:, 1:2], Alu.mult)
        nc.vector.tensor_tensor(recon, recon, p2t, Alu.add)
        nc.vector.tensor_tensor(recon, recon, logterm, Alu.add)
        ov = lin.tile([BL, 1], F32, tag="ov")
        nc.vector.tensor_tensor(ov, latv, trv, Alu.add)
        nc.vector.tensor_scalar(ov, ov, 0.5, None, Alu.mult)
        nc.vector.tensor_tensor(ov, ov, recon, Alu.add)
        nc.vector.tensor_tensor(ov, ov, ldv, Alu.add)
        nc.vector.tensor_scalar(ov, ov, 1.0 / D, None, Alu.mult)
        nc.sync.dma_start(out[:], ov)

    legalize_waits(nc)
    return nc


def shard_inputs(inputs):
    """Host-side prep: returns in_maps list for the 8 cores."""
    bf = ml_dtypes.bfloat16
    x = np.ascontiguousarray(np.asarray(inputs["x"], np.float32))
    eps = np.ascontiguousarray(np.asarray(inputs["eps"], np.float32))
    eW1 = np.ascontiguousarray(np.asarray(inputs["enc_W1"], np.float32))
    eb1 = np.asarray(inputs["enc_b1"], np.float32)
    eW2 = np.ascontiguousarray(np.asarray(inputs["enc_W2"], np.float32))
    eb2 = np.asarray(inputs["enc_b2"], np.float32)
    dW1 = np.ascontiguousarray(np.asarray(inputs["dec_W1"], np.float32))
    db1 = np.asarray(inputs["dec_b1"], np.float32)
    dW2 = np.ascontiguousarray(np.asarray(inputs["dec_W2"], np.float32))
    db2 = np.asarray(inputs["dec_b2"], np.float32)
    sW = np.asarray(inputs["sig_W"], np.float32)
    sb = np.asarray(inputs["sig_b"], np.float32)

    xT = np.ascontiguousarray(x.T).astype(bf)
    xTp = np.ascontiguousarray(
        xT.reshape(KT_D, P, B).transpose(1, 0, 2).reshape(P, KT_D * B))
    dW2T = np.ascontiguousarray(dW2.T)
    dW1T = np.ascontiguousarray(dW1.T).astype(bf)
    dW1b = dW1.astype(bf)
    sigv = np.zeros((1, 130), np.float32)
    sigv[0, 0:32] = sW[:, 0]
    sigv[0, 32:64] = sW[:, 1]
    sigv[0, 64:66] = sb
    sigv[0, 66:98] = sW[:, 0] * np.sqrt(N / 2.0)
    sigv[0, 98:130] = sW[:, 1] * np.sqrt((D - N) / 2.0)

    maps = []
    for k in range(NCORES):
        sel = np.zeros((B, BL), np.float32)
        for i in range(BL):
            sel[k * BL + i, i] = 1.0
        w1s = np.ascontiguousarray(eW1[:, k * HS:(k + 1) * HS]).astype(bf)
        maps.append({
            "xTp": xTp,
            "xmb": np.ascontiguousarray(
                x[:, k * DS:(k + 1) * DS]
                - db2[None, k * DS:(k + 1) * DS]).astype(bf),
            "w1esp": np.ascontiguousarray(
                w1s.reshape(KT_D, P, HS).transpose(1, 0, 2)
                .reshape(P, KT_D * HS)),
            "b1es": np.ascontiguousarray(eb1[None, k * HS:(k + 1) * HS]).astype(bf),
            "w2es": np.ascontiguousarray(eW2[k * HS:(k + 1) * HS, :]).astype(bf),
            "b2e": np.ascontiguousarray(eb2[None, :]).astype(bf),
            "w2Ts": (np.ascontiguousarray(dW2T[k * DS:(k + 1) * DS, :]) * W2SC
                     ).astype(ml_dtypes.float8_e4m3fn),
            "w2s": (np.ascontiguousarray(dW2[:, k * DS:(k + 1) * DS]) * W2SC
                    ).astype(ml_dtypes.float8_e4m3fn),
            "w1": dW1b,
            "w1Td": dW1T,
            "b1d": np.ascontiguousarray(db1[None, :]).astype(bf),
            "sigw": sigv,
            "sel8": sel.astype(bf),
            "epsin": np.ascontiguousarray(eps[k * BL:(k + 1) * BL, :]),
        })
    return maps


_NC_CACHE = None


def kernel(**inputs) -> np.ndarray:
    global _NC_CACHE
    from concourse.bass_utils import run_bass_kernel_spmd
    if _NC_CACHE is None:
        _NC_CACHE = build_nc()
    nc = _NC_CACHE
    maps = shard_inputs(inputs)
    res = run_bass_kernel_spmd(nc, maps, list(range(NCORES)))
    outs = [res.results[k]["out"].reshape(BL) for k in range(NCORES)]
    return np.concatenate(outs).astype(np.float32)
